# revision 14
# baseline (speedup 1.0000x reference)
"""CapsuleNetwork Trainium2 Bass kernel.

8-core data-parallel: batch 128 -> 16 per core, all weights replicated.
Per-core pipeline (all on device):
  conv1 (im2col matmul) -> conv2 (tap-accumulation matmul) -> primary-caps
  squash -> dynamic routing (2 rounds, via block-diagonal matmul tricks)
  -> norm/argmax/mask -> 3-layer decoder.
"""

import os
import sys

import numpy as np

sys.path.insert(0, "/opt/trn_rl_repo")

import concourse.bass as bass
import concourse.bacc as bacc
import concourse.tile as tile
from concourse import mybir

try:
    import ml_dtypes

    BF16 = ml_dtypes.bfloat16
except ImportError:  # pragma: no cover
    BF16 = None

EPS = 1e-7
B_LOC = 16  # images per core
N_CORES = 8
F32 = mybir.dt.float32
F32R = mybir.dt.float32r
BF16D = mybir.dt.bfloat16
I32 = mybir.dt.int32
U32 = mybir.dt.uint32
AF = mybir.ActivationFunctionType
ALU = mybir.AluOpType
AX = mybir.AxisListType


def _ap(base, offset_elems, dims):
    """Raw AP at base.offset + offset_elems with explicit [step,count] dims."""
    return bass.AP(tensor=base.tensor, offset=base.offset + offset_elems, ap=dims)


def r32(ap):
    return ap.bitcast(F32R)


def build_nc(debug=False):
    nc = bacc.Bacc("TRN2", target_bir_lowering=False, debug=False)

    # ---------------- DRAM I/O ----------------
    img = nc.dram_tensor("img", [B_LOC, 784], F32R, kind="ExternalInput").ap()
    tgt = nc.dram_tensor("tgt", [B_LOC, 1], I32, kind="ExternalInput").ap()
    w1d = nc.dram_tensor("w1d", [81, 256], F32R, kind="ExternalInput").ap()
    b1d = nc.dram_tensor("b1d", [128, 2], F32, kind="ExternalInput").ap()
    # (tap, cin_low, cinh, couth, cout_low)
    w2d = nc.dram_tensor("w2d", [81, 128, 2, 2, 128], F32R, kind="ExternalInput").ap()
    b2d = nc.dram_tensor("b2d", [128, 2], F32, kind="ExternalInput").ap()
    wtd = nc.dram_tensor("wtd", [128, 72, 160], F32R, kind="ExternalInput").ap()
    wt2ad = nc.dram_tensor("wt2ad", [128, 9216], BF16D, kind="ExternalInput").ap()
    wt2bd = nc.dram_tensor("wt2bd", [32, 9216], BF16D, kind="ExternalInput").ap()
    d1wd = nc.dram_tensor("d1wd", [160, 512], F32R, kind="ExternalInput").ap()
    d2wd = nc.dram_tensor("d2wd", [4, 128, 1024], F32R, kind="ExternalInput").ap()
    d3wd = nc.dram_tensor("d3wd", [8, 128, 784], F32R, kind="ExternalInput").ap()
    d1bd = nc.dram_tensor("d1bd", [512], F32, kind="ExternalInput").ap()
    d2bd = nc.dram_tensor("d2bd", [1024], F32, kind="ExternalInput").ap()
    d3bd = nc.dram_tensor("d3bd", [784], F32, kind="ExternalInput").ap()

    normd = nc.dram_tensor("normd", [B_LOC, 10], F32, kind="ExternalOutput").ap()
    ypredd = nc.dram_tensor("ypredd", [B_LOC, 1], I32, kind="ExternalOutput").ap()
    decd = nc.dram_tensor("decd", [B_LOC, 784], F32, kind="ExternalOutput").ap()

    dbg = {}
    if debug:
        for name, shape in [
            ("dbg_x1", [2, 128, 16, 400]),
            ("dbg_x2", [2, 128, 16, 36]),
            ("dbg_pc", [2, 128, 16, 36]),
            ("dbg_s1", [B_LOC, 160]),
            ("dbg_v1", [B_LOC, 160]),
            ("dbg_uv1", [160, 1152]),
            ("dbg_c2z", [160, 1152]),
            ("dbg_s2", [B_LOC, 160]),
            ("dbg_v2", [B_LOC, 160]),
            ("dbg_h0", [B_LOC, 160]),
        ]:
            dbg[name] = nc.dram_tensor(name, shape, F32, kind="ExternalOutput").ap()

    # Constants embedded in the NEFF.
    ident_np = np.eye(128, dtype=np.float32)
    identd = nc.inline_tensor(ident_np, "identc").ap()
    s_np = np.zeros((128, 16), np.float32)
    for c in range(128):
        s_np[c, c // 8] = 1.0
    sd = nc.inline_tensor(s_np, "sconst").ap()
    iota_np = np.arange(10, dtype=np.float32).reshape(1, 10)
    iotad = nc.inline_tensor(iota_np, "iotac").ap()

    with tile.TileContext(nc) as tc:
        with tc.tile_pool(name="persist", bufs=1) as pp:
            # ---- persistent tiles ----
            ident = pp.tile([128, 128], F32, tag="ident")
            nc.sync.dma_start(out=ident, in_=identd)
            smat = pp.tile([128, 16], F32, tag="smat")
            nc.sync.dma_start(out=smat, in_=sd)
            smat_r = pp.tile([128, 16], F32R, tag="smat_r")
            nc.vector.tensor_copy(smat_r, smat)
            identr = pp.tile([128, 128], F32R, tag="identr")
            nc.vector.tensor_copy(identr, ident)
            iota = pp.tile([B_LOC, 10], F32, tag="iota")
            nc.sync.dma_start(out=iota, in_=_ap(iotad, 0, [[0, B_LOC], [1, 10]]))

            w1 = pp.tile([81, 256], F32R, tag="w1")
            nc.sync.dma_start(out=w1, in_=w1d)
            b1 = pp.tile([128, 2], F32, tag="b1")
            nc.sync.dma_start(out=b1, in_=b1d)
            b2 = pp.tile([128, 2], F32, tag="b2")
            nc.sync.dma_start(out=b2, in_=b2d)


            # conv2 output (post-relu), layout [cout_low, (b, hw36)] x couth
            x2 = [pp.tile([128, B_LOC, 36], F32, tag=f"x2_{h}", name=f"x2_{h}") for h in range(2)]
            # primary capsules, same layout
            pc2 = [pp.tile([128, B_LOC, 36], F32R, tag=f"pc2_{h}", name=f"pc2_{h}") for h in range(2)]

            # small working tiles for routing scalars
            s1 = pp.tile([B_LOC, 160], F32, tag="s1")
            v1 = pp.tile([B_LOC, 160], F32, tag="v1")
            s2 = pp.tile([B_LOC, 160], F32, tag="s2")
            v2 = pp.tile([B_LOC, 160], F32, tag="v2")
            h0 = pp.tile([B_LOC, 160], F32, tag="h0")
            norm = pp.tile([B_LOC, 10], F32, tag="norm")
            epsb = pp.tile([128, 1], F32, tag="epsb")
            nc.vector.memset(epsb, EPS)

            # ---------------- Phase 1: conv1 ----------------
            # out[couth*128 + cl, (b, 400)] = relu(W1.T @ patches + b1)
            with (
                tc.tile_pool(name="c1work", bufs=3) as c1w,
                tc.tile_pool(name="c1ps", bufs=4, space="PSUM") as c1p,
                tc.tile_pool(name="x1pool", bufs=1) as x1pool,
            ):
                x1 = [
                    x1pool.tile([128, B_LOC, 400], F32R, tag=f"x1_{h}", name=f"x1_{h}") for h in range(2)
                ]
                for b in range(B_LOC):
                    patch = c1w.tile([81, 400], F32R, tag="patch")
                    for kh in range(9):
                        srcap = _ap(
                            img, b * 784 + kh * 28, [[1, 9], [28, 20], [1, 20]]
                        )
                        dstap = patch[kh * 9 : (kh + 1) * 9, :].rearrange(
                            "a (h w) -> a h w", w=20
                        )
                        nc.sync.dma_start(out=dstap, in_=srcap)
                    for h in range(2):
                        ps = c1p.tile([128, 400], F32, tag="c1ps")
                        nc.tensor.matmul(
                            ps,
                            w1[:, h * 128 : (h + 1) * 128],
                            patch,
                            start=True,
                            stop=True,
                        )
                        nc.scalar.activation(
                            out=x1[h][:, b, :],
                            in_=ps,
                            func=AF.Relu,
                            bias=b1[:, h : h + 1],
                            scale=1.0,
                        )
                if debug:
                    for h in range(2):
                        nc.sync.dma_start(out=dbg["dbg_x1"][h], in_=x1[h].bitcast(F32))

                # ---------------- Phase 2: conv2 ----------------
                # stride-2 9x9: out[cout, (bg*8+b, oh, ow)] accumulated over
                # (tap, cinh); rhs is a strided view of x1.
                with (
                    tc.tile_pool(name="w2stream", bufs=3) as w2s,
                    tc.tile_pool(name="qps", bufs=1, space="PSUM") as qps,
                ):
                    q = [
                        [qps.tile([128, 288], F32, tag=f"q_{h}_{g}", name=f"q_{h}_{g}") for g in range(2)]
                        for h in range(2)
                    ]
                    for tap in range(81):
                        kh, kw = tap // 9, tap % 9
                        w2t = w2s.tile([128, 2, 2, 128], F32R, tag="w2t")
                        nc.sync.dma_start(out=w2t, in_=w2d[tap])
                        for cinh in range(2):
                            base = x1[cinh]
                            for couth in range(2):
                                lhsT = w2t[:, cinh, couth, :]
                                for g in range(2):
                                    off = (g * 8) * 400 + kh * 20 + kw
                                    rhs = _ap(
                                        base,
                                        off,
                                        [base.ap[0], [400, 8], [40, 6], [2, 6]],
                                    )
                                    nc.tensor.matmul(
                                        q[couth][g],
                                        lhsT,
                                        rhs,
                                        start=(tap == 0 and cinh == 0),
                                        stop=(tap == 80 and cinh == 1),
                                    )
                    for couth in range(2):
                        for g in range(2):
                            nc.scalar.activation(
                                out=x2[couth][:, g * 8 : (g + 1) * 8, :],
                                in_=q[couth][g].rearrange("p (b f) -> p b f", f=36),
                                func=AF.Relu,
                                bias=b2[:, couth : couth + 1],
                                scale=1.0,
                            )
            if debug:
                for h in range(2):
                    nc.sync.dma_start(out=dbg["dbg_x2"][h], in_=x2[h])

            # ---------------- Phase 3: squash -> pc2 ----------------
            with (
                tc.tile_pool(name="sqw", bufs=1) as sqw,
                tc.tile_pool(name="sqps", bufs=4, space="PSUM") as sqps,
            ):
                sq = [sqw.tile([128, B_LOC * 36], F32R, tag=f"sq_{h}", name=f"sq_{h}") for h in range(2)]
                fexp = sqw.tile([128, 576], F32, tag="fexp", bufs=2)
                for h in range(2):
                    flat = x2[h].rearrange("p b f -> p (b f)")
                    nc.vector.tensor_mul(sq[h], flat, flat)
                    ssq = sqw.tile([B_LOC, 576], F32, tag="ssq", name="ssq", bufs=2)
                    for g in range(2):
                        ps = sqps.tile([16, 288], F32, tag="sqps", name="sqps")
                        nc.tensor.matmul(
                            ps,
                            smat_r,
                            sq[h][:, g * 288 : (g + 1) * 288],
                            start=True,
                            stop=True,
                        )
                        nc.scalar.copy(
                            out=ssq[:, g * 288 : (g + 1) * 288],
                            in_=ps,
                        )
                    # f = (ssq/(1+ssq)) / sqrt(ssq+eps)
                    a = sqw.tile([B_LOC, 576], F32, tag="fa", name="fa", bufs=2)
                    r = sqw.tile([B_LOC, 576], F32, tag="fr", name="fr", bufs=2)
                    fT = sqw.tile([B_LOC, 576], F32, tag="fT", name="fT", bufs=2)
                    nc.vector.tensor_scalar_add(a, ssq, 1.0)
                    nc.vector.reciprocal(r, a)
                    nc.vector.tensor_mul(r, ssq, r)  # ssq/(1+ssq)
                    nc.scalar.activation(
                        out=a, in_=ssq, func=AF.Sqrt, bias=epsb[:B_LOC, :], scale=1.0
                    )
                    nc.vector.reciprocal(a, a)
                    nc.vector.tensor_mul(fT, r, a)
                    # broadcast f over k=8 partition-groups
                    fe = fexp if h == 0 else sqw.tile(
                        [128, 576], F32, tag="fexp", name="fexp2", bufs=2
                    )
                    nc.sync.dma_start(
                        out=fe, in_=_ap(fT, 0, [fT.ap[0], [0, 8], [1, 576]])
                    )
                    nc.vector.tensor_mul(
                        pc2[h].rearrange("p b f -> p (b f)"),
                        x2[h].rearrange("p b f -> p (b f)"),
                        fe,
                    )
            if debug:
                for h in range(2):
                    nc.sync.dma_start(out=dbg["dbg_pc"][h], in_=pc2[h].bitcast(F32))

            def pc_chunk(n):
                """lhsT [(ik) 128, b] for flat (i,k)-chunk n (=hw*2+chalf)."""
                hw, chalf = n // 2, n % 2
                base = pc2[chalf]
                return _ap(base, hw, [base.ap[0], [36, B_LOC]])

            # ---------------- Phase 4: s1 = 0.1 * sum_i u_hat ----------------
            with (
                tc.tile_pool(name="s1ps", bufs=1, space="PSUM") as s1psp,
                tc.tile_pool(name="wts1", bufs=6) as wts1,
            ):
                s1ps = s1psp.tile([B_LOC, 160], F32, tag="s1ps")
                for n in range(72):
                    wtc = wts1.tile([128, 160], F32R, tag="wtc", name="wtc")
                    nc.sync.dma_start(out=wtc, in_=wtd[:, n, :])
                    nc.tensor.matmul(
                        s1ps,
                        pc_chunk(n),
                        wtc,
                        start=(n == 0),
                        stop=(n == 71),
                    )
                nc.scalar.mul(out=s1, in_=s1ps, mul=0.1)
            if debug:
                nc.sync.dma_start(out=dbg["dbg_s1"], in_=s1)

            # small-squash helper: x [16, nj, 16] -> out, also writes ssq
            def squash16(out_t, in_t, tmp_pool, nj, ssq_out=None):  # noqa: uses epsb
                ssq = tmp_pool.tile([B_LOC, nj], F32, tag="sq_ssq")
                prod = tmp_pool.tile([B_LOC, nj * 16], F32, tag="sq_prod")
                nc.vector.tensor_mul(prod, in_t, in_t)
                nc.vector.reduce_sum(
                    ssq, prod.rearrange("p (j d) -> p j d", d=16), axis=AX.X
                )
                aa = tmp_pool.tile([B_LOC, nj], F32, tag="sq_a")
                rr = tmp_pool.tile([B_LOC, nj], F32, tag="sq_r")
                ff = tmp_pool.tile([B_LOC, nj], F32, tag="sq_f")
                nc.vector.tensor_scalar_add(aa, ssq, 1.0)
                nc.vector.reciprocal(rr, aa)
                nc.vector.tensor_mul(rr, ssq, rr)
                nc.scalar.activation(out=aa, in_=ssq, func=AF.Sqrt, bias=epsb[:B_LOC, :], scale=1.0)
                nc.vector.reciprocal(aa, aa)
                nc.vector.tensor_mul(ff, rr, aa)
                fb = _ap(ff, 0, [ff.ap[0], [1, nj], [0, 16]])
                nc.vector.tensor_tensor(
                    out=out_t.rearrange("p (j d) -> p j d", d=16),
                    in0=in_t.rearrange("p (j d) -> p j d", d=16),
                    in1=fb,
                    op=ALU.mult,
                )
                if ssq_out is not None:
                    nc.vector.tensor_copy(ssq_out, ssq)

            with tc.tile_pool(name="rwork", bufs=1) as rw:
                squash16(v1, s1, rw, 10)
                if debug:
                    nc.sync.dma_start(out=dbg["dbg_v1"], in_=v1)

                # ---- v1 block-diagonal [ (j,d), (j',b) ] in bf16 ----
                with tc.tile_pool(name="tps", bufs=1, space="PSUM") as tps:
                    blkA = rw.tile([128, 160], BF16D, tag="blkA")
                    blkB = rw.tile([32, 160], BF16D, tag="blkB")
                    nc.vector.memset(blkA, 0.0)
                    nc.vector.memset(blkB, 0.0)
                    t1 = tps.tile([128, 16], F32, tag="t1")
                    nc.tensor.transpose(t1, v1[:, 0:128], ident[:16, :16])
                    t2 = tps.tile([32, 16], F32, tag="t2")
                    nc.tensor.transpose(t2, v1[:, 128:160], ident[:16, :16])
                    v1T1 = rw.tile([128, 16], BF16D, tag="v1T1")
                    v1T2 = rw.tile([32, 16], BF16D, tag="v1T2")
                    nc.scalar.copy(out=v1T1, in_=t1)
                    nc.scalar.copy(out=v1T2, in_=t2)
                    for j in range(8):
                        nc.sync.dma_start(
                            out=blkA[j * 16 : (j + 1) * 16, j * 16 : (j + 1) * 16],
                            in_=v1T1[j * 16 : (j + 1) * 16, :],
                        )
                    for j in range(8, 10):
                        nc.sync.dma_start(
                            out=blkB[(j - 8) * 16 : (j - 7) * 16, j * 16 : (j + 1) * 16],
                            in_=v1T2[(j - 8) * 16 : (j - 7) * 16, :],
                        )

                    # ---- A[(j',b), (ik)] = sum_d v1 * W  (bf16, full rate) ----
                    A1 = rw.tile([128, 9216], BF16D, tag="A1")
                    A2 = rw.tile([32, 9216], BF16D, tag="A2")
                    with (
                        tc.tile_pool(name="aps", bufs=2, space="PSUM") as aps,
                        tc.tile_pool(name="wt2s", bufs=4) as wt2s,
                    ):
                        for cn in range(18):
                            sl = slice(cn * 512, (cn + 1) * 512)
                            wa = wt2s.tile([128, 512], BF16D, tag="wa", name="wa")
                            nc.sync.dma_start(out=wa, in_=wt2ad[:, sl])
                            wb = wt2s.tile([32, 512], BF16D, tag="wb", name="wb")
                            nc.sync.dma_start(out=wb, in_=wt2bd[:, sl])
                            for mi, (mdst, msl) in enumerate(
                                [(A1, slice(0, 128)), (A2, slice(128, 160))]
                            ):
                                ps = aps.tile(
                                    [128 if mi == 0 else 32, 512], F32, tag=f"aps{mi}", name=f"aps{mi}"
                                )
                                nc.tensor.matmul(
                                    ps, blkA[:, msl], wa, start=True, stop=False
                                )
                                nc.tensor.matmul(
                                    ps, blkB[:, msl], wb, start=False, stop=True
                                )
                                nc.scalar.copy(out=mdst[:, sl], in_=ps)

                    # ---- pcT [b, (ik)] bf16 via PE transposes ----
                    pcT = rw.tile([B_LOC, 9216], BF16D, tag="pcT")
                    stage = rw.tile([36, 256], BF16D, tag="stage")
                    for b in range(B_LOC):
                        for h in range(2):
                            ps = tps.tile([36, 128], F32R, tag="tp36")
                            nc.tensor.transpose(ps, pc2[h][:, b, :], identr)
                            nc.scalar.copy(
                                out=stage[:, h * 128 : (h + 1) * 128], in_=ps
                            )
                        nc.sync.dma_start(out=pcT[b : b + 1, :], in_=stage)

                # ---- pc_rep [(j,b), (ik)] bf16 ----
                R1 = rw.tile([128, 9216], BF16D, tag="R1")
                R2 = rw.tile([32, 9216], BF16D, tag="R2")
                for j in range(8):
                    nc.sync.dma_start(out=R1[j * 16 : (j + 1) * 16, :], in_=pcT)
                for j in range(2):
                    nc.sync.dma_start(out=R2[j * 16 : (j + 1) * 16, :], in_=pcT)

                # ---- uv1[(j,b), i] = sum_k A * pc ----
                uv1a = rw.tile([128, 1152], F32, tag="uv1a")
                uv1b = rw.tile([32, 1152], F32, tag="uv1b")
                nc.vector.tensor_mul(A1, A1, R1)
                nc.vector.reduce_sum(
                    uv1a, A1.rearrange("p (i k) -> p i k", k=8), axis=AX.X
                )
                nc.vector.tensor_mul(A2, A2, R2)
                nc.vector.reduce_sum(
                    uv1b, A2.rearrange("p (i k) -> p i k", k=8), axis=AX.X
                )
                if debug:
                    nc.sync.dma_start(out=dbg["dbg_uv1"][0:128], in_=uv1a)
                    nc.sync.dma_start(out=dbg["dbg_uv1"][128:160], in_=uv1b)

                # ---- softmax over j (no max-subtraction; uv1 is small) ----
                e1 = rw.tile([128, 1152], F32, tag="e1")
                e2 = rw.tile([32, 1152], F32, tag="e2")
                nc.scalar.activation(out=e1, in_=uv1a, func=AF.Exp)
                nc.scalar.activation(out=e2, in_=uv1b, func=AF.Exp)

                # gather exp into [b, (i, j10)] via DMA, reduce over j
                zbuf = rw.tile([B_LOC, 1152, 10], F32, tag="zbuf")
                for j in range(8):
                    nc.sync.dma_start(
                        out=zbuf[:, :, j : j + 1],
                        in_=e1[j * 16 : (j + 1) * 16, :],
                    )
                for j in range(2):
                    nc.sync.dma_start(
                        out=zbuf[:, :, 8 + j : 9 + j],
                        in_=e2[j * 16 : (j + 1) * 16, :],
                    )
                zsum = rw.tile([B_LOC, 1152], F32, tag="zsum")
                zt = rw.tile([B_LOC, 1152], F32, tag="zt")
                nc.vector.reduce_sum(zsum, zbuf, axis=AX.X)
                nc.vector.reciprocal(zt, zsum)
                # replicate 1/Z across j partition groups
                zrep1 = rw.tile([128, 1152], F32, tag="zrep1")
                zrep2 = rw.tile([32, 1152], F32, tag="zrep2")
                for j in range(8):
                    nc.sync.dma_start(out=zrep1[j * 16 : (j + 1) * 16, :], in_=zt)
                for j in range(2):
                    nc.sync.dma_start(out=zrep2[j * 16 : (j + 1) * 16, :], in_=zt)
                nc.vector.tensor_mul(e1, e1, zrep1)
                nc.vector.tensor_mul(e2, e2, zrep2)
                if debug:
                    nc.sync.dma_start(out=dbg["dbg_c2z"][0:128], in_=e1)
                    nc.sync.dma_start(out=dbg["dbg_c2z"][128:160], in_=e2)

                # ---- c2zT [i(9x128), (j,b)160] via PE transposes ----
                c2zT = rw.tile([128, 9, 160], F32, tag="c2zT")
                with tc.tile_pool(name="tps2", bufs=2, space="PSUM") as tps2:
                    for cb in range(9):
                        ps = tps2.tile([128, 128], F32, tag="ct1")
                        nc.tensor.transpose(
                            ps, e1[:, cb * 128 : (cb + 1) * 128], ident
                        )
                        nc.scalar.copy(out=c2zT[:, cb, 0:128], in_=ps)
                        ps2 = tps2.tile([128, 32], F32, tag="ct2")
                        nc.tensor.transpose(
                            ps2, e2[:, cb * 128 : (cb + 1) * 128], ident[:32, :32]
                        )
                        nc.scalar.copy(out=c2zT[:, cb, 128:160], in_=ps2)

                # ---- s2: accumulate over 72 (ik)-chunks ----
                with (
                    tc.tile_pool(name="s2w", bufs=4) as s2w,
                    tc.tile_pool(name="wts2", bufs=6) as wts2,
                    tc.tile_pool(name="s2ps", bufs=1, space="PSUM") as s2psp,
                ):
                    psA = s2psp.tile([128, 160], F32, tag="psA")
                    psB = s2psp.tile([32, 160], F32, tag="psB")
                    for n in range(72):
                        i0 = 16 * n
                        rep = s2w.tile([128, 160], F32, tag="rep")
                        src = c2zT[i0 % 128 : i0 % 128 + 16, n // 8, :]
                        nc.sync.dma_start(
                            out=rep,
                            in_=_ap(src, 0, [src.ap[0], [0, 8], [1, 160]]),
                        )
                        g = s2w.tile([128, 160], F32R, tag="g")
                        pcb = pc_chunk(n)
                        nc.vector.tensor_tensor(
                            out=g.rearrange("p (j b) -> p j b", j=10),
                            in0=rep.rearrange("p (j b) -> p j b", j=10),
                            in1=_ap(pcb, 0, [pcb.ap[0], [0, 10], [36, B_LOC]]),
                            op=ALU.mult,
                        )
                        wtc2 = wts2.tile([128, 160], F32R, tag="wtc2", name="wtc2")
                        nc.sync.dma_start(out=wtc2, in_=wtd[:, n, :])
                        nc.tensor.matmul(
                            psA,
                            g[:, 0:128],
                            wtc2,
                            start=(n == 0),
                            stop=(n == 71),
                        )
                        nc.tensor.matmul(
                            psB,
                            g[:, 128:160],
                            wtc2,
                            start=(n == 0),
                            stop=(n == 71),
                        )
                    sA = s2w.tile([128, 160], F32, tag="sA")
                    sB = s2w.tile([32, 160], F32, tag="sB")
                    nc.scalar.copy(out=sA, in_=psA)
                    nc.scalar.copy(out=sB, in_=psB)
                    for j in range(8):
                        nc.sync.dma_start(
                            out=s2[:, j * 16 : (j + 1) * 16],
                            in_=sA[j * 16 : (j + 1) * 16, j * 16 : (j + 1) * 16],
                        )
                    for j in range(8, 10):
                        nc.sync.dma_start(
                            out=s2[:, j * 16 : (j + 1) * 16],
                            in_=sB[(j - 8) * 16 : (j - 7) * 16, j * 16 : (j + 1) * 16],
                        )
                if debug:
                    nc.sync.dma_start(out=dbg["dbg_s2"], in_=s2)

                # ---- v2, norm, y_pred, mask ----
                ssq2 = rw.tile([B_LOC, 10], F32, tag="ssq2")
                v2sq = rw.tile([B_LOC, 160], F32, tag="v2sq")
                squash16(v2, s2, rw, 10)
                nc.vector.tensor_mul(v2sq, v2, v2)
                nc.vector.reduce_sum(
                    ssq2, v2sq.rearrange("p (j d) -> p j d", d=16), axis=AX.X
                )
                nc.scalar.activation(out=norm, in_=ssq2, func=AF.Sqrt, bias=epsb[:B_LOC, :], scale=1.0)
                nc.sync.dma_start(out=normd, in_=norm)
                if debug:
                    nc.sync.dma_start(out=dbg["dbg_v2"], in_=v2)

                vmax = rw.tile([B_LOC, 8], F32, tag="vmax")
                vidx = rw.tile([B_LOC, 8], U32, tag="vidx")
                nc.vector.max_with_indices(vmax, vidx, norm)
                ypi = rw.tile([B_LOC, 1], I32, tag="ypi")
                nc.vector.tensor_copy(ypi, vidx[:, 0:1])
                nc.sync.dma_start(out=ypredd, in_=ypi)

                tgtf = rw.tile([B_LOC, 1], F32, tag="tgtf")
                tgti = rw.tile([B_LOC, 1], I32, tag="tgti")
                nc.sync.dma_start(out=tgti, in_=tgt)
                nc.vector.tensor_copy(tgtf, tgti)
                mask = rw.tile([B_LOC, 10], F32, tag="mask")
                nc.vector.tensor_scalar(
                    out=mask, in0=iota, scalar1=tgtf, scalar2=None, op0=ALU.is_equal
                )
                nc.vector.tensor_tensor(
                    out=h0.rearrange("p (j d) -> p j d", d=16),
                    in0=v2.rearrange("p (j d) -> p j d", d=16),
                    in1=_ap(mask, 0, [mask.ap[0], [1, 10], [0, 16]]),
                    op=ALU.mult,
                )
                if debug:
                    nc.sync.dma_start(out=dbg["dbg_h0"], in_=h0)

            # ---------------- Phase 5: decoder ----------------
            with (
                tc.tile_pool(name="dwpool", bufs=1) as dw,
                tc.tile_pool(name="dps", bufs=2, space="PSUM") as dps,
                tc.tile_pool(name="dwork", bufs=1) as dwk,
            ):
                d1w = dw.tile([128, 512], F32R, tag="d1w_hi")
                d1wb = dw.tile([32, 512], F32R, tag="d1w_lo")
                nc.sync.dma_start(out=d1w, in_=d1wd[0:128])
                nc.sync.dma_start(out=d1wb, in_=d1wd[128:160])
                d2w = dw.tile([128, 4, 1024], F32R, tag="d2w")
                for c in range(4):
                    nc.sync.dma_start(out=d2w[:, c, :], in_=d2wd[c])
                d3w = dw.tile([128, 8, 784], F32R, tag="d3w")
                for c in range(8):
                    nc.sync.dma_start(out=d3w[:, c, :], in_=d3wd[c])
                b1r = dwk.tile([B_LOC, 512], F32, tag="b1r")
                nc.sync.dma_start(out=b1r, in_=_ap(d1bd, 0, [[0, B_LOC], [1, 512]]))
                b2r = dwk.tile([B_LOC, 1024], F32, tag="b2r")
                nc.sync.dma_start(out=b2r, in_=_ap(d2bd, 0, [[0, B_LOC], [1, 1024]]))
                b3r = dwk.tile([B_LOC, 784], F32, tag="b3r")
                nc.sync.dma_start(out=b3r, in_=_ap(d3bd, 0, [[0, B_LOC], [1, 784]]))

                # h0 -> h0T
                h0T1 = dwk.tile([128, 16], F32R, tag="h0T1")
                h0T2 = dwk.tile([32, 16], F32R, tag="h0T2")
                ps = dps.tile([128, 16], F32, tag="dtp")
                nc.tensor.transpose(ps, h0[:, 0:128], ident[:16, :16])
                nc.scalar.copy(out=h0T1, in_=ps)
                ps = dps.tile([32, 16], F32, tag="dtp")
                nc.tensor.transpose(ps, h0[:, 128:160], ident[:16, :16])
                nc.scalar.copy(out=h0T2, in_=ps)

                # L1: h1 = relu(h0 @ d1w + b1)
                h1 = dwk.tile([B_LOC, 512], F32, tag="h1")
                hp = dps.tile([B_LOC, 512], F32, tag="mmp")
                nc.tensor.matmul(hp, h0T1, d1w, start=True, stop=False)
                nc.tensor.matmul(hp, h0T2, d1wb, start=False, stop=True)
                nc.vector.tensor_add(h1, hp, b1r)
                nc.scalar.activation(out=h1, in_=h1, func=AF.Relu)

                # L2
                h1T = dwk.tile([128, 4, 16], F32R, tag="h1T")
                for c in range(4):
                    ps = dps.tile([128, 16], F32, tag="dtp")
                    nc.tensor.transpose(
                        ps, h1[:, c * 128 : (c + 1) * 128], ident[:16, :16]
                    )
                    nc.scalar.copy(out=h1T[:, c, :], in_=ps)
                h2 = dwk.tile([B_LOC, 1024], F32, tag="h2")
                for nh in range(2):
                    hp2 = dps.tile([B_LOC, 512], F32, tag="mmp", name="hp2")
                    for c in range(4):
                        nc.tensor.matmul(
                            hp2,
                            h1T[:, c, :],
                            d2w[:, c, nh * 512 : (nh + 1) * 512],
                            start=(c == 0),
                            stop=(c == 3),
                        )
                    sl = slice(nh * 512, (nh + 1) * 512)
                    nc.vector.tensor_add(h2[:, sl], hp2, b2r[:, sl])
                    nc.scalar.activation(out=h2[:, sl], in_=h2[:, sl], func=AF.Relu)

                # L3
                h2T = dwk.tile([128, 8, 16], F32R, tag="h2T")
                for c in range(8):
                    ps = dps.tile([128, 16], F32, tag="dtp")
                    nc.tensor.transpose(
                        ps, h2[:, c * 128 : (c + 1) * 128], ident[:16, :16]
                    )
                    nc.scalar.copy(out=h2T[:, c, :], in_=ps)
                dec = dwk.tile([B_LOC, 784], F32, tag="dec")
                for nh, (n0, n1) in enumerate([(0, 512), (512, 784)]):
                    hp3 = dps.tile([B_LOC, 512], F32, tag="mmp", name="hp3")[:, : n1 - n0]
                    for c in range(8):
                        nc.tensor.matmul(
                            hp3,
                            h2T[:, c, :],
                            d3w[:, c, n0:n1],
                            start=(c == 0),
                            stop=(c == 7),
                        )
                    nc.vector.tensor_add(dec[:, n0:n1], hp3, b3r[:, n0:n1])
                nc.scalar.activation(out=dec, in_=dec, func=AF.Sigmoid)
                nc.sync.dma_start(out=decd, in_=dec)

    nc.compile()
    return nc


# ---------------------------------------------------------------------------
# Host side
# ---------------------------------------------------------------------------


def prep_inputs(image, target, conv1_w, conv1_b, conv2_w, conv2_b, W,
                d1_w, d1_b, d2_w, d2_b, do_w, do_b):
    """Full inputs -> list of 8 per-core input maps."""
    image = np.asarray(image, np.float32)
    target = np.asarray(target)
    B = image.shape[0]
    per = B // N_CORES

    w1 = np.asarray(conv1_w, np.float32).reshape(81, 256)
    b1 = np.zeros((128, 2), np.float32)
    b1[:, 0] = np.asarray(conv1_b, np.float32)[0:128]
    b1[:, 1] = np.asarray(conv1_b, np.float32)[128:256]
    w2 = (
        np.asarray(conv2_w, np.float32)
        .reshape(81, 2, 128, 2, 128)
        .transpose(0, 2, 1, 3, 4)
        .copy()
    )
    b2 = np.zeros((128, 2), np.float32)
    b2[:, 0] = np.asarray(conv2_b, np.float32)[0:128]
    b2[:, 1] = np.asarray(conv2_b, np.float32)[128:256]

    W0 = np.asarray(W, np.float32)[0]  # [1152, 10, 16, 8]
    wt = (
        W0.transpose(0, 3, 1, 2)  # i, k, j, d
        .reshape(9216, 160)
        .reshape(72, 128, 160)
        .transpose(1, 0, 2)
        .copy()
    )
    wt2 = W0.transpose(1, 2, 0, 3).reshape(160, 9216)  # (j,d), (i,k)
    wt2a = wt2[0:128].astype(BF16)
    wt2b = wt2[128:160].astype(BF16)

    d1w = np.asarray(d1_w, np.float32)
    d2w = np.asarray(d2_w, np.float32).reshape(4, 128, 1024)
    d3w = np.asarray(do_w, np.float32).reshape(8, 128, 784)

    shared = dict(
        w1d=w1, b1d=b1, w2d=w2, b2d=b2, wtd=wt, wt2ad=wt2a, wt2bd=wt2b,
        d1wd=d1w, d2wd=d2w, d3wd=d3w,
        d1bd=np.asarray(d1_b, np.float32),
        d2bd=np.asarray(d2_b, np.float32),
        d3bd=np.asarray(do_b, np.float32),
    )
    maps = []
    for c in range(N_CORES):
        m = dict(shared)
        m["img"] = image[c * per : (c + 1) * per].reshape(per, 784).copy()
        m["tgt"] = target[c * per : (c + 1) * per].astype(np.int32).reshape(per, 1)
        maps.append(m)
    return maps


_NC_CACHE = {}


def _get_nc(debug=False):
    key = bool(debug)
    if key not in _NC_CACHE:
        _NC_CACHE[key] = build_nc(debug=key)
    return _NC_CACHE[key]


def kernel(**inputs):
    from concourse import bass_utils

    target = np.asarray(inputs["target"])
    maps = prep_inputs(**inputs)
    nc = _get_nc(debug=False)
    res = bass_utils.run_bass_kernel_spmd(nc, maps, core_ids=list(range(N_CORES)))
    outs = res.results
    B = N_CORES * B_LOC
    norm = np.concatenate([o["normd"] for o in outs], 0).reshape(B, 1, 10, 1, 1)
    ypred = np.concatenate([o["ypredd"] for o in outs], 0).reshape(B, 1)
    dec = np.concatenate([o["decd"] for o in outs], 0).reshape(B, 1, 784)
    if target.dtype == np.int64:
        ypred = ypred.astype(np.int64)
    else:
        ypred = ypred.astype(np.int32)
    return norm, ypred, dec


# revision 24
# speedup vs baseline: 1.7605x; 1.7605x over previous
"""CapsuleNetwork Trainium2 Bass kernel.

8-core data-parallel: batch 128 -> 16 per core, all weights replicated.
Per-core pipeline (all on device):
  conv1 (host-im2col patches x W1 matmul) -> conv2 (tap-accumulation
  matmul, fp32r full rate) -> primary-caps squash (partition-group
  sum-of-squares via ones-blockdiag matmul) -> dynamic routing
  (2 rounds; s1 via K=9216 matmul, A-tensor via block-diag-v1 matmul,
  softmax transpose+replicate via permuted-identity matmuls) ->
  norm/argmax/mask -> 3-layer decoder.
"""

import sys

import numpy as np

sys.path.insert(0, "/opt/trn_rl_repo")

import concourse.bass as bass
import concourse.bacc as bacc
import concourse.tile as tile
from concourse import mybir

import ml_dtypes

BF16 = ml_dtypes.bfloat16

EPS = 1e-7
B_LOC = 16  # images per core
N_CORES = 8
F32 = mybir.dt.float32
F32R = mybir.dt.float32r
BF16D = mybir.dt.bfloat16
I32 = mybir.dt.int32
U32 = mybir.dt.uint32
AF = mybir.ActivationFunctionType
ALU = mybir.AluOpType
AX = mybir.AxisListType


def _ap(base, offset_elems, dims):
    """Raw AP at base.offset + offset_elems with explicit [step,count] dims."""
    return bass.AP(tensor=base.tensor, offset=base.offset + offset_elems, ap=dims)


def build_nc(debug=False):
    nc = bacc.Bacc("TRN2", target_bir_lowering=False, debug=False)

    # ---------------- DRAM I/O ----------------
    patd = nc.dram_tensor("patd", [B_LOC, 81, 400], F32R, kind="ExternalInput").ap()
    tgt = nc.dram_tensor("tgt", [B_LOC, 1], I32, kind="ExternalInput").ap()
    w1d = nc.dram_tensor("w1d", [81, 256], F32R, kind="ExternalInput").ap()
    b1d = nc.dram_tensor("b1d", [128, 2], F32, kind="ExternalInput").ap()
    # (tapgroup 27, cin_low 128, tap3, cinh, couth, cout_low)
    w2d = nc.dram_tensor(
        "w2d", [27, 128, 3, 2, 2, 128], F32R, kind="ExternalInput"
    ).ap()
    b2d = nc.dram_tensor("b2d", [128, 2], F32, kind="ExternalInput").ap()
    # (super-chunk 6, cin_low 128, m 12, jd 160)
    wtd = nc.dram_tensor("wtd", [6, 128, 12, 160], F32R, kind="ExternalInput").ap()
    wt2ad = nc.dram_tensor("wt2ad", [128, 9216], BF16D, kind="ExternalInput").ap()
    wt2bd = nc.dram_tensor("wt2bd", [32, 9216], BF16D, kind="ExternalInput").ap()
    d1wd = nc.dram_tensor("d1wd", [160, 512], F32R, kind="ExternalInput").ap()
    d2wd = nc.dram_tensor("d2wd", [4, 128, 1024], F32R, kind="ExternalInput").ap()
    d3wd = nc.dram_tensor("d3wd", [8, 128, 784], F32R, kind="ExternalInput").ap()
    d1bd = nc.dram_tensor("d1bd", [512], F32, kind="ExternalInput").ap()
    d2bd = nc.dram_tensor("d2bd", [1024], F32, kind="ExternalInput").ap()
    d3bd = nc.dram_tensor("d3bd", [784], F32, kind="ExternalInput").ap()

    normd = nc.dram_tensor("normd", [B_LOC, 10], F32, kind="ExternalOutput").ap()
    ypredd = nc.dram_tensor("ypredd", [B_LOC, 1], I32, kind="ExternalOutput").ap()
    decd = nc.dram_tensor("decd", [B_LOC, 784], F32, kind="ExternalOutput").ap()

    dbg = {}
    if debug:
        for name, shape in [
            ("dbg_x1", [2, 128, 16, 400]),
            ("dbg_x2", [2, 128, 16, 36]),
            ("dbg_pc", [2, 128, 16, 36]),
            ("dbg_s1", [B_LOC, 160]),
            ("dbg_v1", [B_LOC, 160]),
            ("dbg_uv1", [160, 1152]),
            ("dbg_A", [160, 9216]),
            ("dbg_pcT", [16, 9216]),
            ("dbg_R", [160, 9216]),
            ("dbg_s2", [B_LOC, 160]),
            ("dbg_v2", [B_LOC, 160]),
            ("dbg_h0", [B_LOC, 160]),
        ]:
            dbg[name] = nc.dram_tensor(name, shape, F32, kind="ExternalOutput").ap()

    # ---------------- constants (embedded in NEFF) ----------------
    ident_np = np.eye(128, dtype=np.float32)
    identd = nc.inline_tensor(ident_np, "identc").ap()
    s_np = np.zeros((128, 16), np.float32)
    for c in range(128):
        s_np[c, c // 8] = 1.0
    sd = nc.inline_tensor(s_np, "sconst").ap()
    iota_np = np.arange(10, dtype=np.float32).reshape(1, 10)
    iotad = nc.inline_tensor(iota_np, "iotac").ap()
    # extended identities: row (j,b) -> col (j*16+b)
    ie1_np = np.zeros((128, 160), np.float32)
    ie1_np[:, 0:128] = np.eye(128)
    ie2_np = np.zeros((32, 160), np.float32)
    ie2_np[:, 128:160] = np.eye(32)
    ie1d = nc.inline_tensor(ie1_np.astype(BF16), "ie1c").ap()
    ie2d = nc.inline_tensor(ie2_np.astype(BF16), "ie2c").ap()

    with tile.TileContext(nc) as tc:
        with tc.tile_pool(name="persist", bufs=1) as pp:
            # ---- persistent tiles / consts ----
            ident = pp.tile([128, 128], F32, tag="ident")
            nc.sync.dma_start(out=ident, in_=identd)
            identr = pp.tile([128, 128], F32R, tag="identr")
            nc.vector.tensor_copy(identr, ident)
            smat = pp.tile([128, 16], F32, tag="smat")
            nc.sync.dma_start(out=smat, in_=sd)
            smat_r = pp.tile([128, 16], F32R, tag="smat_r")
            nc.vector.tensor_copy(smat_r, smat)
            iota = pp.tile([B_LOC, 10], F32, tag="iota")
            nc.sync.dma_start(out=iota, in_=_ap(iotad, 0, [[0, B_LOC], [1, 10]]))
            ie1 = pp.tile([128, 160], BF16D, tag="ie1")
            nc.sync.dma_start(out=ie1, in_=ie1d)
            ie2 = pp.tile([32, 160], BF16D, tag="ie2")
            nc.sync.dma_start(out=ie2, in_=ie2d)
            epsb = pp.tile([128, 1], F32, tag="epsb")
            nc.vector.memset(epsb, EPS)

            w1 = pp.tile([81, 256], F32R, tag="w1")
            nc.sync.dma_start(out=w1, in_=w1d)
            b1 = pp.tile([128, 2], F32, tag="b1")
            nc.sync.dma_start(out=b1, in_=b1d)
            b2 = pp.tile([128, 2], F32, tag="b2")
            nc.sync.dma_start(out=b2, in_=b2d)

            # decoder weights: load once, early (overlaps conv compute)
            d1w = pp.tile([128, 512], F32R, tag="d1w_hi")
            d1wb = pp.tile([32, 512], F32R, tag="d1w_lo")
            nc.sync.dma_start(out=d1w, in_=d1wd[0:128])
            nc.sync.dma_start(out=d1wb, in_=d1wd[128:160])
            d2w = pp.tile([128, 4, 1024], F32R, tag="d2w")
            for c in range(4):
                nc.sync.dma_start(out=d2w[:, c, :], in_=d2wd[c])
            d3w = pp.tile([128, 8, 784], F32R, tag="d3w")
            for c in range(8):
                nc.sync.dma_start(out=d3w[:, c, :], in_=d3wd[c])
            b1r = pp.tile([B_LOC, 512], F32, tag="b1r")
            nc.sync.dma_start(out=b1r, in_=_ap(d1bd, 0, [[0, B_LOC], [1, 512]]))
            b2r = pp.tile([B_LOC, 1024], F32, tag="b2r")
            nc.sync.dma_start(out=b2r, in_=_ap(d2bd, 0, [[0, B_LOC], [1, 1024]]))
            b3r = pp.tile([B_LOC, 784], F32, tag="b3r")
            nc.sync.dma_start(out=b3r, in_=_ap(d3bd, 0, [[0, B_LOC], [1, 784]]))

            # conv2 output (post-relu), layout [cout_low, (b, hw36)] x couth
            x2 = [pp.tile([128, B_LOC, 36], F32, tag=f"x2_{h}", name=f"x2_{h}") for h in range(2)]
            pc2 = [pp.tile([128, B_LOC, 36], F32R, tag=f"pc2_{h}", name=f"pc2_{h}") for h in range(2)]

            s1 = pp.tile([B_LOC, 160], F32, tag="s1")
            v1 = pp.tile([B_LOC, 160], F32, tag="v1")
            s2 = pp.tile([B_LOC, 160], F32, tag="s2")
            v2 = pp.tile([B_LOC, 160], F32, tag="v2")
            h0 = pp.tile([B_LOC, 160], F32, tag="h0")
            norm = pp.tile([B_LOC, 10], F32, tag="norm")

            # ---------------- Phase 1: conv1 ----------------
            with (
                tc.tile_pool(name="c1work", bufs=3) as c1w,
                tc.tile_pool(name="c1ps", bufs=4, space="PSUM") as c1p,
                tc.tile_pool(name="x1pool", bufs=1) as x1pool,
            ):
                x1 = [
                    x1pool.tile([128, B_LOC, 400], F32R, tag=f"x1_{h}", name=f"x1_{h}")
                    for h in range(2)
                ]
                for b in range(B_LOC):
                    patch = c1w.tile([81, 400], F32R, tag="patch")
                    nc.sync.dma_start(out=patch, in_=patd[b])
                    for h in range(2):
                        ps = c1p.tile([128, 400], F32, tag="c1ps")
                        nc.tensor.matmul(
                            ps,
                            w1[:, h * 128 : (h + 1) * 128],
                            patch,
                            start=True,
                            stop=True,
                        )
                        nc.scalar.activation(
                            out=x1[h][:, b, :],
                            in_=ps,
                            func=AF.Relu,
                            bias=b1[:, h : h + 1],
                            scale=1.0,
                        )
                if debug:
                    for h in range(2):
                        nc.sync.dma_start(
                            out=dbg["dbg_x1"][h], in_=x1[h].bitcast(F32)
                        )

                # ---------------- Phase 2: conv2 ----------------
                with (
                    tc.tile_pool(name="w2stream", bufs=3) as w2s,
                    tc.tile_pool(name="qps", bufs=1, space="PSUM") as qps,
                ):
                    q = [
                        [qps.tile([128, 288], F32, tag=f"q_{h}_{g}", name=f"q_{h}_{g}") for g in range(2)]
                        for h in range(2)
                    ]
                    for tg in range(27):
                        w2t = w2s.tile([128, 3, 2, 2, 128], F32R, tag="w2t")
                        nc.sync.dma_start(out=w2t, in_=w2d[tg])
                        for t3 in range(3):
                            tap = tg * 3 + t3
                            kh, kw = tap // 9, tap % 9
                            for cinh in range(2):
                                base = x1[cinh]
                                for couth in range(2):
                                    lhsT = w2t[:, t3, cinh, couth, :]
                                    for g in range(2):
                                        off = (g * 8) * 400 + kh * 20 + kw
                                        rhs = _ap(
                                            base,
                                            off,
                                            [base.ap[0], [400, 8], [40, 6], [2, 6]],
                                        )
                                        nc.tensor.matmul(
                                            q[couth][g],
                                            lhsT,
                                            rhs,
                                            start=(tap == 0 and cinh == 0),
                                            stop=(tap == 80 and cinh == 1),
                                        )
                    for couth in range(2):
                        for g in range(2):
                            nc.scalar.activation(
                                out=x2[couth][:, g * 8 : (g + 1) * 8, :],
                                in_=q[couth][g].rearrange("p (b f) -> p b f", f=36),
                                func=AF.Relu,
                                bias=b2[:, couth : couth + 1],
                                scale=1.0,
                            )
            if debug:
                for h in range(2):
                    nc.sync.dma_start(out=dbg["dbg_x2"][h], in_=x2[h])

            # ---------------- Phase 3: squash -> pc2 ----------------
            with (
                tc.tile_pool(name="sqw", bufs=1) as sqw,
                tc.tile_pool(name="sqps", bufs=4, space="PSUM") as sqps,
            ):
                sq = [sqw.tile([128, B_LOC * 36], F32R, tag=f"sq_{h}", name=f"sq_{h}") for h in range(2)]
                for h in range(2):
                    flat = x2[h].rearrange("p b f -> p (b f)")
                    nc.vector.tensor_mul(sq[h], flat, flat)
                    ssq = sqw.tile([B_LOC, 576], F32, tag="ssq", name="ssq", bufs=2)
                    for g in range(2):
                        ps = sqps.tile([16, 288], F32, tag="sqps", name="sqps")
                        nc.tensor.matmul(
                            ps,
                            smat_r,
                            sq[h][:, g * 288 : (g + 1) * 288],
                            start=True,
                            stop=True,
                        )
                        nc.scalar.copy(
                            out=ssq[:, g * 288 : (g + 1) * 288],
                            in_=ps,
                        )
                    # f = (ssq/(1+ssq)) / sqrt(ssq+eps)
                    a = sqw.tile([B_LOC, 576], F32, tag="fa", name="fa", bufs=2)
                    r = sqw.tile([B_LOC, 576], F32, tag="fr", name="fr", bufs=2)
                    fT = sqw.tile([B_LOC, 576], F32, tag="fT", name="fT", bufs=2)
                    nc.vector.tensor_scalar_add(a, ssq, 1.0)
                    nc.vector.reciprocal(r, a)
                    nc.vector.tensor_mul(r, ssq, r)
                    nc.scalar.activation(
                        out=a, in_=ssq, func=AF.Sqrt, bias=epsb[:B_LOC, :], scale=1.0
                    )
                    nc.vector.reciprocal(a, a)
                    nc.vector.tensor_mul(fT, r, a)
                    fe = sqw.tile([128, 576], F32, tag="fexp", name="fexp", bufs=2)
                    nc.sync.dma_start(
                        out=fe, in_=_ap(fT, 0, [fT.ap[0], [0, 8], [1, 576]])
                    )
                    nc.vector.tensor_mul(
                        pc2[h].rearrange("p b f -> p (b f)"),
                        x2[h].rearrange("p b f -> p (b f)"),
                        fe,
                    )
            if debug:
                for h in range(2):
                    nc.sync.dma_start(out=dbg["dbg_pc"][h], in_=pc2[h].bitcast(F32))

            def pc_chunk(n):
                """lhsT [(ik) 128, b] for flat (i,k)-chunk n (=hw*2+chalf)."""
                hw, chalf = n // 2, n % 2
                base = pc2[chalf]
                return _ap(base, hw, [base.ap[0], [36, B_LOC]])

            # ---------------- Phase 4: s1 = 0.1 * sum_i u_hat ----------------
            with (
                tc.tile_pool(name="s1ps", bufs=1, space="PSUM") as s1psp,
                tc.tile_pool(name="wts1", bufs=3) as wts1,
            ):
                s1ps = s1psp.tile([B_LOC, 160], F32, tag="s1ps")
                for sc in range(6):
                    wtc = wts1.tile([128, 12, 160], F32R, tag="wtc", name="wtc")
                    nc.sync.dma_start(out=wtc, in_=wtd[sc])
                    for m in range(12):
                        n = sc * 12 + m
                        nc.tensor.matmul(
                            s1ps,
                            pc_chunk(n),
                            wtc[:, m, :],
                            start=(n == 0),
                            stop=(n == 71),
                        )
                nc.scalar.mul(out=s1, in_=s1ps, mul=0.1)
            if debug:
                nc.sync.dma_start(out=dbg["dbg_s1"], in_=s1)

            def squash16(out_t, in_t, tmp_pool, nj):
                ssq = tmp_pool.tile([B_LOC, nj], F32, tag="sq_ssq")
                prod = tmp_pool.tile([B_LOC, nj * 16], F32, tag="sq_prod")
                nc.vector.tensor_mul(prod, in_t, in_t)
                nc.vector.reduce_sum(
                    ssq, prod.rearrange("p (j d) -> p j d", d=16), axis=AX.X
                )
                aa = tmp_pool.tile([B_LOC, nj], F32, tag="sq_a")
                rr = tmp_pool.tile([B_LOC, nj], F32, tag="sq_r")
                ff = tmp_pool.tile([B_LOC, nj], F32, tag="sq_f")
                nc.vector.tensor_scalar_add(aa, ssq, 1.0)
                nc.vector.reciprocal(rr, aa)
                nc.vector.tensor_mul(rr, ssq, rr)
                nc.scalar.activation(
                    out=aa, in_=ssq, func=AF.Sqrt, bias=epsb[:B_LOC, :], scale=1.0
                )
                nc.vector.reciprocal(aa, aa)
                nc.vector.tensor_mul(ff, rr, aa)
                fb = _ap(ff, 0, [ff.ap[0], [1, nj], [0, 16]])
                nc.vector.tensor_tensor(
                    out=out_t.rearrange("p (j d) -> p j d", d=16),
                    in0=in_t.rearrange("p (j d) -> p j d", d=16),
                    in1=fb,
                    op=ALU.mult,
                )

            with tc.tile_pool(name="rwork", bufs=1) as rw:
                squash16(v1, s1, rw, 10)
                if debug:
                    nc.sync.dma_start(out=dbg["dbg_v1"], in_=v1)

                # ---- v1 block-diagonal [ (j,d), (j',b) ] in bf16 ----
                with tc.tile_pool(name="tps", bufs=1, space="PSUM") as tps:
                    blkA = rw.tile([128, 160], BF16D, tag="blkA")
                    blkB = rw.tile([32, 160], BF16D, tag="blkB")
                    nc.vector.memset(blkA, 0.0)
                    nc.vector.memset(blkB, 0.0)
                    t1 = tps.tile([128, 16], F32, tag="t1")
                    nc.tensor.transpose(t1, v1[:, 0:128], ident[:16, :16])
                    t2 = tps.tile([32, 16], F32, tag="t2")
                    nc.tensor.transpose(t2, v1[:, 128:160], ident[:16, :16])
                    v1T1 = rw.tile([128, 16], BF16D, tag="v1T1")
                    v1T2 = rw.tile([32, 16], BF16D, tag="v1T2")
                    nc.scalar.copy(out=v1T1, in_=t1)
                    nc.scalar.copy(out=v1T2, in_=t2)
                    for j in range(8):
                        nc.sync.dma_start(
                            out=blkA[j * 16 : (j + 1) * 16, j * 16 : (j + 1) * 16],
                            in_=v1T1[j * 16 : (j + 1) * 16, :],
                        )
                    for j in range(8, 10):
                        nc.sync.dma_start(
                            out=blkB[(j - 8) * 16 : (j - 7) * 16, j * 16 : (j + 1) * 16],
                            in_=v1T2[(j - 8) * 16 : (j - 7) * 16, :],
                        )

                    # ---- A[(j',b), (ik)] = sum_d v1 * W  (bf16, full rate) ----
                    A1 = rw.tile([128, 9216], BF16D, tag="A1")
                    A2 = rw.tile([32, 9216], BF16D, tag="A2")
                    with (
                        tc.tile_pool(name="aps", bufs=2, space="PSUM") as aps,
                        tc.tile_pool(name="wt2s", bufs=1) as wt2s,
                    ):
                        for half in range(2):
                            hsl = slice(half * 4608, (half + 1) * 4608)
                            wa = wt2s.tile([128, 4608], BF16D, tag="wa", name="wa")
                            nc.sync.dma_start(out=wa, in_=wt2ad[:, hsl])
                            wb = wt2s.tile([32, 4608], BF16D, tag="wb", name="wb")
                            nc.sync.dma_start(out=wb, in_=wt2bd[:, hsl])
                            for cn in range(9):
                                gsl = slice(
                                    half * 4608 + cn * 512, half * 4608 + (cn + 1) * 512
                                )
                                lsl = slice(cn * 512, (cn + 1) * 512)
                                for mi, (mdst, msl) in enumerate(
                                    [(A1, slice(0, 128)), (A2, slice(128, 160))]
                                ):
                                    ps = aps.tile(
                                        [128 if mi == 0 else 32, 512],
                                        F32,
                                        tag=f"aps{mi}",
                                        name=f"aps{mi}",
                                    )
                                    nc.tensor.matmul(
                                        ps,
                                        blkA[:, msl],
                                        wa[:, lsl],
                                        start=True,
                                        stop=False,
                                    )
                                    nc.tensor.matmul(
                                        ps,
                                        blkB[:, msl],
                                        wb[:, lsl],
                                        start=False,
                                        stop=True,
                                    )
                                    nc.scalar.copy(out=mdst[:, gsl], in_=ps)

                    # ---- pcT [b, (ik)] bf16, built with zero DMAs:
                    # per (h, hw): transpose pc2[:, :, hw] [c128, b16] ->
                    # psum [16, c128] -> ACT copy into pcT slice.
                    pcT = rw.tile([B_LOC, 9216], BF16D, tag="pcT")
                    for hw in range(36):
                        for h in range(2):
                            base = pc2[h]
                            sl = _ap(base, hw, [base.ap[0], [36, B_LOC]])
                            ps = tps.tile([16, 128], F32R, tag="tp36")
                            nc.tensor.transpose(ps, sl, identr)
                            nc.scalar.copy(
                                out=pcT[
                                    :, hw * 256 + h * 128 : hw * 256 + (h + 1) * 128
                                ],
                                in_=ps,
                            )

                # ---- pc_rep [(j,b), (ik)] bf16 ----
                R1 = rw.tile([128, 9216], BF16D, tag="R1")
                R2 = rw.tile([32, 9216], BF16D, tag="R2")
                for j in range(8):
                    nc.sync.dma_start(out=R1[j * 16 : (j + 1) * 16, :], in_=pcT)
                for j in range(2):
                    nc.sync.dma_start(out=R2[j * 16 : (j + 1) * 16, :], in_=pcT)

                if debug:
                    nc.gpsimd.dma_start(out=dbg["dbg_A"][0:128], in_=A1)
                    nc.gpsimd.dma_start(out=dbg["dbg_A"][128:160], in_=A2)
                    nc.gpsimd.dma_start(out=dbg["dbg_pcT"], in_=pcT)
                    nc.gpsimd.dma_start(out=dbg["dbg_R"][0:128], in_=R1)
                    nc.gpsimd.dma_start(out=dbg["dbg_R"][128:160], in_=R2)
                # ---- uv1[(j,b), i] = sum_k A * pc ----
                uv1a = rw.tile([128, 1152], F32, tag="uv1a")
                uv1b = rw.tile([32, 1152], F32, tag="uv1b")
                nc.vector.tensor_mul(A1, A1, R1)
                nc.vector.reduce_sum(
                    uv1a, A1.rearrange("p (i k) -> p i k", k=8), axis=AX.X
                )
                nc.vector.tensor_mul(A2, A2, R2)
                nc.vector.reduce_sum(
                    uv1b, A2.rearrange("p (i k) -> p i k", k=8), axis=AX.X
                )
                if debug:
                    nc.sync.dma_start(out=dbg["dbg_uv1"][0:128], in_=uv1a)
                    nc.sync.dma_start(out=dbg["dbg_uv1"][128:160], in_=uv1b)

                # ---- softmax numerator: exp (bf16; Z folded in during s2) ----
                e1 = rw.tile([128, 1152], BF16D, tag="e1")
                e2 = rw.tile([32, 1152], BF16D, tag="e2")
                nc.scalar.activation(out=e1, in_=uv1a, func=AF.Exp)
                nc.scalar.activation(out=e2, in_=uv1b, func=AF.Exp)
                # replicate exp x8 along free (i -> (i, rep)); reuse A slots
                e1rep = rw.tile([128, 9216], BF16D, tag="A1", name="e1rep")
                e2rep = rw.tile([32, 9216], BF16D, tag="A2", name="e2rep")
                nc.vector.tensor_copy(
                    e1rep.rearrange("p (i r) -> p i r", r=8),
                    _ap(e1, 0, [e1.ap[0], [1, 1152], [0, 8]]),
                )
                nc.vector.tensor_copy(
                    e2rep.rearrange("p (i r) -> p i r", r=8),
                    _ap(e2, 0, [e2.ap[0], [1, 1152], [0, 8]]),
                )

                # ---- s2: accumulate over 72 (ik)-chunks ----
                # per chunk: transpose+replicate exp via permuted-identity
                # matmuls -> psum [ (i,rep)=128, (b,j)=160 ]; Z = reduce over
                # j; g = rep * (1/Z) * pc; two s2 matmuls accumulate.
                with (
                    tc.tile_pool(name="s2w", bufs=4) as s2w,
                    tc.tile_pool(name="wts2", bufs=2) as wts2,
                    tc.tile_pool(name="reps", bufs=4, space="PSUM") as repsp,
                    tc.tile_pool(name="s2ps", bufs=1, space="PSUM") as s2psp,
                ):
                    psA = s2psp.tile([128, 160], F32, tag="psA")
                    psB = s2psp.tile([32, 160], F32, tag="psB")
                    for sc in range(6):
                        wtc2 = wts2.tile([128, 12, 160], F32R, tag="wtc2", name="wtc2")
                        nc.sync.dma_start(out=wtc2, in_=wtd[sc])
                        for m in range(12):
                            n = sc * 12 + m
                            rep = repsp.tile([128, 160], F32, tag="rep", name="rep")
                            nc.tensor.matmul(
                                rep,
                                e1rep[:, 128 * n : 128 * (n + 1)],
                                ie1,
                                start=True,
                                stop=False,
                            )
                            nc.tensor.matmul(
                                rep,
                                e2rep[:, 128 * n : 128 * (n + 1)],
                                ie2,
                                start=False,
                                stop=True,
                            )
                            zc = s2w.tile([128, 16], F32, tag="zc", name="zc")
                            # rep free layout (j,b): strided view -> (b, j)
                            nc.vector.reduce_sum(
                                zc,
                                _ap(rep, 0, [rep.ap[0], [1, 16], [16, 10]]),
                                axis=AX.X,
                            )
                            zr = s2w.tile([128, 16], F32, tag="zr", name="zr")
                            nc.vector.reciprocal(zr, zc)
                            g = s2w.tile([128, 160], F32R, tag="g", name="g")
                            nc.vector.tensor_tensor(
                                out=g.rearrange("p (j b) -> p j b", j=10),
                                in0=rep.rearrange("p (j b) -> p j b", j=10),
                                in1=_ap(zr, 0, [zr.ap[0], [0, 10], [1, 16]]),
                                op=ALU.mult,
                            )
                            pcb = pc_chunk(n)
                            nc.vector.tensor_tensor(
                                out=g.rearrange("p (j b) -> p j b", j=10),
                                in0=g.rearrange("p (j b) -> p j b", j=10),
                                in1=_ap(pcb, 0, [pcb.ap[0], [0, 10], [36, B_LOC]]),
                                op=ALU.mult,
                            )
                            nc.tensor.matmul(
                                psA,
                                g[:, 0:128],
                                wtc2[:, m, :],
                                start=(n == 0),
                                stop=(n == 71),
                            )
                            nc.tensor.matmul(
                                psB,
                                g[:, 128:160],
                                wtc2[:, m, :],
                                start=(n == 0),
                                stop=(n == 71),
                            )
                    # diagonal extraction: psum row m=(j*16+b) -> s2[b, (j,:)]
                    sA = s2w.tile([128, 160], F32, tag="sA")
                    sB = s2w.tile([32, 160], F32, tag="sB")
                    nc.scalar.copy(out=sA, in_=psA)
                    nc.scalar.copy(out=sB, in_=psB)
                    for j in range(8):
                        nc.sync.dma_start(
                            out=s2[:, j * 16 : (j + 1) * 16],
                            in_=sA[j * 16 : (j + 1) * 16, j * 16 : (j + 1) * 16],
                        )
                    for j in range(8, 10):
                        nc.sync.dma_start(
                            out=s2[:, j * 16 : (j + 1) * 16],
                            in_=sB[(j - 8) * 16 : (j - 7) * 16, j * 16 : (j + 1) * 16],
                        )
                if debug:
                    nc.sync.dma_start(out=dbg["dbg_s2"], in_=s2)

                # ---- v2, norm, y_pred, mask ----
                ssq2 = rw.tile([B_LOC, 10], F32, tag="ssq2")
                v2sq = rw.tile([B_LOC, 160], F32, tag="v2sq")
                squash16(v2, s2, rw, 10)
                nc.vector.tensor_mul(v2sq, v2, v2)
                nc.vector.reduce_sum(
                    ssq2, v2sq.rearrange("p (j d) -> p j d", d=16), axis=AX.X
                )
                nc.scalar.activation(
                    out=norm, in_=ssq2, func=AF.Sqrt, bias=epsb[:B_LOC, :], scale=1.0
                )
                nc.sync.dma_start(out=normd, in_=norm)
                if debug:
                    nc.sync.dma_start(out=dbg["dbg_v2"], in_=v2)

                vmax = rw.tile([B_LOC, 8], F32, tag="vmax")
                vidx = rw.tile([B_LOC, 8], U32, tag="vidx")
                nc.vector.max_with_indices(vmax, vidx, norm)
                ypi = rw.tile([B_LOC, 1], I32, tag="ypi")
                nc.vector.tensor_copy(ypi, vidx[:, 0:1])
                nc.sync.dma_start(out=ypredd, in_=ypi)

                tgtf = rw.tile([B_LOC, 1], F32, tag="tgtf")
                tgti = rw.tile([B_LOC, 1], I32, tag="tgti")
                nc.sync.dma_start(out=tgti, in_=tgt)
                nc.vector.tensor_copy(tgtf, tgti)
                mask = rw.tile([B_LOC, 10], F32, tag="mask")
                nc.vector.tensor_scalar(
                    out=mask, in0=iota, scalar1=tgtf, scalar2=None, op0=ALU.is_equal
                )
                nc.vector.tensor_tensor(
                    out=h0.rearrange("p (j d) -> p j d", d=16),
                    in0=v2.rearrange("p (j d) -> p j d", d=16),
                    in1=_ap(mask, 0, [mask.ap[0], [1, 10], [0, 16]]),
                    op=ALU.mult,
                )
                if debug:
                    nc.sync.dma_start(out=dbg["dbg_h0"], in_=h0)

            # ---------------- Phase 5: decoder ----------------
            with (
                tc.tile_pool(name="dps", bufs=2, space="PSUM") as dps,
                tc.tile_pool(name="dwork", bufs=1) as dwk,
            ):
                h0T1 = dwk.tile([128, 16], F32R, tag="h0T1")
                h0T2 = dwk.tile([32, 16], F32R, tag="h0T2")
                ps = dps.tile([128, 16], F32, tag="dtp")
                nc.tensor.transpose(ps, h0[:, 0:128], ident[:16, :16])
                nc.scalar.copy(out=h0T1, in_=ps)
                ps = dps.tile([32, 16], F32, tag="dtp", name="dtp2")
                nc.tensor.transpose(ps, h0[:, 128:160], ident[:16, :16])
                nc.scalar.copy(out=h0T2, in_=ps)

                h1 = dwk.tile([B_LOC, 512], F32, tag="h1")
                hp = dps.tile([B_LOC, 512], F32, tag="mmp")
                nc.tensor.matmul(hp, h0T1, d1w, start=True, stop=False)
                nc.tensor.matmul(hp, h0T2, d1wb, start=False, stop=True)
                nc.vector.tensor_add(h1, hp, b1r)
                nc.scalar.activation(out=h1, in_=h1, func=AF.Relu)

                h1T = dwk.tile([128, 4, 16], F32R, tag="h1T")
                for c in range(4):
                    ps = dps.tile([128, 16], F32, tag="dtp", name="dtp3")
                    nc.tensor.transpose(
                        ps, h1[:, c * 128 : (c + 1) * 128], ident[:16, :16]
                    )
                    nc.scalar.copy(out=h1T[:, c, :], in_=ps)
                h2 = dwk.tile([B_LOC, 1024], F32, tag="h2")
                for nh in range(2):
                    hp2 = dps.tile([B_LOC, 512], F32, tag="mmp", name="hp2")
                    for c in range(4):
                        nc.tensor.matmul(
                            hp2,
                            h1T[:, c, :],
                            d2w[:, c, nh * 512 : (nh + 1) * 512],
                            start=(c == 0),
                            stop=(c == 3),
                        )
                    sl = slice(nh * 512, (nh + 1) * 512)
                    nc.vector.tensor_add(h2[:, sl], hp2, b2r[:, sl])
                    nc.scalar.activation(out=h2[:, sl], in_=h2[:, sl], func=AF.Relu)

                h2T = dwk.tile([128, 8, 16], F32R, tag="h2T")
                for c in range(8):
                    ps = dps.tile([128, 16], F32, tag="dtp", name="dtp4")
                    nc.tensor.transpose(
                        ps, h2[:, c * 128 : (c + 1) * 128], ident[:16, :16]
                    )
                    nc.scalar.copy(out=h2T[:, c, :], in_=ps)
                dec = dwk.tile([B_LOC, 784], F32, tag="dec")
                for nh, (n0, n1) in enumerate([(0, 512), (512, 784)]):
                    hp3 = dps.tile([B_LOC, 512], F32, tag="mmp", name="hp3")[
                        :, : n1 - n0
                    ]
                    for c in range(8):
                        nc.tensor.matmul(
                            hp3,
                            h2T[:, c, :],
                            d3w[:, c, n0:n1],
                            start=(c == 0),
                            stop=(c == 7),
                        )
                    nc.vector.tensor_add(dec[:, n0:n1], hp3, b3r[:, n0:n1])
                nc.scalar.activation(out=dec, in_=dec, func=AF.Sigmoid)
                nc.sync.dma_start(out=decd, in_=dec)

    nc.compile()
    return nc


# ---------------------------------------------------------------------------
# Host side
# ---------------------------------------------------------------------------


def prep_inputs(image, target, conv1_w, conv1_b, conv2_w, conv2_b, W,
                d1_w, d1_b, d2_w, d2_b, do_w, do_b):
    """Full inputs -> list of 8 per-core input maps."""
    image = np.asarray(image, np.float32)
    target = np.asarray(target)
    B = image.shape[0]
    per = B // N_CORES

    # host im2col for conv1: [B, 81, 400]
    img = image[:, :, :, 0]
    sw = np.lib.stride_tricks.sliding_window_view(img, (9, 9), axis=(1, 2))
    # sw: [B, 20, 20, 9, 9] -> [B, (kh kw), (oh ow)]
    pat = np.ascontiguousarray(sw.transpose(0, 3, 4, 1, 2)).reshape(B, 81, 400)

    w1 = np.asarray(conv1_w, np.float32).reshape(81, 256)
    b1 = np.zeros((128, 2), np.float32)
    b1[:, 0] = np.asarray(conv1_b, np.float32)[0:128]
    b1[:, 1] = np.asarray(conv1_b, np.float32)[128:256]
    w2 = (
        np.asarray(conv2_w, np.float32)
        .reshape(81, 2, 128, 2, 128)
        .transpose(0, 2, 1, 3, 4)  # tap, cin_low, cinh, couth, cout_low
        .reshape(27, 3, 128, 2, 2, 128)
        .transpose(0, 2, 1, 3, 4, 5)  # tg, cin_low, tap3, cinh, couth, cout
        .copy()
    )
    b2 = np.zeros((128, 2), np.float32)
    b2[:, 0] = np.asarray(conv2_b, np.float32)[0:128]
    b2[:, 1] = np.asarray(conv2_b, np.float32)[128:256]

    W0 = np.asarray(W, np.float32)[0]  # [1152, 10, 16, 8]
    wt = (
        W0.transpose(0, 3, 1, 2)  # i, k, j, d
        .reshape(6, 12, 128, 160)
        .transpose(0, 2, 1, 3)  # sc, ik_low(128), m, jd
        .copy()
    )
    wt2 = W0.transpose(1, 2, 0, 3).reshape(160, 9216)  # (j,d), (i,k)
    wt2a = wt2[0:128].astype(BF16)
    wt2b = wt2[128:160].astype(BF16)

    d1w = np.asarray(d1_w, np.float32)
    d2w = np.asarray(d2_w, np.float32).reshape(4, 128, 1024)
    d3w = np.asarray(do_w, np.float32).reshape(8, 128, 784)

    shared = dict(
        w1d=w1, b1d=b1, w2d=w2, b2d=b2, wtd=wt, wt2ad=wt2a, wt2bd=wt2b,
        d1wd=d1w, d2wd=d2w, d3wd=d3w,
        d1bd=np.asarray(d1_b, np.float32),
        d2bd=np.asarray(d2_b, np.float32),
        d3bd=np.asarray(do_b, np.float32),
    )
    maps = []
    for c in range(N_CORES):
        m = dict(shared)
        m["patd"] = pat[c * per : (c + 1) * per]
        m["tgt"] = target[c * per : (c + 1) * per].astype(np.int32).reshape(per, 1)
        maps.append(m)
    return maps


_NC_CACHE = {}


def _get_nc(debug=False):
    key = bool(debug)
    if key not in _NC_CACHE:
        _NC_CACHE[key] = build_nc(debug=key)
    return _NC_CACHE[key]


def kernel(**inputs):
    from concourse import bass_utils

    target = np.asarray(inputs["target"])
    maps = prep_inputs(**inputs)
    nc = _get_nc(debug=False)
    res = bass_utils.run_bass_kernel_spmd(nc, maps, core_ids=list(range(N_CORES)))
    outs = res.results
    B = N_CORES * B_LOC
    norm = np.concatenate([o["normd"] for o in outs], 0).reshape(B, 1, 10, 1, 1)
    ypred = np.concatenate([o["ypredd"] for o in outs], 0).reshape(B, 1)
    dec = np.concatenate([o["decd"] for o in outs], 0).reshape(B, 1, 784)
    if target.dtype == np.int64:
        ypred = ypred.astype(np.int64)
    else:
        ypred = ypred.astype(np.int32)
    return norm, ypred, dec


# revision 27
# speedup vs baseline: 1.7691x; 1.0049x over previous
"""CapsuleNetwork Trainium2 Bass kernel.

8-core data-parallel: batch 128 -> 16 per core, all weights replicated.
Per-core pipeline (all on device):
  conv1 (host-im2col patches x W1 matmul) -> conv2 (tap-accumulation
  matmul, fp32r full rate) -> primary-caps squash (partition-group
  sum-of-squares via ones-blockdiag matmul) -> dynamic routing
  (2 rounds; s1 via K=9216 matmul, A-tensor via block-diag-v1 matmul,
  softmax transpose+replicate via permuted-identity matmuls) ->
  norm/argmax/mask -> 3-layer decoder.
"""

import sys

import numpy as np

sys.path.insert(0, "/opt/trn_rl_repo")

import concourse.bass as bass
import concourse.bacc as bacc
import concourse.tile as tile
from concourse import mybir

import ml_dtypes

BF16 = ml_dtypes.bfloat16

EPS = 1e-7
B_LOC = 16  # images per core
N_CORES = 8
F32 = mybir.dt.float32
F32R = mybir.dt.float32r
BF16D = mybir.dt.bfloat16
I32 = mybir.dt.int32
U32 = mybir.dt.uint32
AF = mybir.ActivationFunctionType
ALU = mybir.AluOpType
AX = mybir.AxisListType


def _ap(base, offset_elems, dims):
    """Raw AP at base.offset + offset_elems with explicit [step,count] dims."""
    return bass.AP(tensor=base.tensor, offset=base.offset + offset_elems, ap=dims)


def build_nc(debug=False):
    nc = bacc.Bacc("TRN2", target_bir_lowering=False, debug=False)

    # ---------------- DRAM I/O ----------------
    patd = nc.dram_tensor("patd", [B_LOC, 81, 400], F32R, kind="ExternalInput").ap()
    tgt = nc.dram_tensor("tgt", [B_LOC, 1], I32, kind="ExternalInput").ap()
    w1d = nc.dram_tensor("w1d", [81, 256], F32R, kind="ExternalInput").ap()
    b1d = nc.dram_tensor("b1d", [128, 2], F32, kind="ExternalInput").ap()
    # (tapgroup 27, cin_low 128, tap3, cinh, couth, cout_low)
    w2d = nc.dram_tensor(
        "w2d", [27, 128, 3, 2, 2, 128], F32R, kind="ExternalInput"
    ).ap()
    b2d = nc.dram_tensor("b2d", [128, 2], F32, kind="ExternalInput").ap()
    # (super-chunk 6, cin_low 128, m 12, jd 160)
    wtd = nc.dram_tensor("wtd", [6, 128, 12, 160], F32R, kind="ExternalInput").ap()
    wt2ad = nc.dram_tensor("wt2ad", [128, 9216], BF16D, kind="ExternalInput").ap()
    wt2bd = nc.dram_tensor("wt2bd", [32, 9216], BF16D, kind="ExternalInput").ap()
    d1wd = nc.dram_tensor("d1wd", [160, 512], F32R, kind="ExternalInput").ap()
    d2wd = nc.dram_tensor("d2wd", [4, 128, 1024], F32R, kind="ExternalInput").ap()
    d3wd = nc.dram_tensor("d3wd", [8, 128, 784], F32R, kind="ExternalInput").ap()
    d1bd = nc.dram_tensor("d1bd", [512], F32, kind="ExternalInput").ap()
    d2bd = nc.dram_tensor("d2bd", [1024], F32, kind="ExternalInput").ap()
    d3bd = nc.dram_tensor("d3bd", [784], F32, kind="ExternalInput").ap()

    normd = nc.dram_tensor("normd", [B_LOC, 10], F32, kind="ExternalOutput").ap()
    ypredd = nc.dram_tensor("ypredd", [B_LOC, 1], I32, kind="ExternalOutput").ap()
    decd = nc.dram_tensor("decd", [B_LOC, 784], F32, kind="ExternalOutput").ap()

    dbg = {}
    if debug:
        for name, shape in [
            ("dbg_x1", [2, 128, 16, 400]),
            ("dbg_x2", [2, 128, 16, 36]),
            ("dbg_pc", [2, 128, 16, 36]),
            ("dbg_s1", [B_LOC, 160]),
            ("dbg_v1", [B_LOC, 160]),
            ("dbg_uv1", [160, 1152]),
            ("dbg_A", [160, 9216]),
            ("dbg_pcT", [16, 9216]),
            ("dbg_R", [160, 9216]),
            ("dbg_s2", [B_LOC, 160]),
            ("dbg_v2", [B_LOC, 160]),
            ("dbg_h0", [B_LOC, 160]),
        ]:
            dbg[name] = nc.dram_tensor(name, shape, F32, kind="ExternalOutput").ap()

    # ---------------- constants (embedded in NEFF) ----------------
    ident_np = np.eye(128, dtype=np.float32)
    identd = nc.inline_tensor(ident_np, "identc").ap()
    s_np = np.zeros((128, 16), np.float32)
    for c in range(128):
        s_np[c, c // 8] = 1.0
    sd = nc.inline_tensor(s_np, "sconst").ap()
    iota_np = np.arange(10, dtype=np.float32).reshape(1, 10)
    iotad = nc.inline_tensor(iota_np, "iotac").ap()
    # extended identities: row (j,b) -> col (j*16+b)
    ie1_np = np.zeros((128, 160), np.float32)
    ie1_np[:, 0:128] = np.eye(128)
    ie2_np = np.zeros((32, 160), np.float32)
    ie2_np[:, 128:160] = np.eye(32)
    ie1d = nc.inline_tensor(ie1_np.astype(BF16), "ie1c").ap()
    ie2d = nc.inline_tensor(ie2_np.astype(BF16), "ie2c").ap()

    with tile.TileContext(nc) as tc:
        with tc.tile_pool(name="persist", bufs=1) as pp:
            # ---- persistent tiles / consts ----
            ident = pp.tile([128, 128], F32, tag="ident")
            nc.sync.dma_start(out=ident, in_=identd)
            identr = pp.tile([128, 128], F32R, tag="identr")
            nc.vector.tensor_copy(identr, ident)
            identb = pp.tile([128, 128], BF16D, tag="identb")
            nc.vector.tensor_copy(identb, ident)
            smat = pp.tile([128, 16], F32, tag="smat")
            nc.sync.dma_start(out=smat, in_=sd)
            smat_r = pp.tile([128, 16], F32R, tag="smat_r")
            nc.vector.tensor_copy(smat_r, smat)
            iota = pp.tile([B_LOC, 10], F32, tag="iota")
            nc.sync.dma_start(out=iota, in_=_ap(iotad, 0, [[0, B_LOC], [1, 10]]))
            ie1 = pp.tile([128, 160], BF16D, tag="ie1")
            nc.sync.dma_start(out=ie1, in_=ie1d)
            ie2 = pp.tile([32, 160], BF16D, tag="ie2")
            nc.sync.dma_start(out=ie2, in_=ie2d)
            epsb = pp.tile([128, 1], F32, tag="epsb")
            nc.vector.memset(epsb, EPS)

            w1 = pp.tile([81, 256], F32R, tag="w1")
            nc.sync.dma_start(out=w1, in_=w1d)
            b1 = pp.tile([128, 2], F32, tag="b1")
            nc.sync.dma_start(out=b1, in_=b1d)
            b2 = pp.tile([128, 2], F32, tag="b2")
            nc.sync.dma_start(out=b2, in_=b2d)

            # decoder weights: load once, early (overlaps conv compute)
            d1w = pp.tile([128, 512], F32R, tag="d1w_hi")
            d1wb = pp.tile([32, 512], F32R, tag="d1w_lo")
            nc.sync.dma_start(out=d1w, in_=d1wd[0:128])
            nc.sync.dma_start(out=d1wb, in_=d1wd[128:160])
            d2w = pp.tile([128, 4, 1024], F32R, tag="d2w")
            for c in range(4):
                nc.sync.dma_start(out=d2w[:, c, :], in_=d2wd[c])
            d3w = pp.tile([128, 8, 784], F32R, tag="d3w")
            for c in range(8):
                nc.sync.dma_start(out=d3w[:, c, :], in_=d3wd[c])
            b1r = pp.tile([B_LOC, 512], F32, tag="b1r")
            nc.sync.dma_start(out=b1r, in_=_ap(d1bd, 0, [[0, B_LOC], [1, 512]]))
            b2r = pp.tile([B_LOC, 1024], F32, tag="b2r")
            nc.sync.dma_start(out=b2r, in_=_ap(d2bd, 0, [[0, B_LOC], [1, 1024]]))
            b3r = pp.tile([B_LOC, 784], F32, tag="b3r")
            nc.sync.dma_start(out=b3r, in_=_ap(d3bd, 0, [[0, B_LOC], [1, 784]]))

            # conv2 output (post-relu), layout [cout_low, (b, hw36)] x couth
            x2 = [pp.tile([128, B_LOC, 36], F32, tag=f"x2_{h}", name=f"x2_{h}") for h in range(2)]
            pc2 = [pp.tile([128, B_LOC, 36], F32R, tag=f"pc2_{h}", name=f"pc2_{h}") for h in range(2)]

            s1 = pp.tile([B_LOC, 160], F32, tag="s1")
            v1 = pp.tile([B_LOC, 160], F32, tag="v1")
            s2 = pp.tile([B_LOC, 160], F32, tag="s2")
            v2 = pp.tile([B_LOC, 160], F32, tag="v2")
            h0 = pp.tile([B_LOC, 160], F32, tag="h0")
            norm = pp.tile([B_LOC, 10], F32, tag="norm")

            # ---------------- Phase 1: conv1 ----------------
            with (
                tc.tile_pool(name="c1work", bufs=3) as c1w,
                tc.tile_pool(name="c1ps", bufs=4, space="PSUM") as c1p,
                tc.tile_pool(name="x1pool", bufs=1) as x1pool,
            ):
                x1 = [
                    x1pool.tile([128, B_LOC, 400], F32R, tag=f"x1_{h}", name=f"x1_{h}")
                    for h in range(2)
                ]
                for b in range(B_LOC):
                    patch = c1w.tile([81, 400], F32R, tag="patch")
                    nc.sync.dma_start(out=patch, in_=patd[b])
                    for h in range(2):
                        ps = c1p.tile([128, 400], F32, tag="c1ps")
                        nc.tensor.matmul(
                            ps,
                            w1[:, h * 128 : (h + 1) * 128],
                            patch,
                            start=True,
                            stop=True,
                        )
                        nc.scalar.activation(
                            out=x1[h][:, b, :],
                            in_=ps,
                            func=AF.Relu,
                            bias=b1[:, h : h + 1],
                            scale=1.0,
                        )
                if debug:
                    for h in range(2):
                        nc.gpsimd.dma_start(out=dbg["dbg_x1"][h], in_=x1[h])

                # ---------------- Phase 2: conv2 ----------------
                with (
                    tc.tile_pool(name="w2stream", bufs=3) as w2s,
                    tc.tile_pool(name="qps", bufs=1, space="PSUM") as qps,
                ):
                    q = [
                        [qps.tile([128, 288], F32, tag=f"q_{h}_{g}", name=f"q_{h}_{g}") for g in range(2)]
                        for h in range(2)
                    ]
                    for tg in range(27):
                        w2t = w2s.tile([128, 3, 2, 2, 128], F32R, tag="w2t")
                        nc.sync.dma_start(out=w2t, in_=w2d[tg])
                        for t3 in range(3):
                            tap = tg * 3 + t3
                            kh, kw = tap // 9, tap % 9
                            for cinh in range(2):
                                base = x1[cinh]
                                for couth in range(2):
                                    lhsT = w2t[:, t3, cinh, couth, :]
                                    for g in range(2):
                                        off = (g * 8) * 400 + kh * 20 + kw
                                        rhs = _ap(
                                            base,
                                            off,
                                            [base.ap[0], [400, 8], [40, 6], [2, 6]],
                                        )
                                        nc.tensor.matmul(
                                            q[couth][g],
                                            lhsT,
                                            rhs,
                                            start=(tap == 0 and cinh == 0),
                                            stop=(tap == 80 and cinh == 1),
                                        )
                    for couth in range(2):
                        for g in range(2):
                            nc.scalar.activation(
                                out=x2[couth][:, g * 8 : (g + 1) * 8, :],
                                in_=q[couth][g].rearrange("p (b f) -> p b f", f=36),
                                func=AF.Relu,
                                bias=b2[:, couth : couth + 1],
                                scale=1.0,
                            )
            if debug:
                for h in range(2):
                    nc.sync.dma_start(out=dbg["dbg_x2"][h], in_=x2[h])

            # ---------------- Phase 3: squash -> pc2 ----------------
            with (
                tc.tile_pool(name="sqw", bufs=1) as sqw,
                tc.tile_pool(name="sqps", bufs=4, space="PSUM") as sqps,
            ):
                sq = [sqw.tile([128, B_LOC * 36], F32R, tag=f"sq_{h}", name=f"sq_{h}") for h in range(2)]
                for h in range(2):
                    flat = x2[h].rearrange("p b f -> p (b f)")
                    nc.vector.tensor_mul(sq[h], flat, flat)
                    ssq = sqw.tile([B_LOC, 576], F32, tag="ssq", name="ssq", bufs=2)
                    for g in range(2):
                        ps = sqps.tile([16, 288], F32, tag="sqps", name="sqps")
                        nc.tensor.matmul(
                            ps,
                            smat_r,
                            sq[h][:, g * 288 : (g + 1) * 288],
                            start=True,
                            stop=True,
                        )
                        nc.scalar.copy(
                            out=ssq[:, g * 288 : (g + 1) * 288],
                            in_=ps,
                        )
                    # f = (ssq/(1+ssq)) / sqrt(ssq+eps)
                    a = sqw.tile([B_LOC, 576], F32, tag="fa", name="fa", bufs=2)
                    r = sqw.tile([B_LOC, 576], F32, tag="fr", name="fr", bufs=2)
                    fT = sqw.tile([B_LOC, 576], F32, tag="fT", name="fT", bufs=2)
                    nc.vector.tensor_scalar_add(a, ssq, 1.0)
                    nc.vector.reciprocal(r, a)
                    nc.vector.tensor_mul(r, ssq, r)
                    nc.scalar.activation(
                        out=a, in_=ssq, func=AF.Sqrt, bias=epsb[:B_LOC, :], scale=1.0
                    )
                    nc.vector.reciprocal(a, a)
                    nc.vector.tensor_mul(fT, r, a)
                    fe = sqw.tile([128, 576], F32, tag="fexp", name="fexp", bufs=2)
                    nc.sync.dma_start(
                        out=fe, in_=_ap(fT, 0, [fT.ap[0], [0, 8], [1, 576]])
                    )
                    nc.vector.tensor_mul(
                        pc2[h].rearrange("p b f -> p (b f)"),
                        x2[h].rearrange("p b f -> p (b f)"),
                        fe,
                    )
            if debug:
                for h in range(2):
                    nc.gpsimd.dma_start(out=dbg["dbg_pc"][h], in_=pc2[h])

            def pc_chunk(n):
                """lhsT [(ik) 128, b] for flat (i,k)-chunk n (=hw*2+chalf)."""
                hw, chalf = n // 2, n % 2
                base = pc2[chalf]
                return _ap(base, hw, [base.ap[0], [36, B_LOC]])

            # ---------------- Phase 4: s1 = 0.1 * sum_i u_hat ----------------
            with (
                tc.tile_pool(name="s1ps", bufs=1, space="PSUM") as s1psp,
                tc.tile_pool(name="wts1", bufs=3) as wts1,
            ):
                s1ps = s1psp.tile([B_LOC, 160], F32, tag="s1ps")
                for sc in range(6):
                    wtc = wts1.tile([128, 12, 160], F32R, tag="wtc", name="wtc")
                    nc.sync.dma_start(out=wtc, in_=wtd[sc])
                    for m in range(12):
                        n = sc * 12 + m
                        nc.tensor.matmul(
                            s1ps,
                            pc_chunk(n),
                            wtc[:, m, :],
                            start=(n == 0),
                            stop=(n == 71),
                        )
                nc.scalar.mul(out=s1, in_=s1ps, mul=0.1)
            if debug:
                nc.sync.dma_start(out=dbg["dbg_s1"], in_=s1)

            def squash16(out_t, in_t, tmp_pool, nj):
                ssq = tmp_pool.tile([B_LOC, nj], F32, tag="sq_ssq")
                prod = tmp_pool.tile([B_LOC, nj * 16], F32, tag="sq_prod")
                nc.vector.tensor_mul(prod, in_t, in_t)
                nc.vector.reduce_sum(
                    ssq, prod.rearrange("p (j d) -> p j d", d=16), axis=AX.X
                )
                aa = tmp_pool.tile([B_LOC, nj], F32, tag="sq_a")
                rr = tmp_pool.tile([B_LOC, nj], F32, tag="sq_r")
                ff = tmp_pool.tile([B_LOC, nj], F32, tag="sq_f")
                nc.vector.tensor_scalar_add(aa, ssq, 1.0)
                nc.vector.reciprocal(rr, aa)
                nc.vector.tensor_mul(rr, ssq, rr)
                nc.scalar.activation(
                    out=aa, in_=ssq, func=AF.Sqrt, bias=epsb[:B_LOC, :], scale=1.0
                )
                nc.vector.reciprocal(aa, aa)
                nc.vector.tensor_mul(ff, rr, aa)
                fb = _ap(ff, 0, [ff.ap[0], [1, nj], [0, 16]])
                nc.vector.tensor_tensor(
                    out=out_t.rearrange("p (j d) -> p j d", d=16),
                    in0=in_t.rearrange("p (j d) -> p j d", d=16),
                    in1=fb,
                    op=ALU.mult,
                )

            with tc.tile_pool(name="rwork", bufs=1) as rw:
                squash16(v1, s1, rw, 10)
                if debug:
                    nc.sync.dma_start(out=dbg["dbg_v1"], in_=v1)

                # ---- v1 block-diagonal [ (j,d), (j',b) ] in bf16 ----
                with tc.tile_pool(name="tps", bufs=1, space="PSUM") as tps:
                    blkA = rw.tile([128, 160], BF16D, tag="blkA")
                    blkB = rw.tile([32, 160], BF16D, tag="blkB")
                    nc.vector.memset(blkA, 0.0)
                    nc.vector.memset(blkB, 0.0)
                    t1 = tps.tile([128, 16], F32, tag="t1")
                    nc.tensor.transpose(t1, v1[:, 0:128], ident[:16, :16])
                    t2 = tps.tile([32, 16], F32, tag="t2")
                    nc.tensor.transpose(t2, v1[:, 128:160], ident[:16, :16])
                    v1T1 = rw.tile([128, 16], BF16D, tag="v1T1")
                    v1T2 = rw.tile([32, 16], BF16D, tag="v1T2")
                    nc.scalar.copy(out=v1T1, in_=t1)
                    nc.scalar.copy(out=v1T2, in_=t2)
                    for j in range(8):
                        nc.sync.dma_start(
                            out=blkA[j * 16 : (j + 1) * 16, j * 16 : (j + 1) * 16],
                            in_=v1T1[j * 16 : (j + 1) * 16, :],
                        )
                    for j in range(8, 10):
                        nc.sync.dma_start(
                            out=blkB[(j - 8) * 16 : (j - 7) * 16, j * 16 : (j + 1) * 16],
                            in_=v1T2[(j - 8) * 16 : (j - 7) * 16, :],
                        )

                    # ---- A[(j',b), (ik)] = sum_d v1 * W  (bf16, full rate) ----
                    A1 = rw.tile([128, 9216], BF16D, tag="A1")
                    A2 = rw.tile([32, 9216], BF16D, tag="A2")
                    with (
                        tc.tile_pool(name="aps", bufs=2, space="PSUM") as aps,
                        tc.tile_pool(name="wt2s", bufs=1) as wt2s,
                    ):
                        for half in range(2):
                            hsl = slice(half * 4608, (half + 1) * 4608)
                            wa = wt2s.tile([128, 4608], BF16D, tag="wa", name="wa")
                            nc.sync.dma_start(out=wa, in_=wt2ad[:, hsl])
                            wb = wt2s.tile([32, 4608], BF16D, tag="wb", name="wb")
                            nc.sync.dma_start(out=wb, in_=wt2bd[:, hsl])
                            for cn in range(9):
                                gsl = slice(
                                    half * 4608 + cn * 512, half * 4608 + (cn + 1) * 512
                                )
                                lsl = slice(cn * 512, (cn + 1) * 512)
                                for mi, (mdst, msl) in enumerate(
                                    [(A1, slice(0, 128)), (A2, slice(128, 160))]
                                ):
                                    ps = aps.tile(
                                        [128 if mi == 0 else 32, 512],
                                        F32,
                                        tag=f"aps{mi}",
                                        name=f"aps{mi}",
                                    )
                                    nc.tensor.matmul(
                                        ps,
                                        blkA[:, msl],
                                        wa[:, lsl],
                                        start=True,
                                        stop=False,
                                    )
                                    nc.tensor.matmul(
                                        ps,
                                        blkB[:, msl],
                                        wb[:, lsl],
                                        start=False,
                                        stop=True,
                                    )
                                    nc.scalar.copy(out=mdst[:, gsl], in_=ps)

                    # ---- pcT [b, (ik)] bf16, built with zero DMAs:
                    # per (h, hw): transpose pc2[:, :, hw] [c128, b16] ->
                    # psum [16, c128] -> ACT copy into pcT slice.
                    pcT = rw.tile([B_LOC, 9216], BF16D, tag="pcT")
                    for hw in range(36):
                        for h in range(2):
                            base = pc2[h]
                            sl = _ap(base, hw, [base.ap[0], [36, B_LOC]])
                            ps = tps.tile([16, 128], F32R, tag="tp36")
                            nc.tensor.transpose(ps, sl, identr)
                            nc.scalar.copy(
                                out=pcT[
                                    :, hw * 256 + h * 128 : hw * 256 + (h + 1) * 128
                                ],
                                in_=ps,
                            )

                # ---- pc_rep [(j,b), (ik)] bf16 ----
                R1 = rw.tile([128, 9216], BF16D, tag="R1")
                R2 = rw.tile([32, 9216], BF16D, tag="R2")
                for j in range(8):
                    nc.sync.dma_start(out=R1[j * 16 : (j + 1) * 16, :], in_=pcT)
                for j in range(2):
                    nc.sync.dma_start(out=R2[j * 16 : (j + 1) * 16, :], in_=pcT)

                if debug:
                    nc.gpsimd.dma_start(out=dbg["dbg_A"][0:128], in_=A1)
                    nc.gpsimd.dma_start(out=dbg["dbg_A"][128:160], in_=A2)
                    nc.gpsimd.dma_start(out=dbg["dbg_pcT"], in_=pcT)
                    nc.gpsimd.dma_start(out=dbg["dbg_R"][0:128], in_=R1)
                    nc.gpsimd.dma_start(out=dbg["dbg_R"][128:160], in_=R2)
                # ---- uv1[(j,b), i] = sum_k A * pc ----
                uv1a = rw.tile([128, 1152], F32, tag="uv1a")
                uv1b = rw.tile([32, 1152], F32, tag="uv1b")
                nc.vector.tensor_mul(A1, A1, R1)
                nc.vector.reduce_sum(
                    uv1a, A1.rearrange("p (i k) -> p i k", k=8), axis=AX.X
                )
                nc.vector.tensor_mul(A2, A2, R2)
                nc.vector.reduce_sum(
                    uv1b, A2.rearrange("p (i k) -> p i k", k=8), axis=AX.X
                )
                if debug:
                    nc.sync.dma_start(out=dbg["dbg_uv1"][0:128], in_=uv1a)
                    nc.sync.dma_start(out=dbg["dbg_uv1"][128:160], in_=uv1b)

                # ---- softmax numerator: exp (bf16; Z folded in during s2) ----
                e1 = rw.tile([128, 1152], BF16D, tag="e1")
                e2 = rw.tile([32, 1152], BF16D, tag="e2")
                nc.scalar.activation(out=e1, in_=uv1a, func=AF.Exp)
                nc.scalar.activation(out=e2, in_=uv1b, func=AF.Exp)
                # replicate exp x8 along free (i -> (i, rep)); reuse A slots
                e1rep = rw.tile([128, 9216], BF16D, tag="A1", name="e1rep")
                e2rep = rw.tile([32, 9216], BF16D, tag="A2", name="e2rep")
                nc.vector.tensor_copy(
                    e1rep.rearrange("p (i r) -> p i r", r=8),
                    _ap(e1, 0, [e1.ap[0], [1, 1152], [0, 8]]),
                )
                nc.vector.tensor_copy(
                    e2rep.rearrange("p (i r) -> p i r", r=8),
                    _ap(e2, 0, [e2.ap[0], [1, 1152], [0, 8]]),
                )

                # ---- s2: accumulate over 72 (ik)-chunks ----
                # per chunk: transpose+replicate exp via permuted-identity
                # matmuls -> psum [ (i,rep)=128, (b,j)=160 ]; Z = reduce over
                # j; g = rep * (1/Z) * pc; two s2 matmuls accumulate.
                with (
                    tc.tile_pool(name="s2w", bufs=4) as s2w,
                    tc.tile_pool(name="wts2", bufs=2) as wts2,
                    tc.tile_pool(name="reps", bufs=4, space="PSUM") as repsp,
                    tc.tile_pool(name="s2ps", bufs=1, space="PSUM") as s2psp,
                ):
                    psA = s2psp.tile([128, 160], F32, tag="psA")
                    psB = s2psp.tile([32, 160], F32, tag="psB")
                    for sc in range(6):
                        wtc2 = wts2.tile([128, 12, 160], F32R, tag="wtc2", name="wtc2")
                        nc.sync.dma_start(out=wtc2, in_=wtd[sc])
                        for m in range(12):
                            n = sc * 12 + m
                            rep = repsp.tile([128, 160], F32, tag="rep", name="rep")
                            nc.tensor.matmul(
                                rep,
                                e1rep[:, 128 * n : 128 * (n + 1)],
                                ie1,
                                start=True,
                                stop=False,
                            )
                            nc.tensor.matmul(
                                rep,
                                e2rep[:, 128 * n : 128 * (n + 1)],
                                ie2,
                                start=False,
                                stop=True,
                            )
                            zc = s2w.tile([128, 16], F32, tag="zc", name="zc")
                            # rep free layout (j,b): strided view -> (b, j)
                            nc.vector.reduce_sum(
                                zc,
                                _ap(rep, 0, [rep.ap[0], [1, 16], [16, 10]]),
                                axis=AX.X,
                            )
                            zr = s2w.tile([128, 16], F32, tag="zr", name="zr")
                            nc.vector.reciprocal(zr, zc)
                            g = s2w.tile([128, 160], F32R, tag="g", name="g")
                            nc.vector.tensor_tensor(
                                out=g.rearrange("p (j b) -> p j b", j=10),
                                in0=rep.rearrange("p (j b) -> p j b", j=10),
                                in1=_ap(zr, 0, [zr.ap[0], [0, 10], [1, 16]]),
                                op=ALU.mult,
                            )
                            pcb = pc_chunk(n)
                            nc.vector.tensor_tensor(
                                out=g.rearrange("p (j b) -> p j b", j=10),
                                in0=g.rearrange("p (j b) -> p j b", j=10),
                                in1=_ap(pcb, 0, [pcb.ap[0], [0, 10], [36, B_LOC]]),
                                op=ALU.mult,
                            )
                            nc.tensor.matmul(
                                psA,
                                g[:, 0:128],
                                wtc2[:, m, :],
                                start=(n == 0),
                                stop=(n == 71),
                            )
                            nc.tensor.matmul(
                                psB,
                                g[:, 128:160],
                                wtc2[:, m, :],
                                start=(n == 0),
                                stop=(n == 71),
                            )
                    # diagonal extraction: psum row m=(j*16+b) -> s2[b, (j,:)]
                    sA = s2w.tile([128, 160], F32, tag="sA")
                    sB = s2w.tile([32, 160], F32, tag="sB")
                    nc.scalar.copy(out=sA, in_=psA)
                    nc.scalar.copy(out=sB, in_=psB)
                    for j in range(8):
                        nc.sync.dma_start(
                            out=s2[:, j * 16 : (j + 1) * 16],
                            in_=sA[j * 16 : (j + 1) * 16, j * 16 : (j + 1) * 16],
                        )
                    for j in range(8, 10):
                        nc.sync.dma_start(
                            out=s2[:, j * 16 : (j + 1) * 16],
                            in_=sB[(j - 8) * 16 : (j - 7) * 16, j * 16 : (j + 1) * 16],
                        )
                if debug:
                    nc.sync.dma_start(out=dbg["dbg_s2"], in_=s2)

                # ---- v2, norm, y_pred, mask ----
                ssq2 = rw.tile([B_LOC, 10], F32, tag="ssq2")
                v2sq = rw.tile([B_LOC, 160], F32, tag="v2sq")
                squash16(v2, s2, rw, 10)
                nc.vector.tensor_mul(v2sq, v2, v2)
                nc.vector.reduce_sum(
                    ssq2, v2sq.rearrange("p (j d) -> p j d", d=16), axis=AX.X
                )
                nc.scalar.activation(
                    out=norm, in_=ssq2, func=AF.Sqrt, bias=epsb[:B_LOC, :], scale=1.0
                )
                nc.sync.dma_start(out=normd, in_=norm)
                if debug:
                    nc.sync.dma_start(out=dbg["dbg_v2"], in_=v2)

                vmax = rw.tile([B_LOC, 8], F32, tag="vmax")
                vidx = rw.tile([B_LOC, 8], U32, tag="vidx")
                nc.vector.max_with_indices(vmax, vidx, norm)
                ypi = rw.tile([B_LOC, 1], I32, tag="ypi")
                nc.vector.tensor_copy(ypi, vidx[:, 0:1])
                nc.sync.dma_start(out=ypredd, in_=ypi)

                tgtf = rw.tile([B_LOC, 1], F32, tag="tgtf")
                tgti = rw.tile([B_LOC, 1], I32, tag="tgti")
                nc.sync.dma_start(out=tgti, in_=tgt)
                nc.vector.tensor_copy(tgtf, tgti)
                mask = rw.tile([B_LOC, 10], F32, tag="mask")
                nc.vector.tensor_scalar(
                    out=mask, in0=iota, scalar1=tgtf, scalar2=None, op0=ALU.is_equal
                )
                nc.vector.tensor_tensor(
                    out=h0.rearrange("p (j d) -> p j d", d=16),
                    in0=v2.rearrange("p (j d) -> p j d", d=16),
                    in1=_ap(mask, 0, [mask.ap[0], [1, 10], [0, 16]]),
                    op=ALU.mult,
                )
                if debug:
                    nc.sync.dma_start(out=dbg["dbg_h0"], in_=h0)

            # ---------------- Phase 5: decoder ----------------
            with (
                tc.tile_pool(name="dps", bufs=2, space="PSUM") as dps,
                tc.tile_pool(name="dwork", bufs=1) as dwk,
            ):
                h0T1 = dwk.tile([128, 16], F32R, tag="h0T1")
                h0T2 = dwk.tile([32, 16], F32R, tag="h0T2")
                ps = dps.tile([128, 16], F32, tag="dtp")
                nc.tensor.transpose(ps, h0[:, 0:128], ident[:16, :16])
                nc.scalar.copy(out=h0T1, in_=ps)
                ps = dps.tile([32, 16], F32, tag="dtp", name="dtp2")
                nc.tensor.transpose(ps, h0[:, 128:160], ident[:16, :16])
                nc.scalar.copy(out=h0T2, in_=ps)

                h1 = dwk.tile([B_LOC, 512], F32, tag="h1")
                hp = dps.tile([B_LOC, 512], F32, tag="mmp")
                nc.tensor.matmul(hp, h0T1, d1w, start=True, stop=False)
                nc.tensor.matmul(hp, h0T2, d1wb, start=False, stop=True)
                nc.vector.tensor_add(h1, hp, b1r)
                nc.scalar.activation(out=h1, in_=h1, func=AF.Relu)

                h1T = dwk.tile([128, 4, 16], F32R, tag="h1T")
                for c in range(4):
                    ps = dps.tile([128, 16], F32, tag="dtp", name="dtp3")
                    nc.tensor.transpose(
                        ps, h1[:, c * 128 : (c + 1) * 128], ident[:16, :16]
                    )
                    nc.scalar.copy(out=h1T[:, c, :], in_=ps)
                h2 = dwk.tile([B_LOC, 1024], F32, tag="h2")
                for nh in range(2):
                    hp2 = dps.tile([B_LOC, 512], F32, tag="mmp", name="hp2")
                    for c in range(4):
                        nc.tensor.matmul(
                            hp2,
                            h1T[:, c, :],
                            d2w[:, c, nh * 512 : (nh + 1) * 512],
                            start=(c == 0),
                            stop=(c == 3),
                        )
                    sl = slice(nh * 512, (nh + 1) * 512)
                    nc.vector.tensor_add(h2[:, sl], hp2, b2r[:, sl])
                    nc.scalar.activation(out=h2[:, sl], in_=h2[:, sl], func=AF.Relu)

                h2T = dwk.tile([128, 8, 16], F32R, tag="h2T")
                for c in range(8):
                    ps = dps.tile([128, 16], F32, tag="dtp", name="dtp4")
                    nc.tensor.transpose(
                        ps, h2[:, c * 128 : (c + 1) * 128], ident[:16, :16]
                    )
                    nc.scalar.copy(out=h2T[:, c, :], in_=ps)
                dec = dwk.tile([B_LOC, 784], F32, tag="dec")
                for nh, (n0, n1) in enumerate([(0, 512), (512, 784)]):
                    hp3 = dps.tile([B_LOC, 512], F32, tag="mmp", name="hp3")[
                        :, : n1 - n0
                    ]
                    for c in range(8):
                        nc.tensor.matmul(
                            hp3,
                            h2T[:, c, :],
                            d3w[:, c, n0:n1],
                            start=(c == 0),
                            stop=(c == 7),
                        )
                    nc.vector.tensor_add(dec[:, n0:n1], hp3, b3r[:, n0:n1])
                nc.scalar.activation(out=dec, in_=dec, func=AF.Sigmoid)
                nc.sync.dma_start(out=decd, in_=dec)

    nc.compile()
    return nc


# ---------------------------------------------------------------------------
# Host side
# ---------------------------------------------------------------------------


def prep_inputs(image, target, conv1_w, conv1_b, conv2_w, conv2_b, W,
                d1_w, d1_b, d2_w, d2_b, do_w, do_b):
    """Full inputs -> list of 8 per-core input maps."""
    image = np.asarray(image, np.float32)
    target = np.asarray(target)
    B = image.shape[0]
    per = B // N_CORES

    # host im2col for conv1: [B, 81, 400]
    img = image[:, :, :, 0]
    sw = np.lib.stride_tricks.sliding_window_view(img, (9, 9), axis=(1, 2))
    # sw: [B, 20, 20, 9, 9] -> [B, (kh kw), (oh ow)]
    pat = np.ascontiguousarray(sw.transpose(0, 3, 4, 1, 2)).reshape(B, 81, 400)

    w1 = np.asarray(conv1_w, np.float32).reshape(81, 256)
    b1 = np.zeros((128, 2), np.float32)
    b1[:, 0] = np.asarray(conv1_b, np.float32)[0:128]
    b1[:, 1] = np.asarray(conv1_b, np.float32)[128:256]
    w2 = (
        np.asarray(conv2_w, np.float32)
        .reshape(81, 2, 128, 2, 128)
        .transpose(0, 2, 1, 3, 4)  # tap, cin_low, cinh, couth, cout_low
        .reshape(27, 3, 128, 2, 2, 128)
        .transpose(0, 2, 1, 3, 4, 5)  # tg, cin_low, tap3, cinh, couth, cout
        .copy()
    )
    b2 = np.zeros((128, 2), np.float32)
    b2[:, 0] = np.asarray(conv2_b, np.float32)[0:128]
    b2[:, 1] = np.asarray(conv2_b, np.float32)[128:256]

    W0 = np.asarray(W, np.float32)[0]  # [1152, 10, 16, 8]
    wt = (
        W0.transpose(0, 3, 1, 2)  # i, k, j, d
        .reshape(6, 12, 128, 160)
        .transpose(0, 2, 1, 3)  # sc, ik_low(128), m, jd
        .copy()
    )
    wt2 = W0.transpose(1, 2, 0, 3).reshape(160, 9216)  # (j,d), (i,k)
    wt2a = wt2[0:128].astype(BF16)
    wt2b = wt2[128:160].astype(BF16)

    d1w = np.asarray(d1_w, np.float32)
    d2w = np.asarray(d2_w, np.float32).reshape(4, 128, 1024)
    d3w = np.asarray(do_w, np.float32).reshape(8, 128, 784)

    shared = dict(
        w1d=w1, b1d=b1, w2d=w2, b2d=b2, wtd=wt, wt2ad=wt2a, wt2bd=wt2b,
        d1wd=d1w, d2wd=d2w, d3wd=d3w,
        d1bd=np.asarray(d1_b, np.float32),
        d2bd=np.asarray(d2_b, np.float32),
        d3bd=np.asarray(do_b, np.float32),
    )
    maps = []
    for c in range(N_CORES):
        m = dict(shared)
        m["patd"] = pat[c * per : (c + 1) * per]
        m["tgt"] = target[c * per : (c + 1) * per].astype(np.int32).reshape(per, 1)
        maps.append(m)
    return maps


_NC_CACHE = {}


def _get_nc(debug=False):
    key = bool(debug)
    if key not in _NC_CACHE:
        _NC_CACHE[key] = build_nc(debug=key)
    return _NC_CACHE[key]


def kernel(**inputs):
    from concourse import bass_utils

    target = np.asarray(inputs["target"])
    maps = prep_inputs(**inputs)
    nc = _get_nc(debug=False)
    res = bass_utils.run_bass_kernel_spmd(nc, maps, core_ids=list(range(N_CORES)))
    outs = res.results
    B = N_CORES * B_LOC
    norm = np.concatenate([o["normd"] for o in outs], 0).reshape(B, 1, 10, 1, 1)
    ypred = np.concatenate([o["ypredd"] for o in outs], 0).reshape(B, 1)
    dec = np.concatenate([o["decd"] for o in outs], 0).reshape(B, 1, 784)
    if target.dtype == np.int64:
        ypred = ypred.astype(np.int64)
    else:
        ypred = ypred.astype(np.int32)
    return norm, ypred, dec


# revision 29
# speedup vs baseline: 1.8523x; 1.0470x over previous
"""CapsuleNetwork Trainium2 Bass kernel.

8-core data-parallel: batch 128 -> 16 per core, all weights replicated.
Per-core pipeline (all on device):
  conv1 (host-im2col patches x W1 matmul) -> conv2 (tap-accumulation
  matmul, fp32r full rate) -> primary-caps squash (partition-group
  sum-of-squares via ones-blockdiag matmul) -> dynamic routing
  (2 rounds; s1 via K=9216 matmul, A-tensor via block-diag-v1 matmul,
  softmax transpose+replicate via permuted-identity matmuls) ->
  norm/argmax/mask -> 3-layer decoder.
"""

import sys

import numpy as np

sys.path.insert(0, "/opt/trn_rl_repo")

import concourse.bass as bass
import concourse.bacc as bacc
import concourse.tile as tile
from concourse import mybir

import ml_dtypes

BF16 = ml_dtypes.bfloat16

EPS = 1e-7
B_LOC = 16  # images per core
N_CORES = 8
F32 = mybir.dt.float32
F32R = mybir.dt.float32r
BF16D = mybir.dt.bfloat16
I32 = mybir.dt.int32
U32 = mybir.dt.uint32
AF = mybir.ActivationFunctionType
ALU = mybir.AluOpType
AX = mybir.AxisListType


def _ap(base, offset_elems, dims):
    """Raw AP at base.offset + offset_elems with explicit [step,count] dims."""
    return bass.AP(tensor=base.tensor, offset=base.offset + offset_elems, ap=dims)


def build_nc(debug=False):
    nc = bacc.Bacc("TRN2", target_bir_lowering=False, debug=False)

    # ---------------- DRAM I/O ----------------
    patd = nc.dram_tensor("patd", [B_LOC, 81, 400], F32R, kind="ExternalInput").ap()
    tgt = nc.dram_tensor("tgt", [B_LOC, 1], I32, kind="ExternalInput").ap()
    w1d = nc.dram_tensor("w1d", [81, 256], F32R, kind="ExternalInput").ap()
    b1d = nc.dram_tensor("b1d", [128, 2], F32, kind="ExternalInput").ap()
    # (tapgroup 27, cin_low 128, tap3, cinh, couth, cout_low)
    w2d = nc.dram_tensor(
        "w2d", [27, 128, 3, 2, 2, 128], F32R, kind="ExternalInput"
    ).ap()
    b2d = nc.dram_tensor("b2d", [128, 2], F32, kind="ExternalInput").ap()
    # (super-chunk 6, cin_low 128, m 12, jd 160)
    wtd = nc.dram_tensor("wtd", [6, 128, 12, 160], F32R, kind="ExternalInput").ap()
    wt2ad = nc.dram_tensor("wt2ad", [128, 9216], BF16D, kind="ExternalInput").ap()
    wt2bd = nc.dram_tensor("wt2bd", [32, 9216], BF16D, kind="ExternalInput").ap()
    d1wd = nc.dram_tensor("d1wd", [160, 512], F32R, kind="ExternalInput").ap()
    d2wd = nc.dram_tensor("d2wd", [4, 128, 1024], F32R, kind="ExternalInput").ap()
    d3wd = nc.dram_tensor("d3wd", [8, 128, 784], F32R, kind="ExternalInput").ap()
    d1bd = nc.dram_tensor("d1bd", [512], F32, kind="ExternalInput").ap()
    d2bd = nc.dram_tensor("d2bd", [1024], F32, kind="ExternalInput").ap()
    d3bd = nc.dram_tensor("d3bd", [784], F32, kind="ExternalInput").ap()

    normd = nc.dram_tensor("normd", [B_LOC, 10], F32, kind="ExternalOutput").ap()
    ypredd = nc.dram_tensor("ypredd", [B_LOC, 1], I32, kind="ExternalOutput").ap()
    decd = nc.dram_tensor("decd", [B_LOC, 784], F32, kind="ExternalOutput").ap()

    dbg = {}
    if debug:
        for name, shape in [
            ("dbg_x1", [2, 128, 16, 400]),
            ("dbg_x2", [2, 128, 16, 36]),
            ("dbg_pc", [2, 128, 16, 36]),
            ("dbg_s1", [B_LOC, 160]),
            ("dbg_v1", [B_LOC, 160]),
            ("dbg_uv1", [160, 1152]),
            ("dbg_A", [160, 9216]),
            ("dbg_pcT", [16, 9216]),
            ("dbg_R", [160, 9216]),
            ("dbg_s2", [B_LOC, 160]),
            ("dbg_v2", [B_LOC, 160]),
            ("dbg_h0", [B_LOC, 160]),
        ]:
            dbg[name] = nc.dram_tensor(name, shape, F32, kind="ExternalOutput").ap()

    # ---------------- constants (embedded in NEFF) ----------------
    ident_np = np.eye(128, dtype=np.float32)
    identd = nc.inline_tensor(ident_np, "identc").ap()
    s_np = np.zeros((128, 16), np.float32)
    for c in range(128):
        s_np[c, c // 8] = 1.0
    sd = nc.inline_tensor(s_np, "sconst").ap()
    iota_np = np.arange(10, dtype=np.float32).reshape(1, 10)
    iotad = nc.inline_tensor(iota_np, "iotac").ap()
    # extended identities: row (j,b) -> col (j*16+b)
    ie1_np = np.zeros((128, 160), np.float32)
    ie1_np[:, 0:128] = np.eye(128)
    ie2_np = np.zeros((32, 160), np.float32)
    ie2_np[:, 128:160] = np.eye(32)
    ie1d = nc.inline_tensor(ie1_np.astype(BF16), "ie1c").ap()
    ie2d = nc.inline_tensor(ie2_np.astype(BF16), "ie2c").ap()

    with tile.TileContext(nc) as tc:
        with tc.tile_pool(name="persist", bufs=1) as pp:
            # ---- persistent tiles / consts ----
            ident = pp.tile([128, 128], F32, tag="ident")
            nc.sync.dma_start(out=ident, in_=identd)
            identr = pp.tile([128, 128], F32R, tag="identr")
            nc.vector.tensor_copy(identr, ident)
            identb = pp.tile([128, 128], BF16D, tag="identb")
            nc.vector.tensor_copy(identb, ident)
            smat = pp.tile([128, 16], F32, tag="smat")
            nc.sync.dma_start(out=smat, in_=sd)
            smat_r = pp.tile([128, 16], F32R, tag="smat_r")
            nc.vector.tensor_copy(smat_r, smat)
            iota = pp.tile([B_LOC, 10], F32, tag="iota")
            nc.sync.dma_start(out=iota, in_=_ap(iotad, 0, [[0, B_LOC], [1, 10]]))
            ie1 = pp.tile([128, 160], BF16D, tag="ie1")
            nc.sync.dma_start(out=ie1, in_=ie1d)
            ie2 = pp.tile([32, 160], BF16D, tag="ie2")
            nc.sync.dma_start(out=ie2, in_=ie2d)
            epsb = pp.tile([128, 1], F32, tag="epsb")
            nc.vector.memset(epsb, EPS)

            w1 = pp.tile([81, 256], F32R, tag="w1")
            nc.sync.dma_start(out=w1, in_=w1d)
            b1 = pp.tile([128, 2], F32, tag="b1")
            nc.sync.dma_start(out=b1, in_=b1d)
            b2 = pp.tile([128, 2], F32, tag="b2")
            nc.sync.dma_start(out=b2, in_=b2d)

            # decoder weights: load once, early (overlaps conv compute)
            d1w = pp.tile([128, 512], F32R, tag="d1w_hi")
            d1wb = pp.tile([32, 512], F32R, tag="d1w_lo")
            nc.sync.dma_start(out=d1w, in_=d1wd[0:128])
            nc.sync.dma_start(out=d1wb, in_=d1wd[128:160])
            d2w = pp.tile([128, 4, 1024], F32R, tag="d2w")
            for c in range(4):
                nc.sync.dma_start(out=d2w[:, c, :], in_=d2wd[c])
            d3w = pp.tile([128, 8, 784], F32R, tag="d3w")
            for c in range(8):
                nc.sync.dma_start(out=d3w[:, c, :], in_=d3wd[c])
            b1r = pp.tile([B_LOC, 512], F32, tag="b1r")
            nc.sync.dma_start(out=b1r, in_=_ap(d1bd, 0, [[0, B_LOC], [1, 512]]))
            b2r = pp.tile([B_LOC, 1024], F32, tag="b2r")
            nc.sync.dma_start(out=b2r, in_=_ap(d2bd, 0, [[0, B_LOC], [1, 1024]]))
            b3r = pp.tile([B_LOC, 784], F32, tag="b3r")
            nc.sync.dma_start(out=b3r, in_=_ap(d3bd, 0, [[0, B_LOC], [1, 784]]))

            # conv2 output (post-relu), layout [cout_low, (b, hw36)] x couth
            x2 = [pp.tile([128, B_LOC, 36], F32, tag=f"x2_{h}", name=f"x2_{h}") for h in range(2)]
            pc2 = [pp.tile([128, B_LOC, 36], F32R, tag=f"pc2_{h}", name=f"pc2_{h}") for h in range(2)]

            s1 = pp.tile([B_LOC, 160], F32, tag="s1")
            v1 = pp.tile([B_LOC, 160], F32, tag="v1")
            s2 = pp.tile([B_LOC, 160], F32, tag="s2")
            v2 = pp.tile([B_LOC, 160], F32, tag="v2")
            h0 = pp.tile([B_LOC, 160], F32, tag="h0")
            norm = pp.tile([B_LOC, 10], F32, tag="norm")

            # ---------------- Phase 1: conv1 ----------------
            # wt superchunks for s1 prefetch during conv (pool open early)
            wts1_cm = tc.tile_pool(name="wts1", bufs=6)
            wts1 = wts1_cm.__enter__()
            wtcs = []
            for sc in range(6):
                wtc = wts1.tile([128, 12, 160], F32R, tag="wtc", name=f"wtc{sc}", bufs=6)
                nc.sync.dma_start(out=wtc, in_=wtd[sc])
                wtcs.append(wtc)
            with (
                tc.tile_pool(name="c1work", bufs=3) as c1w,
                tc.tile_pool(name="c1ps", bufs=4, space="PSUM") as c1p,
                tc.tile_pool(name="x1pool", bufs=1) as x1pool,
            ):
                x1 = [
                    x1pool.tile([128, B_LOC, 400], F32R, tag=f"x1_{h}", name=f"x1_{h}")
                    for h in range(2)
                ]
                for b in range(B_LOC):
                    patch = c1w.tile([81, 400], F32R, tag="patch")
                    nc.sync.dma_start(out=patch, in_=patd[b])
                    for h in range(2):
                        ps = c1p.tile([128, 400], F32, tag="c1ps")
                        nc.tensor.matmul(
                            ps,
                            w1[:, h * 128 : (h + 1) * 128],
                            patch,
                            start=True,
                            stop=True,
                        )
                        nc.scalar.activation(
                            out=x1[h][:, b, :],
                            in_=ps,
                            func=AF.Relu,
                            bias=b1[:, h : h + 1],
                            scale=1.0,
                        )
                if debug:
                    for h in range(2):
                        nc.gpsimd.dma_start(out=dbg["dbg_x1"][h], in_=x1[h])

                # ---------------- Phase 2: conv2 ----------------
                with (
                    tc.tile_pool(name="w2stream", bufs=3) as w2s,
                    tc.tile_pool(name="qps", bufs=1, space="PSUM") as qps,
                ):
                    q = [
                        [qps.tile([128, 288], F32, tag=f"q_{h}_{g}", name=f"q_{h}_{g}") for g in range(2)]
                        for h in range(2)
                    ]
                    for tg in range(27):
                        w2t = w2s.tile([128, 3, 2, 2, 128], F32R, tag="w2t")
                        nc.sync.dma_start(out=w2t, in_=w2d[tg])
                        for t3 in range(3):
                            tap = tg * 3 + t3
                            kh, kw = tap // 9, tap % 9
                            for cinh in range(2):
                                base = x1[cinh]
                                for couth in range(2):
                                    lhsT = w2t[:, t3, cinh, couth, :]
                                    for g in range(2):
                                        off = (g * 8) * 400 + kh * 20 + kw
                                        rhs = _ap(
                                            base,
                                            off,
                                            [base.ap[0], [400, 8], [40, 6], [2, 6]],
                                        )
                                        nc.tensor.matmul(
                                            q[couth][g],
                                            lhsT,
                                            rhs,
                                            start=(tap == 0 and cinh == 0),
                                            stop=(tap == 80 and cinh == 1),
                                        )
                    for couth in range(2):
                        for g in range(2):
                            nc.scalar.activation(
                                out=x2[couth][:, g * 8 : (g + 1) * 8, :],
                                in_=q[couth][g].rearrange("p (b f) -> p b f", f=36),
                                func=AF.Relu,
                                bias=b2[:, couth : couth + 1],
                                scale=1.0,
                            )
            if debug:
                for h in range(2):
                    nc.sync.dma_start(out=dbg["dbg_x2"][h], in_=x2[h])

            # ---------------- Phase 3: squash -> pc2 ----------------
            with (
                tc.tile_pool(name="sqw", bufs=1) as sqw,
                tc.tile_pool(name="sqps", bufs=4, space="PSUM") as sqps,
            ):
                sq = [sqw.tile([128, B_LOC * 36], F32R, tag=f"sq_{h}", name=f"sq_{h}") for h in range(2)]
                for h in range(2):
                    flat = x2[h].rearrange("p b f -> p (b f)")
                    nc.vector.tensor_mul(sq[h], flat, flat)
                    ssq = sqw.tile([B_LOC, 576], F32, tag="ssq", name="ssq", bufs=2)
                    for g in range(2):
                        ps = sqps.tile([16, 288], F32, tag="sqps", name="sqps")
                        nc.tensor.matmul(
                            ps,
                            smat_r,
                            sq[h][:, g * 288 : (g + 1) * 288],
                            start=True,
                            stop=True,
                        )
                        nc.scalar.copy(
                            out=ssq[:, g * 288 : (g + 1) * 288],
                            in_=ps,
                        )
                    # f = (ssq/(1+ssq)) / sqrt(ssq+eps)
                    a = sqw.tile([B_LOC, 576], F32, tag="fa", name="fa", bufs=2)
                    r = sqw.tile([B_LOC, 576], F32, tag="fr", name="fr", bufs=2)
                    fT = sqw.tile([B_LOC, 576], F32, tag="fT", name="fT", bufs=2)
                    nc.vector.tensor_scalar_add(a, ssq, 1.0)
                    nc.vector.reciprocal(r, a)
                    nc.vector.tensor_mul(r, ssq, r)
                    nc.scalar.activation(
                        out=a, in_=ssq, func=AF.Sqrt, bias=epsb[:B_LOC, :], scale=1.0
                    )
                    nc.vector.reciprocal(a, a)
                    nc.vector.tensor_mul(fT, r, a)
                    fe = sqw.tile([128, 576], F32, tag="fexp", name="fexp", bufs=2)
                    nc.sync.dma_start(
                        out=fe, in_=_ap(fT, 0, [fT.ap[0], [0, 8], [1, 576]])
                    )
                    nc.vector.tensor_mul(
                        pc2[h].rearrange("p b f -> p (b f)"),
                        x2[h].rearrange("p b f -> p (b f)"),
                        fe,
                    )
            if debug:
                for h in range(2):
                    nc.gpsimd.dma_start(out=dbg["dbg_pc"][h], in_=pc2[h])

            def pc_chunk(n):
                """lhsT [(ik) 128, b] for flat (i,k)-chunk n (=hw*2+chalf)."""
                hw, chalf = n // 2, n % 2
                base = pc2[chalf]
                return _ap(base, hw, [base.ap[0], [36, B_LOC]])

            # ---------------- Phase 4: s1 = 0.1 * sum_i u_hat ----------------
            with tc.tile_pool(name="s1ps", bufs=1, space="PSUM") as s1psp:
                s1ps = s1psp.tile([B_LOC, 160], F32, tag="s1ps")
                for sc in range(6):
                    for m in range(12):
                        n = sc * 12 + m
                        nc.tensor.matmul(
                            s1ps,
                            pc_chunk(n),
                            wtcs[sc][:, m, :],
                            start=(n == 0),
                            stop=(n == 71),
                        )
                nc.scalar.mul(out=s1, in_=s1ps, mul=0.1)
            wts1_cm.__exit__(None, None, None)
            if debug:
                nc.sync.dma_start(out=dbg["dbg_s1"], in_=s1)

            def squash16(out_t, in_t, tmp_pool, nj):
                ssq = tmp_pool.tile([B_LOC, nj], F32, tag="sq_ssq")
                prod = tmp_pool.tile([B_LOC, nj * 16], F32, tag="sq_prod")
                nc.vector.tensor_mul(prod, in_t, in_t)
                nc.vector.reduce_sum(
                    ssq, prod.rearrange("p (j d) -> p j d", d=16), axis=AX.X
                )
                aa = tmp_pool.tile([B_LOC, nj], F32, tag="sq_a")
                rr = tmp_pool.tile([B_LOC, nj], F32, tag="sq_r")
                ff = tmp_pool.tile([B_LOC, nj], F32, tag="sq_f")
                nc.vector.tensor_scalar_add(aa, ssq, 1.0)
                nc.vector.reciprocal(rr, aa)
                nc.vector.tensor_mul(rr, ssq, rr)
                nc.scalar.activation(
                    out=aa, in_=ssq, func=AF.Sqrt, bias=epsb[:B_LOC, :], scale=1.0
                )
                nc.vector.reciprocal(aa, aa)
                nc.vector.tensor_mul(ff, rr, aa)
                fb = _ap(ff, 0, [ff.ap[0], [1, nj], [0, 16]])
                nc.vector.tensor_tensor(
                    out=out_t.rearrange("p (j d) -> p j d", d=16),
                    in0=in_t.rearrange("p (j d) -> p j d", d=16),
                    in1=fb,
                    op=ALU.mult,
                )

            with tc.tile_pool(name="rwork", bufs=1) as rw:
                squash16(v1, s1, rw, 10)
                if debug:
                    nc.sync.dma_start(out=dbg["dbg_v1"], in_=v1)

                # ---- v1 block-diagonal [ (j,d), (j',b) ] in bf16 ----
                with tc.tile_pool(name="tps", bufs=1, space="PSUM") as tps:
                    blkA = rw.tile([128, 160], BF16D, tag="blkA")
                    blkB = rw.tile([32, 160], BF16D, tag="blkB")
                    nc.vector.memset(blkA, 0.0)
                    nc.vector.memset(blkB, 0.0)
                    t1 = tps.tile([128, 16], F32, tag="t1")
                    nc.tensor.transpose(t1, v1[:, 0:128], ident[:16, :16])
                    t2 = tps.tile([32, 16], F32, tag="t2")
                    nc.tensor.transpose(t2, v1[:, 128:160], ident[:16, :16])
                    v1T1 = rw.tile([128, 16], BF16D, tag="v1T1")
                    v1T2 = rw.tile([32, 16], BF16D, tag="v1T2")
                    nc.scalar.copy(out=v1T1, in_=t1)
                    nc.scalar.copy(out=v1T2, in_=t2)
                    for j in range(8):
                        nc.sync.dma_start(
                            out=blkA[j * 16 : (j + 1) * 16, j * 16 : (j + 1) * 16],
                            in_=v1T1[j * 16 : (j + 1) * 16, :],
                        )
                    for j in range(8, 10):
                        nc.sync.dma_start(
                            out=blkB[(j - 8) * 16 : (j - 7) * 16, j * 16 : (j + 1) * 16],
                            in_=v1T2[(j - 8) * 16 : (j - 7) * 16, :],
                        )

                    # ---- A[(j',b), (ik)] = sum_d v1 * W  (bf16, full rate) ----
                    A1 = rw.tile([128, 9216], BF16D, tag="A1")
                    A2 = rw.tile([32, 9216], BF16D, tag="A2")
                    with (
                        tc.tile_pool(name="aps", bufs=2, space="PSUM") as aps,
                        tc.tile_pool(name="wt2s", bufs=1) as wt2s,
                    ):
                        for half in range(2):
                            hsl = slice(half * 4608, (half + 1) * 4608)
                            wa = wt2s.tile([128, 4608], BF16D, tag="wa", name="wa")
                            nc.sync.dma_start(out=wa, in_=wt2ad[:, hsl])
                            wb = wt2s.tile([32, 4608], BF16D, tag="wb", name="wb")
                            nc.sync.dma_start(out=wb, in_=wt2bd[:, hsl])
                            for cn in range(9):
                                gsl = slice(
                                    half * 4608 + cn * 512, half * 4608 + (cn + 1) * 512
                                )
                                lsl = slice(cn * 512, (cn + 1) * 512)
                                for mi, (mdst, msl) in enumerate(
                                    [(A1, slice(0, 128)), (A2, slice(128, 160))]
                                ):
                                    ps = aps.tile(
                                        [128 if mi == 0 else 32, 512],
                                        F32,
                                        tag=f"aps{mi}",
                                        name=f"aps{mi}",
                                    )
                                    nc.tensor.matmul(
                                        ps,
                                        blkA[:, msl],
                                        wa[:, lsl],
                                        start=True,
                                        stop=False,
                                    )
                                    nc.tensor.matmul(
                                        ps,
                                        blkB[:, msl],
                                        wb[:, lsl],
                                        start=False,
                                        stop=True,
                                    )
                                    nc.scalar.copy(out=mdst[:, gsl], in_=ps)



                # ---- pcT [b, (ik)] f32r via pipelined PE transposes ----
                pcT = rw.tile([B_LOC, 9216], BF16D, tag="pcT")
                with tc.tile_pool(name="tpp", bufs=6, space="PSUM") as tpp:
                    for hw in range(36):
                        for h in range(2):
                            base = pc2[h]
                            sl = _ap(base, hw, [base.ap[0], [36, B_LOC]])
                            ps = tpp.tile([16, 128], F32R, tag="tp36", name="tp36")
                            nc.tensor.transpose(ps, sl, identr)
                            nc.scalar.copy(
                                out=pcT[
                                    :, hw * 256 + h * 128 : hw * 256 + (h + 1) * 128
                                ],
                                in_=ps,
                            )
                # ---- pc_rep [(j,b), (ik)] bf16 ----
                R1 = rw.tile([128, 9216], BF16D, tag="R1")
                R2 = rw.tile([32, 9216], BF16D, tag="R2")
                for j in range(8):
                    nc.sync.dma_start(out=R1[j * 16 : (j + 1) * 16, :], in_=pcT)
                for j in range(2):
                    nc.sync.dma_start(out=R2[j * 16 : (j + 1) * 16, :], in_=pcT)

                if debug:
                    nc.gpsimd.dma_start(out=dbg["dbg_A"][0:128], in_=A1)
                    nc.gpsimd.dma_start(out=dbg["dbg_A"][128:160], in_=A2)
                    nc.gpsimd.dma_start(out=dbg["dbg_pcT"], in_=pcT)
                    nc.gpsimd.dma_start(out=dbg["dbg_R"][0:128], in_=R1)
                    nc.gpsimd.dma_start(out=dbg["dbg_R"][128:160], in_=R2)
                # ---- uv1[(j,b), i] = sum_k A * pc ----
                uv1a = rw.tile([128, 1152], F32, tag="uv1a")
                uv1b = rw.tile([32, 1152], F32, tag="uv1b")
                nc.vector.tensor_mul(A1, A1, R1)
                nc.vector.reduce_sum(
                    uv1a, A1.rearrange("p (i k) -> p i k", k=8), axis=AX.X
                )
                nc.vector.tensor_mul(A2, A2, R2)
                nc.vector.reduce_sum(
                    uv1b, A2.rearrange("p (i k) -> p i k", k=8), axis=AX.X
                )
                if debug:
                    nc.sync.dma_start(out=dbg["dbg_uv1"][0:128], in_=uv1a)
                    nc.sync.dma_start(out=dbg["dbg_uv1"][128:160], in_=uv1b)

                # ---- softmax numerator: exp (bf16; Z folded in during s2) ----
                e1 = rw.tile([128, 1152], BF16D, tag="e1")
                e2 = rw.tile([32, 1152], BF16D, tag="e2")
                nc.scalar.activation(out=e1, in_=uv1a, func=AF.Exp)
                nc.scalar.activation(out=e2, in_=uv1b, func=AF.Exp)
                # replicate exp x8 along free (i -> (i, rep)); reuse A slots
                e1rep = rw.tile([128, 9216], BF16D, tag="A1", name="e1rep")
                e2rep = rw.tile([32, 9216], BF16D, tag="A2", name="e2rep")
                nc.vector.tensor_copy(
                    e1rep.rearrange("p (i r) -> p i r", r=8),
                    _ap(e1, 0, [e1.ap[0], [1, 1152], [0, 8]]),
                )
                nc.vector.tensor_copy(
                    e2rep.rearrange("p (i r) -> p i r", r=8),
                    _ap(e2, 0, [e2.ap[0], [1, 1152], [0, 8]]),
                )

                # ---- s2: accumulate over 72 (ik)-chunks ----
                # per chunk: transpose+replicate exp via permuted-identity
                # matmuls -> psum [ (i,rep)=128, (b,j)=160 ]; Z = reduce over
                # j; g = rep * (1/Z) * pc; two s2 matmuls accumulate.
                with (
                    tc.tile_pool(name="s2w", bufs=4) as s2w,
                    tc.tile_pool(name="wts2", bufs=2) as wts2,
                    tc.tile_pool(name="reps", bufs=4, space="PSUM") as repsp,
                    tc.tile_pool(name="s2ps", bufs=1, space="PSUM") as s2psp,
                ):
                    psA = s2psp.tile([128, 160], F32, tag="psA")
                    psB = s2psp.tile([32, 160], F32, tag="psB")
                    for sc in range(6):
                        wtc2 = wts2.tile([128, 12, 160], F32R, tag="wtc2", name="wtc2")
                        nc.sync.dma_start(out=wtc2, in_=wtd[sc])
                        for m in range(12):
                            n = sc * 12 + m
                            rep = repsp.tile([128, 160], F32, tag="rep", name="rep")
                            nc.tensor.matmul(
                                rep,
                                e1rep[:, 128 * n : 128 * (n + 1)],
                                ie1,
                                start=True,
                                stop=False,
                            )
                            nc.tensor.matmul(
                                rep,
                                e2rep[:, 128 * n : 128 * (n + 1)],
                                ie2,
                                start=False,
                                stop=True,
                            )
                            zc = s2w.tile([128, 16], F32, tag="zc", name="zc")
                            # rep free layout (j,b): strided view -> (b, j)
                            nc.vector.reduce_sum(
                                zc,
                                _ap(rep, 0, [rep.ap[0], [1, 16], [16, 10]]),
                                axis=AX.X,
                            )
                            zr = s2w.tile([128, 16], F32, tag="zr", name="zr")
                            nc.vector.reciprocal(zr, zc)
                            g = s2w.tile([128, 160], F32R, tag="g", name="g")
                            nc.vector.tensor_tensor(
                                out=g.rearrange("p (j b) -> p j b", j=10),
                                in0=rep.rearrange("p (j b) -> p j b", j=10),
                                in1=_ap(zr, 0, [zr.ap[0], [0, 10], [1, 16]]),
                                op=ALU.mult,
                            )
                            pcb = pc_chunk(n)
                            nc.vector.tensor_tensor(
                                out=g.rearrange("p (j b) -> p j b", j=10),
                                in0=g.rearrange("p (j b) -> p j b", j=10),
                                in1=_ap(pcb, 0, [pcb.ap[0], [0, 10], [36, B_LOC]]),
                                op=ALU.mult,
                            )
                            nc.tensor.matmul(
                                psA,
                                g[:, 0:128],
                                wtc2[:, m, :],
                                start=(n == 0),
                                stop=(n == 71),
                            )
                            nc.tensor.matmul(
                                psB,
                                g[:, 128:160],
                                wtc2[:, m, :],
                                start=(n == 0),
                                stop=(n == 71),
                            )
                    # diagonal extraction: psum row m=(j*16+b) -> s2[b, (j,:)]
                    sA = s2w.tile([128, 160], F32, tag="sA")
                    sB = s2w.tile([32, 160], F32, tag="sB")
                    nc.scalar.copy(out=sA, in_=psA)
                    nc.scalar.copy(out=sB, in_=psB)
                    for j in range(8):
                        nc.sync.dma_start(
                            out=s2[:, j * 16 : (j + 1) * 16],
                            in_=sA[j * 16 : (j + 1) * 16, j * 16 : (j + 1) * 16],
                        )
                    for j in range(8, 10):
                        nc.sync.dma_start(
                            out=s2[:, j * 16 : (j + 1) * 16],
                            in_=sB[(j - 8) * 16 : (j - 7) * 16, j * 16 : (j + 1) * 16],
                        )
                if debug:
                    nc.sync.dma_start(out=dbg["dbg_s2"], in_=s2)

                # ---- v2, norm, y_pred, mask ----
                ssq2 = rw.tile([B_LOC, 10], F32, tag="ssq2")
                v2sq = rw.tile([B_LOC, 160], F32, tag="v2sq")
                squash16(v2, s2, rw, 10)
                nc.vector.tensor_mul(v2sq, v2, v2)
                nc.vector.reduce_sum(
                    ssq2, v2sq.rearrange("p (j d) -> p j d", d=16), axis=AX.X
                )
                nc.scalar.activation(
                    out=norm, in_=ssq2, func=AF.Sqrt, bias=epsb[:B_LOC, :], scale=1.0
                )
                nc.sync.dma_start(out=normd, in_=norm)
                if debug:
                    nc.sync.dma_start(out=dbg["dbg_v2"], in_=v2)

                vmax = rw.tile([B_LOC, 8], F32, tag="vmax")
                vidx = rw.tile([B_LOC, 8], U32, tag="vidx")
                nc.vector.max_with_indices(vmax, vidx, norm)
                ypi = rw.tile([B_LOC, 1], I32, tag="ypi")
                nc.vector.tensor_copy(ypi, vidx[:, 0:1])
                nc.sync.dma_start(out=ypredd, in_=ypi)

                tgtf = rw.tile([B_LOC, 1], F32, tag="tgtf")
                tgti = rw.tile([B_LOC, 1], I32, tag="tgti")
                nc.sync.dma_start(out=tgti, in_=tgt)
                nc.vector.tensor_copy(tgtf, tgti)
                mask = rw.tile([B_LOC, 10], F32, tag="mask")
                nc.vector.tensor_scalar(
                    out=mask, in0=iota, scalar1=tgtf, scalar2=None, op0=ALU.is_equal
                )
                nc.vector.tensor_tensor(
                    out=h0.rearrange("p (j d) -> p j d", d=16),
                    in0=v2.rearrange("p (j d) -> p j d", d=16),
                    in1=_ap(mask, 0, [mask.ap[0], [1, 10], [0, 16]]),
                    op=ALU.mult,
                )
                if debug:
                    nc.sync.dma_start(out=dbg["dbg_h0"], in_=h0)

            # ---------------- Phase 5: decoder ----------------
            with (
                tc.tile_pool(name="dps", bufs=2, space="PSUM") as dps,
                tc.tile_pool(name="dwork", bufs=1) as dwk,
            ):
                h0T1 = dwk.tile([128, 16], F32R, tag="h0T1")
                h0T2 = dwk.tile([32, 16], F32R, tag="h0T2")
                ps = dps.tile([128, 16], F32, tag="dtp")
                nc.tensor.transpose(ps, h0[:, 0:128], ident[:16, :16])
                nc.scalar.copy(out=h0T1, in_=ps)
                ps = dps.tile([32, 16], F32, tag="dtp", name="dtp2")
                nc.tensor.transpose(ps, h0[:, 128:160], ident[:16, :16])
                nc.scalar.copy(out=h0T2, in_=ps)

                h1 = dwk.tile([B_LOC, 512], F32, tag="h1")
                hp = dps.tile([B_LOC, 512], F32, tag="mmp")
                nc.tensor.matmul(hp, h0T1, d1w, start=True, stop=False)
                nc.tensor.matmul(hp, h0T2, d1wb, start=False, stop=True)
                nc.vector.tensor_add(h1, hp, b1r)
                nc.scalar.activation(out=h1, in_=h1, func=AF.Relu)

                h1T = dwk.tile([128, 4, 16], F32R, tag="h1T")
                for c in range(4):
                    ps = dps.tile([128, 16], F32, tag="dtp", name="dtp3")
                    nc.tensor.transpose(
                        ps, h1[:, c * 128 : (c + 1) * 128], ident[:16, :16]
                    )
                    nc.scalar.copy(out=h1T[:, c, :], in_=ps)
                h2 = dwk.tile([B_LOC, 1024], F32, tag="h2")
                for nh in range(2):
                    hp2 = dps.tile([B_LOC, 512], F32, tag="mmp", name="hp2")
                    for c in range(4):
                        nc.tensor.matmul(
                            hp2,
                            h1T[:, c, :],
                            d2w[:, c, nh * 512 : (nh + 1) * 512],
                            start=(c == 0),
                            stop=(c == 3),
                        )
                    sl = slice(nh * 512, (nh + 1) * 512)
                    nc.vector.tensor_add(h2[:, sl], hp2, b2r[:, sl])
                    nc.scalar.activation(out=h2[:, sl], in_=h2[:, sl], func=AF.Relu)

                h2T = dwk.tile([128, 8, 16], F32R, tag="h2T")
                for c in range(8):
                    ps = dps.tile([128, 16], F32, tag="dtp", name="dtp4")
                    nc.tensor.transpose(
                        ps, h2[:, c * 128 : (c + 1) * 128], ident[:16, :16]
                    )
                    nc.scalar.copy(out=h2T[:, c, :], in_=ps)
                dec = dwk.tile([B_LOC, 784], F32, tag="dec")
                for nh, (n0, n1) in enumerate([(0, 512), (512, 784)]):
                    hp3 = dps.tile([B_LOC, 512], F32, tag="mmp", name="hp3")[
                        :, : n1 - n0
                    ]
                    for c in range(8):
                        nc.tensor.matmul(
                            hp3,
                            h2T[:, c, :],
                            d3w[:, c, n0:n1],
                            start=(c == 0),
                            stop=(c == 7),
                        )
                    nc.vector.tensor_add(dec[:, n0:n1], hp3, b3r[:, n0:n1])
                nc.scalar.activation(out=dec, in_=dec, func=AF.Sigmoid)
                nc.sync.dma_start(out=decd, in_=dec)

    nc.compile()
    return nc


# ---------------------------------------------------------------------------
# Host side
# ---------------------------------------------------------------------------


def prep_inputs(image, target, conv1_w, conv1_b, conv2_w, conv2_b, W,
                d1_w, d1_b, d2_w, d2_b, do_w, do_b):
    """Full inputs -> list of 8 per-core input maps."""
    image = np.asarray(image, np.float32)
    target = np.asarray(target)
    B = image.shape[0]
    per = B // N_CORES

    # host im2col for conv1: [B, 81, 400]
    img = image[:, :, :, 0]
    sw = np.lib.stride_tricks.sliding_window_view(img, (9, 9), axis=(1, 2))
    # sw: [B, 20, 20, 9, 9] -> [B, (kh kw), (oh ow)]
    pat = np.ascontiguousarray(sw.transpose(0, 3, 4, 1, 2)).reshape(B, 81, 400)

    w1 = np.asarray(conv1_w, np.float32).reshape(81, 256)
    b1 = np.zeros((128, 2), np.float32)
    b1[:, 0] = np.asarray(conv1_b, np.float32)[0:128]
    b1[:, 1] = np.asarray(conv1_b, np.float32)[128:256]
    w2 = (
        np.asarray(conv2_w, np.float32)
        .reshape(81, 2, 128, 2, 128)
        .transpose(0, 2, 1, 3, 4)  # tap, cin_low, cinh, couth, cout_low
        .reshape(27, 3, 128, 2, 2, 128)
        .transpose(0, 2, 1, 3, 4, 5)  # tg, cin_low, tap3, cinh, couth, cout
        .copy()
    )
    b2 = np.zeros((128, 2), np.float32)
    b2[:, 0] = np.asarray(conv2_b, np.float32)[0:128]
    b2[:, 1] = np.asarray(conv2_b, np.float32)[128:256]

    W0 = np.asarray(W, np.float32)[0]  # [1152, 10, 16, 8]
    wt = (
        W0.transpose(0, 3, 1, 2)  # i, k, j, d
        .reshape(6, 12, 128, 160)
        .transpose(0, 2, 1, 3)  # sc, ik_low(128), m, jd
        .copy()
    )
    wt2 = W0.transpose(1, 2, 0, 3).reshape(160, 9216)  # (j,d), (i,k)
    wt2a = wt2[0:128].astype(BF16)
    wt2b = wt2[128:160].astype(BF16)

    d1w = np.asarray(d1_w, np.float32)
    d2w = np.asarray(d2_w, np.float32).reshape(4, 128, 1024)
    d3w = np.asarray(do_w, np.float32).reshape(8, 128, 784)

    shared = dict(
        w1d=w1, b1d=b1, w2d=w2, b2d=b2, wtd=wt, wt2ad=wt2a, wt2bd=wt2b,
        d1wd=d1w, d2wd=d2w, d3wd=d3w,
        d1bd=np.asarray(d1_b, np.float32),
        d2bd=np.asarray(d2_b, np.float32),
        d3bd=np.asarray(do_b, np.float32),
    )
    maps = []
    for c in range(N_CORES):
        m = dict(shared)
        m["patd"] = pat[c * per : (c + 1) * per]
        m["tgt"] = target[c * per : (c + 1) * per].astype(np.int32).reshape(per, 1)
        maps.append(m)
    return maps


_NC_CACHE = {}


def _get_nc(debug=False):
    key = bool(debug)
    if key not in _NC_CACHE:
        _NC_CACHE[key] = build_nc(debug=key)
    return _NC_CACHE[key]


def kernel(**inputs):
    from concourse import bass_utils

    target = np.asarray(inputs["target"])
    maps = prep_inputs(**inputs)
    nc = _get_nc(debug=False)
    res = bass_utils.run_bass_kernel_spmd(nc, maps, core_ids=list(range(N_CORES)))
    outs = res.results
    B = N_CORES * B_LOC
    norm = np.concatenate([o["normd"] for o in outs], 0).reshape(B, 1, 10, 1, 1)
    ypred = np.concatenate([o["ypredd"] for o in outs], 0).reshape(B, 1)
    dec = np.concatenate([o["decd"] for o in outs], 0).reshape(B, 1, 784)
    if target.dtype == np.int64:
        ypred = ypred.astype(np.int64)
    else:
        ypred = ypred.astype(np.int32)
    return norm, ypred, dec


# revision 31
# speedup vs baseline: 1.8566x; 1.0023x over previous
"""CapsuleNetwork Trainium2 Bass kernel.

8-core data-parallel: batch 128 -> 16 per core, all weights replicated.
Per-core pipeline (all on device):
  conv1 (host-im2col patches x W1 matmul) -> conv2 (tap-accumulation
  matmul, fp32r full rate) -> primary-caps squash (partition-group
  sum-of-squares via ones-blockdiag matmul) -> dynamic routing
  (2 rounds; s1 via K=9216 matmul, A-tensor via block-diag-v1 matmul,
  softmax transpose+replicate via permuted-identity matmuls) ->
  norm/argmax/mask -> 3-layer decoder.
"""

import sys

import numpy as np

sys.path.insert(0, "/opt/trn_rl_repo")

import concourse.bass as bass
import concourse.bacc as bacc
import concourse.tile as tile
from concourse import mybir

import ml_dtypes

BF16 = ml_dtypes.bfloat16

EPS = 1e-7
B_LOC = 16  # images per core
N_CORES = 8
F32 = mybir.dt.float32
F32R = mybir.dt.float32r
BF16D = mybir.dt.bfloat16
I32 = mybir.dt.int32
U32 = mybir.dt.uint32
AF = mybir.ActivationFunctionType
ALU = mybir.AluOpType
AX = mybir.AxisListType


def _ap(base, offset_elems, dims):
    """Raw AP at base.offset + offset_elems with explicit [step,count] dims."""
    return bass.AP(tensor=base.tensor, offset=base.offset + offset_elems, ap=dims)


def build_nc(debug=False):
    nc = bacc.Bacc("TRN2", target_bir_lowering=False, debug=False)

    # ---------------- DRAM I/O ----------------
    patd = nc.dram_tensor("patd", [B_LOC, 81, 400], F32R, kind="ExternalInput").ap()
    tgt = nc.dram_tensor("tgt", [B_LOC, 1], I32, kind="ExternalInput").ap()
    w1d = nc.dram_tensor("w1d", [81, 256], F32R, kind="ExternalInput").ap()
    b1d = nc.dram_tensor("b1d", [128, 2], F32, kind="ExternalInput").ap()
    # (tapgroup 27, cin_low 128, tap3, cinh, couth, cout_low)
    w2d = nc.dram_tensor(
        "w2d", [27, 128, 3, 2, 2, 128], F32R, kind="ExternalInput"
    ).ap()
    b2d = nc.dram_tensor("b2d", [128, 2], F32, kind="ExternalInput").ap()
    # (super-chunk 6, cin_low 128, m 12, jd 160)
    wtd = nc.dram_tensor("wtd", [6, 128, 12, 160], F32R, kind="ExternalInput").ap()
    wt2ad = nc.dram_tensor("wt2ad", [128, 9216], BF16D, kind="ExternalInput").ap()
    wt2bd = nc.dram_tensor("wt2bd", [32, 9216], BF16D, kind="ExternalInput").ap()
    d1wd = nc.dram_tensor("d1wd", [160, 512], F32R, kind="ExternalInput").ap()
    d2wd = nc.dram_tensor("d2wd", [4, 128, 1024], F32R, kind="ExternalInput").ap()
    d3wd = nc.dram_tensor("d3wd", [8, 128, 784], F32R, kind="ExternalInput").ap()
    d1bd = nc.dram_tensor("d1bd", [512], F32, kind="ExternalInput").ap()
    d2bd = nc.dram_tensor("d2bd", [1024], F32, kind="ExternalInput").ap()
    d3bd = nc.dram_tensor("d3bd", [784], F32, kind="ExternalInput").ap()

    normd = nc.dram_tensor("normd", [B_LOC, 10], F32, kind="ExternalOutput").ap()
    ypredd = nc.dram_tensor("ypredd", [B_LOC, 1], I32, kind="ExternalOutput").ap()
    decd = nc.dram_tensor("decd", [B_LOC, 784], F32, kind="ExternalOutput").ap()

    dbg = {}
    if debug:
        for name, shape in [
            ("dbg_x1", [2, 128, 16, 400]),
            ("dbg_x2", [2, 128, 16, 36]),
            ("dbg_pc", [2, 128, 16, 36]),
            ("dbg_s1", [B_LOC, 160]),
            ("dbg_v1", [B_LOC, 160]),
            ("dbg_uv1", [160, 1152]),
            ("dbg_A", [160, 9216]),
            ("dbg_pcT", [16, 9216]),
            ("dbg_R", [160, 9216]),
            ("dbg_s2", [B_LOC, 160]),
            ("dbg_v2", [B_LOC, 160]),
            ("dbg_h0", [B_LOC, 160]),
        ]:
            dbg[name] = nc.dram_tensor(name, shape, F32, kind="ExternalOutput").ap()

    # ---------------- constants (embedded in NEFF) ----------------
    ident_np = np.eye(128, dtype=np.float32)
    identd = nc.inline_tensor(ident_np, "identc").ap()
    s_np = np.zeros((128, 16), np.float32)
    for c in range(128):
        s_np[c, c // 8] = 1.0
    sd = nc.inline_tensor(s_np, "sconst").ap()
    iota_np = np.arange(10, dtype=np.float32).reshape(1, 10)
    iotad = nc.inline_tensor(iota_np, "iotac").ap()
    # extended identities: row (j,b) -> col (j*16+b)
    ie1_np = np.zeros((128, 160), np.float32)
    ie1_np[:, 0:128] = np.eye(128)
    ie2_np = np.zeros((32, 160), np.float32)
    ie2_np[:, 128:160] = np.eye(32)
    ie1d = nc.inline_tensor(ie1_np.astype(BF16), "ie1c").ap()
    ie2d = nc.inline_tensor(ie2_np.astype(BF16), "ie2c").ap()

    with tile.TileContext(nc) as tc:
        with tc.tile_pool(name="persist", bufs=1) as pp:
            # ---- persistent tiles / consts ----
            ident = pp.tile([128, 128], F32, tag="ident")
            nc.sync.dma_start(out=ident, in_=identd)
            identr = pp.tile([128, 128], F32R, tag="identr")
            nc.vector.tensor_copy(identr, ident)
            identb = pp.tile([128, 128], BF16D, tag="identb")
            nc.vector.tensor_copy(identb, ident)
            smat = pp.tile([128, 16], F32, tag="smat")
            nc.sync.dma_start(out=smat, in_=sd)
            smat_r = pp.tile([128, 16], F32R, tag="smat_r")
            nc.vector.tensor_copy(smat_r, smat)
            iota = pp.tile([B_LOC, 10], F32, tag="iota")
            nc.sync.dma_start(out=iota, in_=_ap(iotad, 0, [[0, B_LOC], [1, 10]]))
            ie1 = pp.tile([128, 160], BF16D, tag="ie1")
            nc.sync.dma_start(out=ie1, in_=ie1d)
            ie2 = pp.tile([32, 160], BF16D, tag="ie2")
            nc.sync.dma_start(out=ie2, in_=ie2d)
            epsb = pp.tile([128, 1], F32, tag="epsb")
            nc.vector.memset(epsb, EPS)

            w1 = pp.tile([81, 256], F32R, tag="w1")
            nc.sync.dma_start(out=w1, in_=w1d)
            b1 = pp.tile([128, 2], F32, tag="b1")
            nc.sync.dma_start(out=b1, in_=b1d)
            b2 = pp.tile([128, 2], F32, tag="b2")
            nc.sync.dma_start(out=b2, in_=b2d)

            # decoder weights: load once, early (overlaps conv compute)
            d1w = pp.tile([128, 512], F32R, tag="d1w_hi")
            d1wb = pp.tile([32, 512], F32R, tag="d1w_lo")
            nc.scalar.dma_start(out=d1w, in_=d1wd[0:128])
            nc.scalar.dma_start(out=d1wb, in_=d1wd[128:160])
            d2w = pp.tile([128, 4, 1024], F32R, tag="d2w")
            for c in range(4):
                nc.scalar.dma_start(out=d2w[:, c, :], in_=d2wd[c])
            d3w = pp.tile([128, 8, 784], F32R, tag="d3w")
            for c in range(8):
                nc.scalar.dma_start(out=d3w[:, c, :], in_=d3wd[c])
            b1r = pp.tile([B_LOC, 512], F32, tag="b1r")
            nc.scalar.dma_start(out=b1r, in_=_ap(d1bd, 0, [[0, B_LOC], [1, 512]]))
            b2r = pp.tile([B_LOC, 1024], F32, tag="b2r")
            nc.scalar.dma_start(out=b2r, in_=_ap(d2bd, 0, [[0, B_LOC], [1, 1024]]))
            b3r = pp.tile([B_LOC, 784], F32, tag="b3r")
            nc.scalar.dma_start(out=b3r, in_=_ap(d3bd, 0, [[0, B_LOC], [1, 784]]))

            # conv2 output (post-relu), layout [cout_low, (b, hw36)] x couth
            x2 = [pp.tile([128, B_LOC, 36], F32, tag=f"x2_{h}", name=f"x2_{h}") for h in range(2)]
            pc2 = [pp.tile([128, B_LOC, 36], F32R, tag=f"pc2_{h}", name=f"pc2_{h}") for h in range(2)]

            s1 = pp.tile([B_LOC, 160], F32, tag="s1")
            v1 = pp.tile([B_LOC, 160], F32, tag="v1")
            s2 = pp.tile([B_LOC, 160], F32, tag="s2")
            v2 = pp.tile([B_LOC, 160], F32, tag="v2")
            h0 = pp.tile([B_LOC, 160], F32, tag="h0")
            norm = pp.tile([B_LOC, 10], F32, tag="norm")

            # ---------------- Phase 1: conv1 ----------------
            # wt superchunks for s1 prefetch during conv (pool open early)
            wts1_cm = tc.tile_pool(name="wts1", bufs=6)
            wts1 = wts1_cm.__enter__()
            wtcs = []
            for sc in range(6):
                wtc = wts1.tile([128, 12, 160], F32R, tag="wtc", name=f"wtc{sc}", bufs=6)
                nc.scalar.dma_start(out=wtc, in_=wtd[sc])
                wtcs.append(wtc)
            with (
                tc.tile_pool(name="c1work", bufs=3) as c1w,
                tc.tile_pool(name="c1ps", bufs=4, space="PSUM") as c1p,
                tc.tile_pool(name="x1pool", bufs=1) as x1pool,
            ):
                x1 = [
                    x1pool.tile([128, B_LOC, 400], F32R, tag=f"x1_{h}", name=f"x1_{h}")
                    for h in range(2)
                ]
                for b in range(B_LOC):
                    patch = c1w.tile([81, 400], F32R, tag="patch")
                    nc.sync.dma_start(out=patch, in_=patd[b])
                    for h in range(2):
                        ps = c1p.tile([128, 400], F32, tag="c1ps")
                        nc.tensor.matmul(
                            ps,
                            w1[:, h * 128 : (h + 1) * 128],
                            patch,
                            start=True,
                            stop=True,
                        )
                        nc.scalar.activation(
                            out=x1[h][:, b, :],
                            in_=ps,
                            func=AF.Relu,
                            bias=b1[:, h : h + 1],
                            scale=1.0,
                        )
                if debug:
                    for h in range(2):
                        nc.gpsimd.dma_start(out=dbg["dbg_x1"][h], in_=x1[h])

                # ---------------- Phase 2: conv2 ----------------
                with (
                    tc.tile_pool(name="w2stream", bufs=3) as w2s,
                    tc.tile_pool(name="qps", bufs=1, space="PSUM") as qps,
                ):
                    q = [
                        [qps.tile([128, 288], F32, tag=f"q_{h}_{g}", name=f"q_{h}_{g}") for g in range(2)]
                        for h in range(2)
                    ]
                    for tg in range(27):
                        w2t = w2s.tile([128, 3, 2, 2, 128], F32R, tag="w2t")
                        nc.sync.dma_start(out=w2t, in_=w2d[tg])
                        for t3 in range(3):
                            tap = tg * 3 + t3
                            kh, kw = tap // 9, tap % 9
                            for cinh in range(2):
                                base = x1[cinh]
                                for couth in range(2):
                                    lhsT = w2t[:, t3, cinh, couth, :]
                                    for g in range(2):
                                        off = (g * 8) * 400 + kh * 20 + kw
                                        rhs = _ap(
                                            base,
                                            off,
                                            [base.ap[0], [400, 8], [40, 6], [2, 6]],
                                        )
                                        nc.tensor.matmul(
                                            q[couth][g],
                                            lhsT,
                                            rhs,
                                            start=(tap == 0 and cinh == 0),
                                            stop=(tap == 80 and cinh == 1),
                                        )
                    for couth in range(2):
                        for g in range(2):
                            nc.scalar.activation(
                                out=x2[couth][:, g * 8 : (g + 1) * 8, :],
                                in_=q[couth][g].rearrange("p (b f) -> p b f", f=36),
                                func=AF.Relu,
                                bias=b2[:, couth : couth + 1],
                                scale=1.0,
                            )
            if debug:
                for h in range(2):
                    nc.sync.dma_start(out=dbg["dbg_x2"][h], in_=x2[h])

            # ---------------- Phase 3: squash -> pc2 ----------------
            with (
                tc.tile_pool(name="sqw", bufs=1) as sqw,
                tc.tile_pool(name="sqps", bufs=4, space="PSUM") as sqps,
            ):
                sq = [sqw.tile([128, B_LOC * 36], F32R, tag=f"sq_{h}", name=f"sq_{h}") for h in range(2)]
                for h in range(2):
                    flat = x2[h].rearrange("p b f -> p (b f)")
                    nc.vector.tensor_mul(sq[h], flat, flat)
                    ssq = sqw.tile([B_LOC, 576], F32, tag="ssq", name="ssq", bufs=2)
                    for g in range(2):
                        ps = sqps.tile([16, 288], F32, tag="sqps", name="sqps")
                        nc.tensor.matmul(
                            ps,
                            smat_r,
                            sq[h][:, g * 288 : (g + 1) * 288],
                            start=True,
                            stop=True,
                        )
                        nc.scalar.copy(
                            out=ssq[:, g * 288 : (g + 1) * 288],
                            in_=ps,
                        )
                    # f = (ssq/(1+ssq)) / sqrt(ssq+eps)
                    a = sqw.tile([B_LOC, 576], F32, tag="fa", name="fa", bufs=2)
                    r = sqw.tile([B_LOC, 576], F32, tag="fr", name="fr", bufs=2)
                    fT = sqw.tile([B_LOC, 576], F32, tag="fT", name="fT", bufs=2)
                    nc.vector.tensor_scalar_add(a, ssq, 1.0)
                    nc.vector.reciprocal(r, a)
                    nc.vector.tensor_mul(r, ssq, r)
                    nc.scalar.activation(
                        out=a, in_=ssq, func=AF.Sqrt, bias=epsb[:B_LOC, :], scale=1.0
                    )
                    nc.vector.reciprocal(a, a)
                    nc.vector.tensor_mul(fT, r, a)
                    fe = sqw.tile([128, 576], F32, tag="fexp", name="fexp", bufs=2)
                    nc.sync.dma_start(
                        out=fe, in_=_ap(fT, 0, [fT.ap[0], [0, 8], [1, 576]])
                    )
                    nc.vector.tensor_mul(
                        pc2[h].rearrange("p b f -> p (b f)"),
                        x2[h].rearrange("p b f -> p (b f)"),
                        fe,
                    )
            if debug:
                for h in range(2):
                    nc.gpsimd.dma_start(out=dbg["dbg_pc"][h], in_=pc2[h])

            def pc_chunk(n):
                """lhsT [(ik) 128, b] for flat (i,k)-chunk n (=hw*2+chalf)."""
                hw, chalf = n // 2, n % 2
                base = pc2[chalf]
                return _ap(base, hw, [base.ap[0], [36, B_LOC]])

            # ---------------- Phase 4: s1 = 0.1 * sum_i u_hat ----------------
            with tc.tile_pool(name="s1ps", bufs=1, space="PSUM") as s1psp:
                s1ps = s1psp.tile([B_LOC, 160], F32, tag="s1ps")
                for sc in range(6):
                    for m in range(12):
                        n = sc * 12 + m
                        nc.tensor.matmul(
                            s1ps,
                            pc_chunk(n),
                            wtcs[sc][:, m, :],
                            start=(n == 0),
                            stop=(n == 71),
                        )
                nc.scalar.mul(out=s1, in_=s1ps, mul=0.1)
            wts1_cm.__exit__(None, None, None)
            if debug:
                nc.sync.dma_start(out=dbg["dbg_s1"], in_=s1)

            def squash16(out_t, in_t, tmp_pool, nj):
                ssq = tmp_pool.tile([B_LOC, nj], F32, tag="sq_ssq")
                prod = tmp_pool.tile([B_LOC, nj * 16], F32, tag="sq_prod")
                nc.vector.tensor_mul(prod, in_t, in_t)
                nc.vector.reduce_sum(
                    ssq, prod.rearrange("p (j d) -> p j d", d=16), axis=AX.X
                )
                aa = tmp_pool.tile([B_LOC, nj], F32, tag="sq_a")
                rr = tmp_pool.tile([B_LOC, nj], F32, tag="sq_r")
                ff = tmp_pool.tile([B_LOC, nj], F32, tag="sq_f")
                nc.vector.tensor_scalar_add(aa, ssq, 1.0)
                nc.vector.reciprocal(rr, aa)
                nc.vector.tensor_mul(rr, ssq, rr)
                nc.scalar.activation(
                    out=aa, in_=ssq, func=AF.Sqrt, bias=epsb[:B_LOC, :], scale=1.0
                )
                nc.vector.reciprocal(aa, aa)
                nc.vector.tensor_mul(ff, rr, aa)
                fb = _ap(ff, 0, [ff.ap[0], [1, nj], [0, 16]])
                nc.vector.tensor_tensor(
                    out=out_t.rearrange("p (j d) -> p j d", d=16),
                    in0=in_t.rearrange("p (j d) -> p j d", d=16),
                    in1=fb,
                    op=ALU.mult,
                )

            with tc.tile_pool(name="rwork", bufs=1) as rw:
                squash16(v1, s1, rw, 10)
                if debug:
                    nc.sync.dma_start(out=dbg["dbg_v1"], in_=v1)

                # ---- v1 block-diagonal [ (j,d), (j',b) ] in bf16 ----
                with tc.tile_pool(name="tps", bufs=1, space="PSUM") as tps:
                    blkA = rw.tile([128, 160], BF16D, tag="blkA")
                    blkB = rw.tile([32, 160], BF16D, tag="blkB")
                    nc.vector.memset(blkA, 0.0)
                    nc.vector.memset(blkB, 0.0)
                    t1 = tps.tile([128, 16], F32, tag="t1")
                    nc.tensor.transpose(t1, v1[:, 0:128], ident[:16, :16])
                    t2 = tps.tile([32, 16], F32, tag="t2")
                    nc.tensor.transpose(t2, v1[:, 128:160], ident[:16, :16])
                    v1T1 = rw.tile([128, 16], BF16D, tag="v1T1")
                    v1T2 = rw.tile([32, 16], BF16D, tag="v1T2")
                    nc.scalar.copy(out=v1T1, in_=t1)
                    nc.scalar.copy(out=v1T2, in_=t2)
                    for j in range(8):
                        nc.sync.dma_start(
                            out=blkA[j * 16 : (j + 1) * 16, j * 16 : (j + 1) * 16],
                            in_=v1T1[j * 16 : (j + 1) * 16, :],
                        )
                    for j in range(8, 10):
                        nc.sync.dma_start(
                            out=blkB[(j - 8) * 16 : (j - 7) * 16, j * 16 : (j + 1) * 16],
                            in_=v1T2[(j - 8) * 16 : (j - 7) * 16, :],
                        )

                    # ---- A[(j',b), (ik)] = sum_d v1 * W  (bf16, full rate) ----
                    A1 = rw.tile([128, 9216], BF16D, tag="A1")
                    A2 = rw.tile([32, 9216], BF16D, tag="A2")
                    with (
                        tc.tile_pool(name="aps", bufs=2, space="PSUM") as aps,
                        tc.tile_pool(name="wt2s", bufs=1) as wt2s,
                    ):
                        for half in range(2):
                            hsl = slice(half * 4608, (half + 1) * 4608)
                            wa = wt2s.tile([128, 4608], BF16D, tag="wa", name="wa")
                            nc.scalar.dma_start(out=wa, in_=wt2ad[:, hsl])
                            wb = wt2s.tile([32, 4608], BF16D, tag="wb", name="wb")
                            nc.scalar.dma_start(out=wb, in_=wt2bd[:, hsl])
                            for cn in range(9):
                                gsl = slice(
                                    half * 4608 + cn * 512, half * 4608 + (cn + 1) * 512
                                )
                                lsl = slice(cn * 512, (cn + 1) * 512)
                                for mi, (mdst, msl) in enumerate(
                                    [(A1, slice(0, 128)), (A2, slice(128, 160))]
                                ):
                                    ps = aps.tile(
                                        [128 if mi == 0 else 32, 512],
                                        F32,
                                        tag=f"aps{mi}",
                                        name=f"aps{mi}",
                                    )
                                    nc.tensor.matmul(
                                        ps,
                                        blkA[:, msl],
                                        wa[:, lsl],
                                        start=True,
                                        stop=False,
                                    )
                                    nc.tensor.matmul(
                                        ps,
                                        blkB[:, msl],
                                        wb[:, lsl],
                                        start=False,
                                        stop=True,
                                    )
                                    nc.scalar.copy(out=mdst[:, gsl], in_=ps)



                # ---- pcT [b, (ik)] f32r via pipelined PE transposes ----
                pcT = rw.tile([B_LOC, 9216], BF16D, tag="pcT")
                with tc.tile_pool(name="tpp", bufs=6, space="PSUM") as tpp:
                    for hw in range(36):
                        for h in range(2):
                            base = pc2[h]
                            sl = _ap(base, hw, [base.ap[0], [36, B_LOC]])
                            ps = tpp.tile([16, 128], F32R, tag="tp36", name="tp36")
                            nc.tensor.transpose(ps, sl, identr)
                            nc.scalar.copy(
                                out=pcT[
                                    :, hw * 256 + h * 128 : hw * 256 + (h + 1) * 128
                                ],
                                in_=ps,
                            )
                # ---- pc_rep [(j,b), (ik)] bf16 ----
                R1 = rw.tile([128, 9216], BF16D, tag="R1")
                R2 = rw.tile([32, 9216], BF16D, tag="R2")
                for j in range(8):
                    nc.sync.dma_start(out=R1[j * 16 : (j + 1) * 16, :], in_=pcT)
                for j in range(2):
                    nc.sync.dma_start(out=R2[j * 16 : (j + 1) * 16, :], in_=pcT)

                if debug:
                    nc.gpsimd.dma_start(out=dbg["dbg_A"][0:128], in_=A1)
                    nc.gpsimd.dma_start(out=dbg["dbg_A"][128:160], in_=A2)
                    nc.gpsimd.dma_start(out=dbg["dbg_pcT"], in_=pcT)
                    nc.gpsimd.dma_start(out=dbg["dbg_R"][0:128], in_=R1)
                    nc.gpsimd.dma_start(out=dbg["dbg_R"][128:160], in_=R2)
                # ---- uv1[(j,b), i] = sum_k A * pc ----
                uv1a = rw.tile([128, 1152], F32, tag="uv1a")
                uv1b = rw.tile([32, 1152], F32, tag="uv1b")
                nc.vector.tensor_mul(A1, A1, R1)
                nc.vector.reduce_sum(
                    uv1a, A1.rearrange("p (i k) -> p i k", k=8), axis=AX.X
                )
                nc.vector.tensor_mul(A2, A2, R2)
                nc.vector.reduce_sum(
                    uv1b, A2.rearrange("p (i k) -> p i k", k=8), axis=AX.X
                )
                if debug:
                    nc.sync.dma_start(out=dbg["dbg_uv1"][0:128], in_=uv1a)
                    nc.sync.dma_start(out=dbg["dbg_uv1"][128:160], in_=uv1b)

                # ---- softmax numerator: exp (bf16; Z folded in during s2) ----
                e1 = rw.tile([128, 1152], BF16D, tag="e1")
                e2 = rw.tile([32, 1152], BF16D, tag="e2")
                nc.scalar.activation(out=e1, in_=uv1a, func=AF.Exp)
                nc.scalar.activation(out=e2, in_=uv1b, func=AF.Exp)
                # replicate exp x8 along free (i -> (i, rep)); reuse A slots
                e1rep = rw.tile([128, 9216], BF16D, tag="A1", name="e1rep")
                e2rep = rw.tile([32, 9216], BF16D, tag="A2", name="e2rep")
                nc.scalar.copy(
                    out=e1rep.rearrange("p (i r) -> p i r", r=8),
                    in_=_ap(e1, 0, [e1.ap[0], [1, 1152], [0, 8]]),
                )
                nc.scalar.copy(
                    out=e2rep.rearrange("p (i r) -> p i r", r=8),
                    in_=_ap(e2, 0, [e2.ap[0], [1, 1152], [0, 8]]),
                )

                # ---- s2: accumulate over 72 (ik)-chunks ----
                # per chunk: transpose+replicate exp via permuted-identity
                # matmuls -> psum [ (i,rep)=128, (b,j)=160 ]; Z = reduce over
                # j; g = rep * (1/Z) * pc; two s2 matmuls accumulate.
                with (
                    tc.tile_pool(name="s2w", bufs=4) as s2w,
                    tc.tile_pool(name="wts2", bufs=2) as wts2,
                    tc.tile_pool(name="reps", bufs=4, space="PSUM") as repsp,
                    tc.tile_pool(name="s2ps", bufs=1, space="PSUM") as s2psp,
                ):
                    psA = s2psp.tile([128, 160], F32, tag="psA")
                    psB = s2psp.tile([32, 160], F32, tag="psB")
                    for sc in range(6):
                        wtc2 = wts2.tile([128, 12, 160], F32R, tag="wtc2", name="wtc2")
                        nc.scalar.dma_start(out=wtc2, in_=wtd[sc])
                        for m in range(12):
                            n = sc * 12 + m
                            rep = repsp.tile([128, 160], F32, tag="rep", name="rep")
                            nc.tensor.matmul(
                                rep,
                                e1rep[:, 128 * n : 128 * (n + 1)],
                                ie1,
                                start=True,
                                stop=False,
                            )
                            nc.tensor.matmul(
                                rep,
                                e2rep[:, 128 * n : 128 * (n + 1)],
                                ie2,
                                start=False,
                                stop=True,
                            )
                            zc = s2w.tile([128, 16], F32, tag="zc", name="zc")
                            # rep free layout (j,b): strided view -> (b, j)
                            nc.vector.reduce_sum(
                                zc,
                                _ap(rep, 0, [rep.ap[0], [1, 16], [16, 10]]),
                                axis=AX.X,
                            )
                            zr = s2w.tile([128, 16], F32, tag="zr", name="zr")
                            nc.vector.reciprocal(zr, zc)
                            g = s2w.tile([128, 160], F32R, tag="g", name="g")
                            nc.vector.tensor_tensor(
                                out=g.rearrange("p (j b) -> p j b", j=10),
                                in0=rep.rearrange("p (j b) -> p j b", j=10),
                                in1=_ap(zr, 0, [zr.ap[0], [0, 10], [1, 16]]),
                                op=ALU.mult,
                            )
                            pcb = pc_chunk(n)
                            nc.vector.tensor_tensor(
                                out=g.rearrange("p (j b) -> p j b", j=10),
                                in0=g.rearrange("p (j b) -> p j b", j=10),
                                in1=_ap(pcb, 0, [pcb.ap[0], [0, 10], [36, B_LOC]]),
                                op=ALU.mult,
                            )
                            nc.tensor.matmul(
                                psA,
                                g[:, 0:128],
                                wtc2[:, m, :],
                                start=(n == 0),
                                stop=(n == 71),
                            )
                            nc.tensor.matmul(
                                psB,
                                g[:, 128:160],
                                wtc2[:, m, :],
                                start=(n == 0),
                                stop=(n == 71),
                            )
                    # diagonal extraction: psum row m=(j*16+b) -> s2[b, (j,:)]
                    sA = s2w.tile([128, 160], F32, tag="sA")
                    sB = s2w.tile([32, 160], F32, tag="sB")
                    nc.scalar.copy(out=sA, in_=psA)
                    nc.scalar.copy(out=sB, in_=psB)
                    for j in range(8):
                        nc.sync.dma_start(
                            out=s2[:, j * 16 : (j + 1) * 16],
                            in_=sA[j * 16 : (j + 1) * 16, j * 16 : (j + 1) * 16],
                        )
                    for j in range(8, 10):
                        nc.sync.dma_start(
                            out=s2[:, j * 16 : (j + 1) * 16],
                            in_=sB[(j - 8) * 16 : (j - 7) * 16, j * 16 : (j + 1) * 16],
                        )
                if debug:
                    nc.sync.dma_start(out=dbg["dbg_s2"], in_=s2)

                # ---- v2, norm, y_pred, mask ----
                ssq2 = rw.tile([B_LOC, 10], F32, tag="ssq2")
                v2sq = rw.tile([B_LOC, 160], F32, tag="v2sq")
                squash16(v2, s2, rw, 10)
                nc.vector.tensor_mul(v2sq, v2, v2)
                nc.vector.reduce_sum(
                    ssq2, v2sq.rearrange("p (j d) -> p j d", d=16), axis=AX.X
                )
                nc.scalar.activation(
                    out=norm, in_=ssq2, func=AF.Sqrt, bias=epsb[:B_LOC, :], scale=1.0
                )
                nc.sync.dma_start(out=normd, in_=norm)
                if debug:
                    nc.sync.dma_start(out=dbg["dbg_v2"], in_=v2)

                vmax = rw.tile([B_LOC, 8], F32, tag="vmax")
                vidx = rw.tile([B_LOC, 8], U32, tag="vidx")
                nc.vector.max_with_indices(vmax, vidx, norm)
                ypi = rw.tile([B_LOC, 1], I32, tag="ypi")
                nc.vector.tensor_copy(ypi, vidx[:, 0:1])
                nc.sync.dma_start(out=ypredd, in_=ypi)

                tgtf = rw.tile([B_LOC, 1], F32, tag="tgtf")
                tgti = rw.tile([B_LOC, 1], I32, tag="tgti")
                nc.sync.dma_start(out=tgti, in_=tgt)
                nc.vector.tensor_copy(tgtf, tgti)
                mask = rw.tile([B_LOC, 10], F32, tag="mask")
                nc.vector.tensor_scalar(
                    out=mask, in0=iota, scalar1=tgtf, scalar2=None, op0=ALU.is_equal
                )
                nc.vector.tensor_tensor(
                    out=h0.rearrange("p (j d) -> p j d", d=16),
                    in0=v2.rearrange("p (j d) -> p j d", d=16),
                    in1=_ap(mask, 0, [mask.ap[0], [1, 10], [0, 16]]),
                    op=ALU.mult,
                )
                if debug:
                    nc.sync.dma_start(out=dbg["dbg_h0"], in_=h0)

            # ---------------- Phase 5: decoder ----------------
            with (
                tc.tile_pool(name="dps", bufs=2, space="PSUM") as dps,
                tc.tile_pool(name="dwork", bufs=1) as dwk,
            ):
                h0T1 = dwk.tile([128, 16], F32R, tag="h0T1")
                h0T2 = dwk.tile([32, 16], F32R, tag="h0T2")
                ps = dps.tile([128, 16], F32, tag="dtp")
                nc.tensor.transpose(ps, h0[:, 0:128], ident[:16, :16])
                nc.scalar.copy(out=h0T1, in_=ps)
                ps = dps.tile([32, 16], F32, tag="dtp", name="dtp2")
                nc.tensor.transpose(ps, h0[:, 128:160], ident[:16, :16])
                nc.scalar.copy(out=h0T2, in_=ps)

                h1 = dwk.tile([B_LOC, 512], F32, tag="h1")
                hp = dps.tile([B_LOC, 512], F32, tag="mmp")
                nc.tensor.matmul(hp, h0T1, d1w, start=True, stop=False)
                nc.tensor.matmul(hp, h0T2, d1wb, start=False, stop=True)
                nc.vector.tensor_add(h1, hp, b1r)
                nc.scalar.activation(out=h1, in_=h1, func=AF.Relu)

                h1T = dwk.tile([128, 4, 16], F32R, tag="h1T")
                for c in range(4):
                    ps = dps.tile([128, 16], F32, tag="dtp", name="dtp3")
                    nc.tensor.transpose(
                        ps, h1[:, c * 128 : (c + 1) * 128], ident[:16, :16]
                    )
                    nc.scalar.copy(out=h1T[:, c, :], in_=ps)
                h2 = dwk.tile([B_LOC, 1024], F32, tag="h2")
                for nh in range(2):
                    hp2 = dps.tile([B_LOC, 512], F32, tag="mmp", name="hp2")
                    for c in range(4):
                        nc.tensor.matmul(
                            hp2,
                            h1T[:, c, :],
                            d2w[:, c, nh * 512 : (nh + 1) * 512],
                            start=(c == 0),
                            stop=(c == 3),
                        )
                    sl = slice(nh * 512, (nh + 1) * 512)
                    nc.vector.tensor_add(h2[:, sl], hp2, b2r[:, sl])
                    nc.scalar.activation(out=h2[:, sl], in_=h2[:, sl], func=AF.Relu)

                h2T = dwk.tile([128, 8, 16], F32R, tag="h2T")
                for c in range(8):
                    ps = dps.tile([128, 16], F32, tag="dtp", name="dtp4")
                    nc.tensor.transpose(
                        ps, h2[:, c * 128 : (c + 1) * 128], ident[:16, :16]
                    )
                    nc.scalar.copy(out=h2T[:, c, :], in_=ps)
                dec = dwk.tile([B_LOC, 784], F32, tag="dec")
                for nh, (n0, n1) in enumerate([(0, 512), (512, 784)]):
                    hp3 = dps.tile([B_LOC, 512], F32, tag="mmp", name="hp3")[
                        :, : n1 - n0
                    ]
                    for c in range(8):
                        nc.tensor.matmul(
                            hp3,
                            h2T[:, c, :],
                            d3w[:, c, n0:n1],
                            start=(c == 0),
                            stop=(c == 7),
                        )
                    nc.vector.tensor_add(dec[:, n0:n1], hp3, b3r[:, n0:n1])
                nc.scalar.activation(out=dec, in_=dec, func=AF.Sigmoid)
                nc.sync.dma_start(out=decd, in_=dec)

    nc.compile()
    return nc


# ---------------------------------------------------------------------------
# Host side
# ---------------------------------------------------------------------------


def prep_inputs(image, target, conv1_w, conv1_b, conv2_w, conv2_b, W,
                d1_w, d1_b, d2_w, d2_b, do_w, do_b):
    """Full inputs -> list of 8 per-core input maps."""
    image = np.asarray(image, np.float32)
    target = np.asarray(target)
    B = image.shape[0]
    per = B // N_CORES

    # host im2col for conv1: [B, 81, 400]
    img = image[:, :, :, 0]
    sw = np.lib.stride_tricks.sliding_window_view(img, (9, 9), axis=(1, 2))
    # sw: [B, 20, 20, 9, 9] -> [B, (kh kw), (oh ow)]
    pat = np.ascontiguousarray(sw.transpose(0, 3, 4, 1, 2)).reshape(B, 81, 400)

    w1 = np.asarray(conv1_w, np.float32).reshape(81, 256)
    b1 = np.zeros((128, 2), np.float32)
    b1[:, 0] = np.asarray(conv1_b, np.float32)[0:128]
    b1[:, 1] = np.asarray(conv1_b, np.float32)[128:256]
    w2 = (
        np.asarray(conv2_w, np.float32)
        .reshape(81, 2, 128, 2, 128)
        .transpose(0, 2, 1, 3, 4)  # tap, cin_low, cinh, couth, cout_low
        .reshape(27, 3, 128, 2, 2, 128)
        .transpose(0, 2, 1, 3, 4, 5)  # tg, cin_low, tap3, cinh, couth, cout
        .copy()
    )
    b2 = np.zeros((128, 2), np.float32)
    b2[:, 0] = np.asarray(conv2_b, np.float32)[0:128]
    b2[:, 1] = np.asarray(conv2_b, np.float32)[128:256]

    W0 = np.asarray(W, np.float32)[0]  # [1152, 10, 16, 8]
    wt = (
        W0.transpose(0, 3, 1, 2)  # i, k, j, d
        .reshape(6, 12, 128, 160)
        .transpose(0, 2, 1, 3)  # sc, ik_low(128), m, jd
        .copy()
    )
    wt2 = W0.transpose(1, 2, 0, 3).reshape(160, 9216)  # (j,d), (i,k)
    wt2a = wt2[0:128].astype(BF16)
    wt2b = wt2[128:160].astype(BF16)

    d1w = np.asarray(d1_w, np.float32)
    d2w = np.asarray(d2_w, np.float32).reshape(4, 128, 1024)
    d3w = np.asarray(do_w, np.float32).reshape(8, 128, 784)

    shared = dict(
        w1d=w1, b1d=b1, w2d=w2, b2d=b2, wtd=wt, wt2ad=wt2a, wt2bd=wt2b,
        d1wd=d1w, d2wd=d2w, d3wd=d3w,
        d1bd=np.asarray(d1_b, np.float32),
        d2bd=np.asarray(d2_b, np.float32),
        d3bd=np.asarray(do_b, np.float32),
    )
    maps = []
    for c in range(N_CORES):
        m = dict(shared)
        m["patd"] = pat[c * per : (c + 1) * per]
        m["tgt"] = target[c * per : (c + 1) * per].astype(np.int32).reshape(per, 1)
        maps.append(m)
    return maps


_NC_CACHE = {}


def _get_nc(debug=False):
    key = bool(debug)
    if key not in _NC_CACHE:
        _NC_CACHE[key] = build_nc(debug=key)
    return _NC_CACHE[key]


def kernel(**inputs):
    from concourse import bass_utils

    target = np.asarray(inputs["target"])
    maps = prep_inputs(**inputs)
    nc = _get_nc(debug=False)
    res = bass_utils.run_bass_kernel_spmd(nc, maps, core_ids=list(range(N_CORES)))
    outs = res.results
    B = N_CORES * B_LOC
    norm = np.concatenate([o["normd"] for o in outs], 0).reshape(B, 1, 10, 1, 1)
    ypred = np.concatenate([o["ypredd"] for o in outs], 0).reshape(B, 1)
    dec = np.concatenate([o["decd"] for o in outs], 0).reshape(B, 1, 784)
    if target.dtype == np.int64:
        ypred = ypred.astype(np.int64)
    else:
        ypred = ypred.astype(np.int32)
    return norm, ypred, dec


# revision 34
# speedup vs baseline: 1.9402x; 1.0451x over previous
"""CapsuleNetwork Trainium2 Bass kernel.

8-core data-parallel: batch 128 -> 16 per core, all weights replicated.
Per-core pipeline (all on device):
  conv1 (host-im2col patches x W1 matmul) -> conv2 (tap-accumulation
  matmul, fp32r full rate) -> primary-caps squash (partition-group
  sum-of-squares via ones-blockdiag matmul) -> dynamic routing
  (2 rounds; s1 via K=9216 matmul, A-tensor via block-diag-v1 matmul,
  softmax transpose+replicate via permuted-identity matmuls) ->
  norm/argmax/mask -> 3-layer decoder.
"""

import sys

import numpy as np

sys.path.insert(0, "/opt/trn_rl_repo")

import concourse.bass as bass
import concourse.bacc as bacc
import concourse.tile as tile
from concourse import mybir

import ml_dtypes

BF16 = ml_dtypes.bfloat16

EPS = 1e-7
B_LOC = 16  # images per core
N_CORES = 8
F32 = mybir.dt.float32
F32R = mybir.dt.float32r
BF16D = mybir.dt.bfloat16
I32 = mybir.dt.int32
U32 = mybir.dt.uint32
AF = mybir.ActivationFunctionType
ALU = mybir.AluOpType
AX = mybir.AxisListType


def _ap(base, offset_elems, dims):
    """Raw AP at base.offset + offset_elems with explicit [step,count] dims."""
    return bass.AP(tensor=base.tensor, offset=base.offset + offset_elems, ap=dims)


def build_nc(debug=False):
    nc = bacc.Bacc("TRN2", target_bir_lowering=False, debug=False)

    # ---------------- DRAM I/O ----------------
    patd = nc.dram_tensor("patd", [B_LOC, 81, 400], F32R, kind="ExternalInput").ap()
    tgt = nc.dram_tensor("tgt", [B_LOC, 1], I32, kind="ExternalInput").ap()
    w1d = nc.dram_tensor("w1d", [81, 256], F32R, kind="ExternalInput").ap()
    b1d = nc.dram_tensor("b1d", [128, 2], F32, kind="ExternalInput").ap()
    # (tapgroup 27, cin_low 128, tap3, cinh, couth, cout_low)
    w2d = nc.dram_tensor(
        "w2d", [27, 128, 3, 2, 2, 128], F32R, kind="ExternalInput"
    ).ap()
    b2d = nc.dram_tensor("b2d", [128, 2], F32, kind="ExternalInput").ap()
    # (super-chunk 6, cin_low 128, m 12, jd 160)
    wtd = nc.dram_tensor("wtd", [6, 128, 12, 160], F32R, kind="ExternalInput").ap()
    wt2ad = nc.dram_tensor("wt2ad", [128, 9216], BF16D, kind="ExternalInput").ap()
    wt2bd = nc.dram_tensor("wt2bd", [32, 9216], BF16D, kind="ExternalInput").ap()
    d1wd = nc.dram_tensor("d1wd", [160, 512], F32R, kind="ExternalInput").ap()
    d2wd = nc.dram_tensor("d2wd", [4, 128, 1024], F32R, kind="ExternalInput").ap()
    d3wd = nc.dram_tensor("d3wd", [8, 128, 784], F32R, kind="ExternalInput").ap()
    d1bd = nc.dram_tensor("d1bd", [512], F32, kind="ExternalInput").ap()
    d2bd = nc.dram_tensor("d2bd", [1024], F32, kind="ExternalInput").ap()
    d3bd = nc.dram_tensor("d3bd", [784], F32, kind="ExternalInput").ap()

    normd = nc.dram_tensor("normd", [B_LOC, 10], F32, kind="ExternalOutput").ap()
    ypredd = nc.dram_tensor("ypredd", [B_LOC, 1], I32, kind="ExternalOutput").ap()
    decd = nc.dram_tensor("decd", [B_LOC, 784], F32, kind="ExternalOutput").ap()

    dbg = {}
    if debug:
        for name, shape in [
            ("dbg_x1", [2, 128, 16, 400]),
            ("dbg_x2", [2, 128, 16, 36]),
            ("dbg_pc", [2, 128, 16, 36]),
            ("dbg_s1", [B_LOC, 160]),
            ("dbg_v1", [B_LOC, 160]),
            ("dbg_uv1", [160, 1152]),
            ("dbg_A", [160, 9216]),
            ("dbg_pcT", [16, 9216]),
            ("dbg_R", [160, 9216]),
            ("dbg_s2", [B_LOC, 160]),
            ("dbg_v2", [B_LOC, 160]),
            ("dbg_h0", [B_LOC, 160]),
        ]:
            dbg[name] = nc.dram_tensor(name, shape, F32, kind="ExternalOutput").ap()

    # ---------------- constants (embedded in NEFF) ----------------
    ident_np = np.eye(128, dtype=np.float32)
    identd = nc.inline_tensor(ident_np, "identc").ap()
    s_np = np.zeros((128, 16), np.float32)
    for c in range(128):
        s_np[c, c // 8] = 1.0
    sd = nc.inline_tensor(s_np, "sconst").ap()
    iota_np = np.arange(10, dtype=np.float32).reshape(1, 10)
    iotad = nc.inline_tensor(iota_np, "iotac").ap()
    # extended identities: row (j,b) -> col (j*16+b)
    ie1_np = np.zeros((128, 160), np.float32)
    ie1_np[:, 0:128] = np.eye(128)
    ie2_np = np.zeros((32, 160), np.float32)
    ie2_np[:, 128:160] = np.eye(32)
    ie1d = nc.inline_tensor(ie1_np.astype(BF16), "ie1c").ap()
    ie2d = nc.inline_tensor(ie2_np.astype(BF16), "ie2c").ap()

    with tile.TileContext(nc) as tc:
        with tc.tile_pool(name="persist", bufs=1) as pp:
            # ---- persistent tiles / consts ----
            ident = pp.tile([128, 128], F32, tag="ident")
            nc.sync.dma_start(out=ident, in_=identd)
            identr = pp.tile([128, 128], F32R, tag="identr")
            identb = pp.tile([128, 128], BF16D, tag="identb")
            smat = pp.tile([128, 16], F32, tag="smat")
            smat_r = pp.tile([128, 16], F32R, tag="smat_r")
            iota = pp.tile([B_LOC, 10], F32, tag="iota")
            ie1 = pp.tile([128, 160], BF16D, tag="ie1")
            ie2 = pp.tile([32, 160], BF16D, tag="ie2")
            epsb = pp.tile([128, 1], F32, tag="epsb")
            d1w = pp.tile([128, 512], F32R, tag="d1w_hi")
            d1wb = pp.tile([32, 512], F32R, tag="d1w_lo")
            d2w = pp.tile([128, 4, 1024], F32R, tag="d2w")
            d3w = pp.tile([128, 8, 784], F32R, tag="d3w")
            b1r = pp.tile([B_LOC, 512], F32, tag="b1r")
            b2r = pp.tile([B_LOC, 1024], F32, tag="b2r")
            b3r = pp.tile([B_LOC, 784], F32, tag="b3r")
            w1 = pp.tile([81, 256], F32R, tag="w1")
            nc.sync.dma_start(out=w1, in_=w1d)
            b1 = pp.tile([128, 2], F32, tag="b1")
            nc.sync.dma_start(out=b1, in_=b1d)
            b2 = pp.tile([128, 2], F32, tag="b2")
            nc.sync.dma_start(out=b2, in_=b2d)

            # conv2 output (post-relu), layout [cout_low, (b, hw36)] x couth
            x2 = [pp.tile([128, B_LOC, 36], F32, tag=f"x2_{h}", name=f"x2_{h}") for h in range(2)]
            pc2 = [pp.tile([128, B_LOC, 36], F32R, tag=f"pc2_{h}", name=f"pc2_{h}") for h in range(2)]

            s1 = pp.tile([B_LOC, 160], F32, tag="s1")
            v1 = pp.tile([B_LOC, 160], F32, tag="v1")
            s2 = pp.tile([B_LOC, 160], F32, tag="s2")
            v2 = pp.tile([B_LOC, 160], F32, tag="v2")
            h0 = pp.tile([B_LOC, 160], F32, tag="h0")
            norm = pp.tile([B_LOC, 10], F32, tag="norm")

            # ---------------- Phase 1: conv1 ----------------
            wts1_cm = tc.tile_pool(name="wts1", bufs=6)
            wts1 = wts1_cm.__enter__()
            wtcs = [
                wts1.tile(
                    [128, 12, 160], F32R, tag="wtc", name=f"wtc{sc}", bufs=6
                )
                for sc in range(6)
            ]
            with (
                tc.tile_pool(name="c1work", bufs=3) as c1w,
                tc.tile_pool(name="c1ps", bufs=4, space="PSUM") as c1p,
                tc.tile_pool(name="x1pool", bufs=1) as x1pool,
            ):
                x1 = [
                    x1pool.tile([128, B_LOC, 400], F32R, tag=f"x1_{h}", name=f"x1_{h}")
                    for h in range(2)
                ]
                for b in range(B_LOC):
                    patch = c1w.tile([81, 400], F32R, tag="patch")
                    nc.sync.dma_start(out=patch, in_=patd[b])
                    for h in range(2):
                        ps = c1p.tile([128, 400], F32, tag="c1ps")
                        nc.tensor.matmul(
                            ps,
                            w1[:, h * 128 : (h + 1) * 128],
                            patch,
                            start=True,
                            stop=True,
                        )
                        nc.scalar.activation(
                            out=x1[h][:, b, :],
                            in_=ps,
                            func=AF.Relu,
                            bias=b1[:, h : h + 1],
                            scale=1.0,
                        )
                if debug:
                    for h in range(2):
                        nc.gpsimd.dma_start(out=dbg["dbg_x1"][h], in_=x1[h])

                # ---- prefetch for later phases (after conv1 is queued) ----
                nc.vector.tensor_copy(identr, ident)
                nc.vector.tensor_copy(identb, ident)
                nc.sync.dma_start(out=smat, in_=sd)
                nc.vector.tensor_copy(smat_r, smat)
                nc.sync.dma_start(
                    out=iota, in_=_ap(iotad, 0, [[0, B_LOC], [1, 10]])
                )
                nc.sync.dma_start(out=ie1, in_=ie1d)
                nc.sync.dma_start(out=ie2, in_=ie2d)
                nc.vector.memset(epsb, EPS)
                nc.scalar.dma_start(out=d1w, in_=d1wd[0:128])
                nc.scalar.dma_start(out=d1wb, in_=d1wd[128:160])
                for c in range(4):
                    nc.scalar.dma_start(out=d2w[:, c, :], in_=d2wd[c])
                for c in range(8):
                    nc.scalar.dma_start(out=d3w[:, c, :], in_=d3wd[c])
                nc.scalar.dma_start(
                    out=b1r, in_=_ap(d1bd, 0, [[0, B_LOC], [1, 512]])
                )
                nc.scalar.dma_start(
                    out=b2r, in_=_ap(d2bd, 0, [[0, B_LOC], [1, 1024]])
                )
                nc.scalar.dma_start(
                    out=b3r, in_=_ap(d3bd, 0, [[0, B_LOC], [1, 784]])
                )
                for sc in range(6):
                    nc.scalar.dma_start(out=wtcs[sc], in_=wtd[sc])

                # ---------------- Phase 2: conv2 ----------------
                with (
                    tc.tile_pool(name="w2stream", bufs=3) as w2s,
                    tc.tile_pool(name="qps", bufs=1, space="PSUM") as qps,
                ):
                    q = [
                        [qps.tile([128, 288], F32, tag=f"q_{h}_{g}", name=f"q_{h}_{g}") for g in range(2)]
                        for h in range(2)
                    ]
                    for tg in range(27):
                        w2t = w2s.tile([128, 3, 2, 2, 128], F32R, tag="w2t")
                        nc.sync.dma_start(out=w2t, in_=w2d[tg])
                        for t3 in range(3):
                            tap = tg * 3 + t3
                            kh, kw = tap // 9, tap % 9
                            for cinh in range(2):
                                base = x1[cinh]
                                for couth in range(2):
                                    lhsT = w2t[:, t3, cinh, couth, :]
                                    for g in range(2):
                                        off = (g * 8) * 400 + kh * 20 + kw
                                        rhs = _ap(
                                            base,
                                            off,
                                            [base.ap[0], [400, 8], [40, 6], [2, 6]],
                                        )
                                        nc.tensor.matmul(
                                            q[couth][g],
                                            lhsT,
                                            rhs,
                                            start=(tap == 0 and cinh == 0),
                                            stop=(tap == 80 and cinh == 1),
                                        )
                    for couth in range(2):
                        for g in range(2):
                            nc.scalar.activation(
                                out=x2[couth][:, g * 8 : (g + 1) * 8, :],
                                in_=q[couth][g].rearrange("p (b f) -> p b f", f=36),
                                func=AF.Relu,
                                bias=b2[:, couth : couth + 1],
                                scale=1.0,
                            )
            if debug:
                for h in range(2):
                    nc.sync.dma_start(out=dbg["dbg_x2"][h], in_=x2[h])

            # ---------------- Phase 3: squash -> pc2 ----------------
            with (
                tc.tile_pool(name="sqw", bufs=1) as sqw,
                tc.tile_pool(name="sqps", bufs=4, space="PSUM") as sqps,
            ):
                sq = [sqw.tile([128, B_LOC * 36], F32R, tag=f"sq_{h}", name=f"sq_{h}") for h in range(2)]
                for h in range(2):
                    flat = x2[h].rearrange("p b f -> p (b f)")
                    nc.vector.tensor_mul(sq[h], flat, flat)
                    ssq = sqw.tile([B_LOC, 576], F32, tag="ssq", name="ssq", bufs=2)
                    for g in range(2):
                        ps = sqps.tile([16, 288], F32, tag="sqps", name="sqps")
                        nc.tensor.matmul(
                            ps,
                            smat_r,
                            sq[h][:, g * 288 : (g + 1) * 288],
                            start=True,
                            stop=True,
                        )
                        nc.scalar.copy(
                            out=ssq[:, g * 288 : (g + 1) * 288],
                            in_=ps,
                        )
                    # f = (ssq/(1+ssq)) / sqrt(ssq+eps)
                    a = sqw.tile([B_LOC, 576], F32, tag="fa", name="fa", bufs=2)
                    r = sqw.tile([B_LOC, 576], F32, tag="fr", name="fr", bufs=2)
                    fT = sqw.tile([B_LOC, 576], F32, tag="fT", name="fT", bufs=2)
                    nc.vector.tensor_scalar_add(a, ssq, 1.0)
                    nc.vector.reciprocal(r, a)
                    nc.vector.tensor_mul(r, ssq, r)
                    nc.scalar.activation(
                        out=a, in_=ssq, func=AF.Sqrt, bias=epsb[:B_LOC, :], scale=1.0
                    )
                    nc.vector.reciprocal(a, a)
                    nc.vector.tensor_mul(fT, r, a)
                    fe = sqw.tile([128, 576], F32, tag="fexp", name="fexp", bufs=2)
                    nc.sync.dma_start(
                        out=fe, in_=_ap(fT, 0, [fT.ap[0], [0, 8], [1, 576]])
                    )
                    nc.vector.tensor_mul(
                        pc2[h].rearrange("p b f -> p (b f)"),
                        x2[h].rearrange("p b f -> p (b f)"),
                        fe,
                    )
            if debug:
                for h in range(2):
                    nc.gpsimd.dma_start(out=dbg["dbg_pc"][h], in_=pc2[h])

            def pc_chunk(n):
                """lhsT [(ik) 128, b] for flat (i,k)-chunk n (=hw*2+chalf)."""
                hw, chalf = n // 2, n % 2
                base = pc2[chalf]
                return _ap(base, hw, [base.ap[0], [36, B_LOC]])

            # ---------------- Phase 4: s1 = 0.1 * sum_i u_hat ----------------
            with tc.tile_pool(name="s1ps", bufs=1, space="PSUM") as s1psp:
                s1ps = s1psp.tile([B_LOC, 160], F32, tag="s1ps")
                for sc in range(6):
                    for m in range(12):
                        n = sc * 12 + m
                        nc.tensor.matmul(
                            s1ps,
                            pc_chunk(n),
                            wtcs[sc][:, m, :],
                            start=(n == 0),
                            stop=(n == 71),
                        )
                nc.scalar.mul(out=s1, in_=s1ps, mul=0.1)
            wts1_cm.__exit__(None, None, None)
            if debug:
                nc.sync.dma_start(out=dbg["dbg_s1"], in_=s1)

            def squash16(out_t, in_t, tmp_pool, nj):
                ssq = tmp_pool.tile([B_LOC, nj], F32, tag="sq_ssq")
                prod = tmp_pool.tile([B_LOC, nj * 16], F32, tag="sq_prod")
                nc.vector.tensor_mul(prod, in_t, in_t)
                nc.vector.reduce_sum(
                    ssq, prod.rearrange("p (j d) -> p j d", d=16), axis=AX.X
                )
                aa = tmp_pool.tile([B_LOC, nj], F32, tag="sq_a")
                rr = tmp_pool.tile([B_LOC, nj], F32, tag="sq_r")
                ff = tmp_pool.tile([B_LOC, nj], F32, tag="sq_f")
                nc.vector.tensor_scalar_add(aa, ssq, 1.0)
                nc.vector.reciprocal(rr, aa)
                nc.vector.tensor_mul(rr, ssq, rr)
                nc.scalar.activation(
                    out=aa, in_=ssq, func=AF.Sqrt, bias=epsb[:B_LOC, :], scale=1.0
                )
                nc.vector.reciprocal(aa, aa)
                nc.vector.tensor_mul(ff, rr, aa)
                fb = _ap(ff, 0, [ff.ap[0], [1, nj], [0, 16]])
                nc.vector.tensor_tensor(
                    out=out_t.rearrange("p (j d) -> p j d", d=16),
                    in0=in_t.rearrange("p (j d) -> p j d", d=16),
                    in1=fb,
                    op=ALU.mult,
                )

            with tc.tile_pool(name="rwork", bufs=1) as rw:
                squash16(v1, s1, rw, 10)
                if debug:
                    nc.sync.dma_start(out=dbg["dbg_v1"], in_=v1)

                # ---- v1 block-diagonal [ (j,d), (j',b) ] in bf16 ----
                with tc.tile_pool(name="tps", bufs=1, space="PSUM") as tps:
                    blkA = rw.tile([128, 160], BF16D, tag="blkA")
                    blkB = rw.tile([32, 160], BF16D, tag="blkB")
                    nc.vector.memset(blkA, 0.0)
                    nc.vector.memset(blkB, 0.0)
                    t1 = tps.tile([128, 16], F32, tag="t1")
                    nc.tensor.transpose(t1, v1[:, 0:128], ident[:16, :16])
                    t2 = tps.tile([32, 16], F32, tag="t2")
                    nc.tensor.transpose(t2, v1[:, 128:160], ident[:16, :16])
                    v1T1 = rw.tile([128, 16], BF16D, tag="v1T1")
                    v1T2 = rw.tile([32, 16], BF16D, tag="v1T2")
                    nc.scalar.copy(out=v1T1, in_=t1)
                    nc.scalar.copy(out=v1T2, in_=t2)
                    for j in range(8):
                        nc.sync.dma_start(
                            out=blkA[j * 16 : (j + 1) * 16, j * 16 : (j + 1) * 16],
                            in_=v1T1[j * 16 : (j + 1) * 16, :],
                        )
                    for j in range(8, 10):
                        nc.sync.dma_start(
                            out=blkB[(j - 8) * 16 : (j - 7) * 16, j * 16 : (j + 1) * 16],
                            in_=v1T2[(j - 8) * 16 : (j - 7) * 16, :],
                        )

                    # ---- A[(j',b), (ik)] = sum_d v1 * W  (bf16, full rate) ----
                    A1 = rw.tile([128, 9216], BF16D, tag="A1")
                    A2 = rw.tile([32, 9216], BF16D, tag="A2")
                    with (
                        tc.tile_pool(name="aps", bufs=2, space="PSUM") as aps,
                        tc.tile_pool(name="wt2s", bufs=1) as wt2s,
                    ):
                        for half in range(2):
                            hsl = slice(half * 4608, (half + 1) * 4608)
                            wa = wt2s.tile([128, 4608], BF16D, tag="wa", name="wa")
                            nc.scalar.dma_start(out=wa, in_=wt2ad[:, hsl])
                            wb = wt2s.tile([32, 4608], BF16D, tag="wb", name="wb")
                            nc.scalar.dma_start(out=wb, in_=wt2bd[:, hsl])
                            for cn in range(9):
                                gsl = slice(
                                    half * 4608 + cn * 512, half * 4608 + (cn + 1) * 512
                                )
                                lsl = slice(cn * 512, (cn + 1) * 512)
                                for mi, (mdst, msl) in enumerate(
                                    [(A1, slice(0, 128)), (A2, slice(128, 160))]
                                ):
                                    ps = aps.tile(
                                        [128 if mi == 0 else 32, 512],
                                        F32,
                                        tag=f"aps{mi}",
                                        name=f"aps{mi}",
                                    )
                                    nc.tensor.matmul(
                                        ps,
                                        blkA[:, msl],
                                        wa[:, lsl],
                                        start=True,
                                        stop=False,
                                    )
                                    nc.tensor.matmul(
                                        ps,
                                        blkB[:, msl],
                                        wb[:, lsl],
                                        start=False,
                                        stop=True,
                                    )
                                    nc.scalar.copy(out=mdst[:, gsl], in_=ps)



                # ---- pcT [b, (ik)] f32r via pipelined PE transposes ----
                pcT = rw.tile([B_LOC, 9216], BF16D, tag="pcT")
                with tc.tile_pool(name="tpp", bufs=6, space="PSUM") as tpp:
                    for hw in range(36):
                        for h in range(2):
                            base = pc2[h]
                            sl = _ap(base, hw, [base.ap[0], [36, B_LOC]])
                            ps = tpp.tile([16, 128], F32R, tag="tp36", name="tp36")
                            nc.tensor.transpose(ps, sl, identr)
                            nc.scalar.copy(
                                out=pcT[
                                    :, hw * 256 + h * 128 : hw * 256 + (h + 1) * 128
                                ],
                                in_=ps,
                            )
                # ---- pc_rep [(j,b), (ik)] bf16 ----
                R1 = rw.tile([128, 9216], BF16D, tag="R1")
                R2 = rw.tile([32, 9216], BF16D, tag="R2")
                for j in range(8):
                    nc.sync.dma_start(out=R1[j * 16 : (j + 1) * 16, :], in_=pcT)
                for j in range(2):
                    nc.sync.dma_start(out=R2[j * 16 : (j + 1) * 16, :], in_=pcT)

                if debug:
                    nc.gpsimd.dma_start(out=dbg["dbg_A"][0:128], in_=A1)
                    nc.gpsimd.dma_start(out=dbg["dbg_A"][128:160], in_=A2)
                    nc.gpsimd.dma_start(out=dbg["dbg_pcT"], in_=pcT)
                    nc.gpsimd.dma_start(out=dbg["dbg_R"][0:128], in_=R1)
                    nc.gpsimd.dma_start(out=dbg["dbg_R"][128:160], in_=R2)
                # ---- uv1[(j,b), i] = sum_k A * pc ----
                uv1a = rw.tile([128, 1152], F32, tag="uv1a")
                uv1b = rw.tile([32, 1152], F32, tag="uv1b")
                nc.vector.tensor_mul(A1, A1, R1)
                nc.vector.reduce_sum(
                    uv1a, A1.rearrange("p (i k) -> p i k", k=8), axis=AX.X
                )
                nc.vector.tensor_mul(A2, A2, R2)
                nc.vector.reduce_sum(
                    uv1b, A2.rearrange("p (i k) -> p i k", k=8), axis=AX.X
                )
                if debug:
                    nc.sync.dma_start(out=dbg["dbg_uv1"][0:128], in_=uv1a)
                    nc.sync.dma_start(out=dbg["dbg_uv1"][128:160], in_=uv1b)

                # ---- softmax numerator: exp (bf16; Z folded in during s2) ----
                e1 = rw.tile([128, 1152], BF16D, tag="e1")
                e2 = rw.tile([32, 1152], BF16D, tag="e2")
                nc.scalar.activation(out=e1, in_=uv1a, func=AF.Exp)
                nc.scalar.activation(out=e2, in_=uv1b, func=AF.Exp)
                # replicate exp x8 along free (i -> (i, rep)); reuse A slots
                e1rep = rw.tile([128, 9216], BF16D, tag="A1", name="e1rep")
                e2rep = rw.tile([32, 9216], BF16D, tag="A2", name="e2rep")
                nc.scalar.copy(
                    out=e1rep.rearrange("p (i r) -> p i r", r=8),
                    in_=_ap(e1, 0, [e1.ap[0], [1, 1152], [0, 8]]),
                )
                nc.scalar.copy(
                    out=e2rep.rearrange("p (i r) -> p i r", r=8),
                    in_=_ap(e2, 0, [e2.ap[0], [1, 1152], [0, 8]]),
                )

                # ---- s2: accumulate over 72 (ik)-chunks ----
                # per chunk: transpose+replicate exp via permuted-identity
                # matmuls -> psum [ (i,rep)=128, (b,j)=160 ]; Z = reduce over
                # j; g = rep * (1/Z) * pc; two s2 matmuls accumulate.
                with (
                    tc.tile_pool(name="s2w", bufs=4) as s2w,
                    tc.tile_pool(name="wts2", bufs=2) as wts2,
                    tc.tile_pool(name="reps", bufs=4, space="PSUM") as repsp,
                    tc.tile_pool(name="s2ps", bufs=1, space="PSUM") as s2psp,
                ):
                    psA = s2psp.tile([128, 160], F32, tag="psA")
                    psB = s2psp.tile([32, 160], F32, tag="psB")
                    for sc in range(6):
                        wtc2 = wts2.tile([128, 12, 160], F32R, tag="wtc2", name="wtc2")
                        nc.scalar.dma_start(out=wtc2, in_=wtd[sc])
                        for m in range(12):
                            n = sc * 12 + m
                            rep = repsp.tile([128, 160], F32, tag="rep", name="rep")
                            nc.tensor.matmul(
                                rep,
                                e1rep[:, 128 * n : 128 * (n + 1)],
                                ie1,
                                start=True,
                                stop=False,
                            )
                            nc.tensor.matmul(
                                rep,
                                e2rep[:, 128 * n : 128 * (n + 1)],
                                ie2,
                                start=False,
                                stop=True,
                            )
                            zc = s2w.tile([128, 16], F32, tag="zc", name="zc")
                            # rep free layout (j,b): strided view -> (b, j)
                            nc.vector.reduce_sum(
                                zc,
                                _ap(rep, 0, [rep.ap[0], [1, 16], [16, 10]]),
                                axis=AX.X,
                            )
                            zr = s2w.tile([128, 16], F32, tag="zr", name="zr")
                            nc.vector.reciprocal(zr, zc)
                            g = s2w.tile([128, 160], F32R, tag="g", name="g")
                            nc.vector.tensor_tensor(
                                out=g.rearrange("p (j b) -> p j b", j=10),
                                in0=rep.rearrange("p (j b) -> p j b", j=10),
                                in1=_ap(zr, 0, [zr.ap[0], [0, 10], [1, 16]]),
                                op=ALU.mult,
                            )
                            pcb = pc_chunk(n)
                            nc.vector.tensor_tensor(
                                out=g.rearrange("p (j b) -> p j b", j=10),
                                in0=g.rearrange("p (j b) -> p j b", j=10),
                                in1=_ap(pcb, 0, [pcb.ap[0], [0, 10], [36, B_LOC]]),
                                op=ALU.mult,
                            )
                            nc.tensor.matmul(
                                psA,
                                g[:, 0:128],
                                wtc2[:, m, :],
                                start=(n == 0),
                                stop=(n == 71),
                            )
                            nc.tensor.matmul(
                                psB,
                                g[:, 128:160],
                                wtc2[:, m, :],
                                start=(n == 0),
                                stop=(n == 71),
                            )
                    # diagonal extraction: psum row m=(j*16+b) -> s2[b, (j,:)]
                    sA = s2w.tile([128, 160], F32, tag="sA")
                    sB = s2w.tile([32, 160], F32, tag="sB")
                    nc.scalar.copy(out=sA, in_=psA)
                    nc.scalar.copy(out=sB, in_=psB)
                    for j in range(8):
                        nc.sync.dma_start(
                            out=s2[:, j * 16 : (j + 1) * 16],
                            in_=sA[j * 16 : (j + 1) * 16, j * 16 : (j + 1) * 16],
                        )
                    for j in range(8, 10):
                        nc.sync.dma_start(
                            out=s2[:, j * 16 : (j + 1) * 16],
                            in_=sB[(j - 8) * 16 : (j - 7) * 16, j * 16 : (j + 1) * 16],
                        )
                if debug:
                    nc.sync.dma_start(out=dbg["dbg_s2"], in_=s2)

                # ---- v2, norm, y_pred, mask ----
                ssq2 = rw.tile([B_LOC, 10], F32, tag="ssq2")
                v2sq = rw.tile([B_LOC, 160], F32, tag="v2sq")
                squash16(v2, s2, rw, 10)
                nc.vector.tensor_mul(v2sq, v2, v2)
                nc.vector.reduce_sum(
                    ssq2, v2sq.rearrange("p (j d) -> p j d", d=16), axis=AX.X
                )
                nc.scalar.activation(
                    out=norm, in_=ssq2, func=AF.Sqrt, bias=epsb[:B_LOC, :], scale=1.0
                )
                nc.sync.dma_start(out=normd, in_=norm)
                if debug:
                    nc.sync.dma_start(out=dbg["dbg_v2"], in_=v2)

                vmax = rw.tile([B_LOC, 8], F32, tag="vmax")
                vidx = rw.tile([B_LOC, 8], U32, tag="vidx")
                nc.vector.max_with_indices(vmax, vidx, norm)
                ypi = rw.tile([B_LOC, 1], I32, tag="ypi")
                nc.vector.tensor_copy(ypi, vidx[:, 0:1])
                nc.sync.dma_start(out=ypredd, in_=ypi)

                tgtf = rw.tile([B_LOC, 1], F32, tag="tgtf")
                tgti = rw.tile([B_LOC, 1], I32, tag="tgti")
                nc.sync.dma_start(out=tgti, in_=tgt)
                nc.vector.tensor_copy(tgtf, tgti)
                mask = rw.tile([B_LOC, 10], F32, tag="mask")
                nc.vector.tensor_scalar(
                    out=mask, in0=iota, scalar1=tgtf, scalar2=None, op0=ALU.is_equal
                )
                nc.vector.tensor_tensor(
                    out=h0.rearrange("p (j d) -> p j d", d=16),
                    in0=v2.rearrange("p (j d) -> p j d", d=16),
                    in1=_ap(mask, 0, [mask.ap[0], [1, 10], [0, 16]]),
                    op=ALU.mult,
                )
                if debug:
                    nc.sync.dma_start(out=dbg["dbg_h0"], in_=h0)

            # ---------------- Phase 5: decoder ----------------
            with (
                tc.tile_pool(name="dps", bufs=2, space="PSUM") as dps,
                tc.tile_pool(name="dwork", bufs=1) as dwk,
            ):
                h0T1 = dwk.tile([128, 16], F32R, tag="h0T1")
                h0T2 = dwk.tile([32, 16], F32R, tag="h0T2")
                ps = dps.tile([128, 16], F32, tag="dtp")
                nc.tensor.transpose(ps, h0[:, 0:128], ident[:16, :16])
                nc.scalar.copy(out=h0T1, in_=ps)
                ps = dps.tile([32, 16], F32, tag="dtp", name="dtp2")
                nc.tensor.transpose(ps, h0[:, 128:160], ident[:16, :16])
                nc.scalar.copy(out=h0T2, in_=ps)

                h1 = dwk.tile([B_LOC, 512], F32, tag="h1")
                hp = dps.tile([B_LOC, 512], F32, tag="mmp")
                nc.tensor.matmul(hp, h0T1, d1w, start=True, stop=False)
                nc.tensor.matmul(hp, h0T2, d1wb, start=False, stop=True)
                nc.vector.tensor_add(h1, hp, b1r)
                nc.scalar.activation(out=h1, in_=h1, func=AF.Relu)

                h1T = dwk.tile([128, 4, 16], F32R, tag="h1T")
                for c in range(4):
                    ps = dps.tile([128, 16], F32, tag="dtp", name="dtp3")
                    nc.tensor.transpose(
                        ps, h1[:, c * 128 : (c + 1) * 128], ident[:16, :16]
                    )
                    nc.scalar.copy(out=h1T[:, c, :], in_=ps)
                h2 = dwk.tile([B_LOC, 1024], F32, tag="h2")
                for nh in range(2):
                    hp2 = dps.tile([B_LOC, 512], F32, tag="mmp", name="hp2")
                    for c in range(4):
                        nc.tensor.matmul(
                            hp2,
                            h1T[:, c, :],
                            d2w[:, c, nh * 512 : (nh + 1) * 512],
                            start=(c == 0),
                            stop=(c == 3),
                        )
                    sl = slice(nh * 512, (nh + 1) * 512)
                    nc.vector.tensor_add(h2[:, sl], hp2, b2r[:, sl])
                    nc.scalar.activation(out=h2[:, sl], in_=h2[:, sl], func=AF.Relu)

                h2T = dwk.tile([128, 8, 16], F32R, tag="h2T")
                for c in range(8):
                    ps = dps.tile([128, 16], F32, tag="dtp", name="dtp4")
                    nc.tensor.transpose(
                        ps, h2[:, c * 128 : (c + 1) * 128], ident[:16, :16]
                    )
                    nc.scalar.copy(out=h2T[:, c, :], in_=ps)
                dec = dwk.tile([B_LOC, 784], F32, tag="dec")
                for nh, (n0, n1) in enumerate([(0, 512), (512, 784)]):
                    hp3 = dps.tile([B_LOC, 512], F32, tag="mmp", name="hp3")[
                        :, : n1 - n0
                    ]
                    for c in range(8):
                        nc.tensor.matmul(
                            hp3,
                            h2T[:, c, :],
                            d3w[:, c, n0:n1],
                            start=(c == 0),
                            stop=(c == 7),
                        )
                    nc.vector.tensor_add(dec[:, n0:n1], hp3, b3r[:, n0:n1])
                nc.scalar.activation(out=dec, in_=dec, func=AF.Sigmoid)
                nc.sync.dma_start(out=decd, in_=dec)

    nc.compile()
    return nc


# ---------------------------------------------------------------------------
# Host side
# ---------------------------------------------------------------------------


def prep_inputs(image, target, conv1_w, conv1_b, conv2_w, conv2_b, W,
                d1_w, d1_b, d2_w, d2_b, do_w, do_b):
    """Full inputs -> list of 8 per-core input maps."""
    image = np.asarray(image, np.float32)
    target = np.asarray(target)
    B = image.shape[0]
    per = B // N_CORES

    # host im2col for conv1: [B, 81, 400]
    img = image[:, :, :, 0]
    sw = np.lib.stride_tricks.sliding_window_view(img, (9, 9), axis=(1, 2))
    # sw: [B, 20, 20, 9, 9] -> [B, (kh kw), (oh ow)]
    pat = np.ascontiguousarray(sw.transpose(0, 3, 4, 1, 2)).reshape(B, 81, 400)

    w1 = np.asarray(conv1_w, np.float32).reshape(81, 256)
    b1 = np.zeros((128, 2), np.float32)
    b1[:, 0] = np.asarray(conv1_b, np.float32)[0:128]
    b1[:, 1] = np.asarray(conv1_b, np.float32)[128:256]
    w2 = (
        np.asarray(conv2_w, np.float32)
        .reshape(81, 2, 128, 2, 128)
        .transpose(0, 2, 1, 3, 4)  # tap, cin_low, cinh, couth, cout_low
        .reshape(27, 3, 128, 2, 2, 128)
        .transpose(0, 2, 1, 3, 4, 5)  # tg, cin_low, tap3, cinh, couth, cout
        .copy()
    )
    b2 = np.zeros((128, 2), np.float32)
    b2[:, 0] = np.asarray(conv2_b, np.float32)[0:128]
    b2[:, 1] = np.asarray(conv2_b, np.float32)[128:256]

    W0 = np.asarray(W, np.float32)[0]  # [1152, 10, 16, 8]
    wt = (
        W0.transpose(0, 3, 1, 2)  # i, k, j, d
        .reshape(6, 12, 128, 160)
        .transpose(0, 2, 1, 3)  # sc, ik_low(128), m, jd
        .copy()
    )
    wt2 = W0.transpose(1, 2, 0, 3).reshape(160, 9216)  # (j,d), (i,k)
    wt2a = wt2[0:128].astype(BF16)
    wt2b = wt2[128:160].astype(BF16)

    d1w = np.asarray(d1_w, np.float32)
    d2w = np.asarray(d2_w, np.float32).reshape(4, 128, 1024)
    d3w = np.asarray(do_w, np.float32).reshape(8, 128, 784)

    shared = dict(
        w1d=w1, b1d=b1, w2d=w2, b2d=b2, wtd=wt, wt2ad=wt2a, wt2bd=wt2b,
        d1wd=d1w, d2wd=d2w, d3wd=d3w,
        d1bd=np.asarray(d1_b, np.float32),
        d2bd=np.asarray(d2_b, np.float32),
        d3bd=np.asarray(do_b, np.float32),
    )
    maps = []
    for c in range(N_CORES):
        m = dict(shared)
        m["patd"] = pat[c * per : (c + 1) * per]
        m["tgt"] = target[c * per : (c + 1) * per].astype(np.int32).reshape(per, 1)
        maps.append(m)
    return maps


_NC_CACHE = {}


def _get_nc(debug=False):
    key = bool(debug)
    if key not in _NC_CACHE:
        _NC_CACHE[key] = build_nc(debug=key)
    return _NC_CACHE[key]


def kernel(**inputs):
    from concourse import bass_utils

    target = np.asarray(inputs["target"])
    maps = prep_inputs(**inputs)
    nc = _get_nc(debug=False)
    res = bass_utils.run_bass_kernel_spmd(nc, maps, core_ids=list(range(N_CORES)))
    outs = res.results
    B = N_CORES * B_LOC
    norm = np.concatenate([o["normd"] for o in outs], 0).reshape(B, 1, 10, 1, 1)
    ypred = np.concatenate([o["ypredd"] for o in outs], 0).reshape(B, 1)
    dec = np.concatenate([o["decd"] for o in outs], 0).reshape(B, 1, 784)
    if target.dtype == np.int64:
        ypred = ypred.astype(np.int64)
    else:
        ypred = ypred.astype(np.int32)
    return norm, ypred, dec
